# revision 44
# baseline (speedup 1.0000x reference)
"""Trainium2 Bass kernel for nn_Basic_Block_v1 (spatial/spectral Mamba2 block).

Sharding: data-parallel over batch (16 samples) across 8 NeuronCores,
2 samples per core; all parameters replicated. SSD scans are computed in
closed quadratic form on the TensorEngine. All heavy matmuls run in bf16
(1 cycle/row vs 4 for fp32); the cumulative-decay broadcast uses a bf16
hi/lo split to keep fp32-grade cancellation. LayerNorm scale/bias and the
gated-RMS weight are folded into adjacent projection weights on the host;
the Mamba D-residual is applied as a diagonal matmul accumulated into the
same PSUM as the SSD output.
"""
import sys
sys.path.insert(0, '/opt/trn_rl_repo')
import json

import numpy as np
import ml_dtypes

BF16NP = ml_dtypes.bfloat16

import concourse.bass as bass
import concourse.mybir as mybir
from concourse import tile
from concourse import bass_isa
from concourse.bass_utils import run_bass_kernel_spmd

F32 = mybir.dt.float32
BF = mybir.dt.bfloat16
I32 = mybir.dt.int32
AF = mybir.ActivationFunctionType
ALU = mybir.AluOpType
AX = mybir.AxisListType

NCORES = 8
BPC = 2          # batch per core
L = 256          # spatial tokens
C = 128          # channels
H1 = 4           # spa heads
H2 = 8           # spe heads
NST = 64         # d_state
EPS = 1e-5

# ---------------------------------------------------------------------------
# walrus in this container supports only ONE sync-wait per instruction;
# split extra waits emitted by the Tile scheduler onto preceding NoOps.
_WAIT_LIMIT = 1
_orig_to_json = bass.Bass.to_json_bytes


def _fix_block(b, ctr):
    insts = b.get('instructions')
    if insts:
        out = []
        for ins in insts:
            si = ins.get('sync_info')
            waits = (si or {}).get('on_wait') or []
            if len(waits) > _WAIT_LIMIT:
                while len(waits) > _WAIT_LIMIT:
                    chunk, waits = waits[:_WAIT_LIMIT], waits[_WAIT_LIMIT:]
                    ctr[0] += 1
                    out.append({
                        "debug": ins.get("debug"),
                        "engine": ins["engine"],
                        "ins": [],
                        "name": f"I-wsplit{ctr[0]}",
                        "opcode": "NoOp",
                        "outs": [],
                        "text_hint": "wsplit",
                        "sync_info": {"on_update": [], "on_wait": chunk},
                    })
                si['on_wait'] = waits
            out.append(ins)
        b['instructions'] = out
    for sb in b.get('blocks') or []:
        _fix_block(sb, ctr)


def _patched_to_json(self, *a, **k):
    raw = _orig_to_json(self, *a, **k)
    d = json.loads(raw)
    ctr = [0]
    for f in d.get('functions', []):
        for b in f.get('blocks', []):
            _fix_block(b, ctr)
    if ctr[0] == 0:
        return raw
    return json.dumps(d).encode()


bass.Bass.to_json_bytes = _patched_to_json


# ---------------------------------------------------------------------------
def _sincos_2d(dim, Hg):
    def e1(d, pos):
        omega = 1.0 / (10000.0 ** (np.arange(d // 2, dtype=np.float64) / (d / 2.0)))
        out = pos[:, None] * omega[None, :]
        return np.concatenate([np.sin(out), np.cos(out)], axis=-1)
    gh, gw = np.meshgrid(np.arange(Hg), np.arange(Hg), indexing='ij')
    emb = np.concatenate([e1(dim // 2, gh.reshape(-1)), e1(dim // 2, gw.reshape(-1))], axis=-1)
    return emb.astype(np.float32)


def host_constants():
    d = {}
    d['pe_fm'] = np.ascontiguousarray(_sincos_2d(C, 16).T).astype(BF16NP)   # [128, 256]
    d['ident'] = np.eye(128, dtype=np.float32).astype(BF16NP)
    d['ident32'] = np.eye(8, dtype=np.float32)
    iota = np.arange(L, dtype=np.float32)
    d['iotaC'] = np.stack([iota[:128], iota[128:]], axis=1).copy()          # [128, 2] f32
    sidx = np.arange(L)[:, None]
    tidx = np.arange(L)[None, :]
    m = (sidx <= tidx).astype(np.float32)
    d['maskT_spa'] = np.stack([m[:128], m[128:]], axis=1).copy().astype(BF16NP)
    s2 = np.arange(128)[:, None]
    t2 = np.arange(128)[None, :]
    d['maskT_spe'] = (s2 <= t2).astype(np.float32).astype(BF16NP)
    E1 = np.zeros((H1, 2, 128), np.float32)
    for j in range(2):
        for mm in range(128):
            E1[2 * j + mm // 64, j, mm] = 1.0
    d['E_spaJ'] = E1.astype(BF16NP)
    E2 = np.zeros((H2, 4, 128), np.float32)
    for j in range(4):
        for mm in range(128):
            E2[2 * j + mm // 64, j, mm] = 1.0
    d['E_speJ'] = E2.astype(BF16NP)
    EA = np.zeros((8, 128), np.float32)
    for h in range(8):
        EA[h, h * 16:(h + 1) * 16] = 1.0
    d['E_attn'] = EA.astype(BF16NP)
    d['Emask_q'] = EA.T.copy().astype(BF16NP)
    return d


def _col_order():
    cols = []
    for i in range(2):
        cols += [f"spa_dtb{i}", f"spa_negA{i}",
                 f"spa_cb{i}_0", f"spa_cb{i}_1", f"spa_cbBC{i}",
                 f"spa_zb{i}_0", f"spa_zb{i}_1", f"spa_xb{i}_0", f"spa_xb{i}_1",
                 f"spa_BCb{i}", f"spa_dpc{i}_0", f"spa_dpc{i}_1"]
    for i in range(2):
        cols += [f"spe_dtb{i}", f"spe_negA{i}"]
        cols += [f"spe_cb{i}_{j}" for j in range(4)] + [f"spe_cbBC{i}"]
        cols += [f"spe_zb{i}_{j}" for j in range(4)]
        cols += [f"spe_xb{i}_{j}" for j in range(4)]
        cols += [f"spe_BCb{i}"]
        cols += [f"spe_dpc{i}_{j}" for j in range(4)]
    cols += ["lnw_norm", "lnb_norm", "cprj_b", "aq_b", "ak_b", "av_b", "ao_b",
             "sq_b0", "sq_b1", "sk_b0", "sk_b1"]
    return cols


COL_ORDER = _col_order()
CIDX = {k: ix for ix, k in enumerate(COL_ORDER)}


def prep_weights(inp):
    """Host-side layout prep: bf16 casts, LN scale/bias folded into in_proj,
    rms weight folded into out_proj, D as diagonal matrices."""
    f32 = np.float32
    w = {}
    cols = {}
    # ---- spa in_proj with spa_ln fold ----
    w['spa_in_fold'] = np.zeros((2, 128, 644), BF16NP)
    for i in range(2):
        iw = np.asarray(inp['spa_in_w'][i], f32)                 # [644, 128]
        lw = np.asarray(inp['spa_ln_w'][i], f32)
        lb = np.asarray(inp['spa_ln_b'][i], f32)
        br = iw @ lb                                             # [644]
        w['spa_in_fold'][i] = (iw * lw[None, :]).T.astype(BF16NP)
        cols[f"spa_zb{i}_0"] = br[0:128]
        cols[f"spa_zb{i}_1"] = br[128:256]
        cols[f"spa_xb{i}_0"] = br[256:384]
        cols[f"spa_xb{i}_1"] = br[384:512]
        cols[f"spa_BCb{i}"] = br[512:640]
        cols[f"spa_dtb{i}"] = np.asarray(inp['spa_dt_bias'][i], f32) + br[640:644]
        cols[f"spa_negA{i}"] = -np.exp(np.asarray(inp['spa_A_log'][i], f32))
    cv = np.zeros((128, 2, 3, 4), f32)
    for i in range(2):
        cv[:, i, 0] = inp['spa_conv_w'][i, 0:128]
        cv[:, i, 1] = inp['spa_conv_w'][i, 128:256]
        cv[0:64, i, 2] = inp['spa_conv_w'][i, 256:320]
        cv[64:128, i, 2] = inp['spa_conv_w'][i, 320:384]
    w['spa_conv_pk'] = cv
    # out_proj with rms-weight fold: [feat, i, j, out]
    sow = np.transpose(np.asarray(inp['spa_out_w'], f32), (0, 2, 1)).reshape(2, 2, 128, 128)
    sow = sow * np.asarray(inp['spa_rms_w'], f32).reshape(2, 2, 128)[:, :, :, None]
    w['spa_out_pk'] = np.ascontiguousarray(sow.transpose(2, 0, 1, 3)).astype(BF16NP)
    # ---- spe in_proj with spe_ln fold ----
    w['spe_in_pk'] = np.zeros((2, 128, 2, 1160), BF16NP)
    for i in range(2):
        iw = np.asarray(inp['spe_in_w'][i], f32)                 # [1160, 256]
        lw = np.asarray(inp['spe_ln_w'][i], f32)
        lb = np.asarray(inp['spe_ln_b'][i], f32)
        br = iw @ lb
        iwf = (iw * lw[None, :]).T                               # [256, 1160]
        w['spe_in_pk'][i] = iwf.reshape(2, 128, 1160).transpose(1, 0, 2).astype(BF16NP)
        for j in range(4):
            cols[f"spe_zb{i}_{j}"] = br[j * 128:(j + 1) * 128]
            cols[f"spe_xb{i}_{j}"] = br[512 + j * 128:512 + (j + 1) * 128]
        cols[f"spe_BCb{i}"] = br[1024:1152]
        cols[f"spe_dtb{i}"] = np.asarray(inp['spe_dt_bias'][i], f32) + br[1152:1160]
        cols[f"spe_negA{i}"] = -np.exp(np.asarray(inp['spe_A_log'][i], f32))
    cv2 = np.zeros((128, 2, 5, 4), f32)
    for i in range(2):
        for j in range(4):
            cv2[:, i, j] = inp['spe_conv_w'][i, j * 128:(j + 1) * 128]
        cv2[0:64, i, 4] = inp['spe_conv_w'][i, 512:576]
        cv2[64:128, i, 4] = inp['spe_conv_w'][i, 576:640]
    w['spe_conv_pk'] = cv2
    sew = np.transpose(np.asarray(inp['spe_out_w'], f32), (0, 2, 1)).reshape(2, 4, 128, 256)
    sew = sew * np.asarray(inp['spe_rms_w'], f32).reshape(2, 4, 128)[:, :, :, None]
    w['spe_out_pk'] = np.ascontiguousarray(sew.transpose(0, 2, 1, 3)).astype(BF16NP)
    # ---- attention & tail ----
    w['cprj_pk'] = np.ascontiguousarray(
        np.transpose(np.asarray(inp['cprj_w'], f32), (2, 1, 0)).transpose(1, 0, 2)).astype(BF16NP)
    for nm in ('aq', 'ak', 'av', 'ao'):
        w[nm + 'T'] = np.ascontiguousarray(np.asarray(inp[nm + '_w'], f32).T).astype(BF16NP)
    for nm in ('sq', 'sk', 'sv', 'so'):
        wt_ = np.asarray(inp[nm + '_w'], f32).T.reshape(2, 128, 256)
        w[nm + 'T'] = np.ascontiguousarray(wt_.transpose(1, 0, 2)).astype(BF16NP)
    sqkb = np.zeros((128, 2, 2), f32)
    for ot in range(2):
        sqkb[:, 0, ot] = np.asarray(inp['sq_b'], f32)[ot * 128:(ot + 1) * 128]
        sqkb[:, 1, ot] = np.asarray(inp['sk_b'], f32)[ot * 128:(ot + 1) * 128]
    w['sqkb'] = sqkb
    w['svbB'] = np.ascontiguousarray(
        np.broadcast_to(np.asarray(inp['sv_b'], f32)[None, :], (128, 256))).astype(BF16NP)
    w['sobB'] = np.ascontiguousarray(
        np.broadcast_to(np.asarray(inp['so_b'], f32)[None, :], (128, 256))).astype(BF16NP)
    w['dsw_pk'] = np.ascontiguousarray(
        np.asarray(inp['ds_conv_w'], f32).reshape(9, 128, 128).transpose(1, 0, 2)).astype(BF16NP)
    w['ds_ln_wB'] = np.ascontiguousarray(
        np.broadcast_to(np.asarray(inp['ds_ln_w'], f32)[None, :], (64, 128)))
    w['ds_ln_bB'] = np.ascontiguousarray(
        np.broadcast_to(np.asarray(inp['ds_ln_b'], f32)[None, :], (64, 128)))
    # ---- small column-packed params (fp32 scalar operands) ----
    for i in range(2):
        cols[f"spa_cb{i}_0"] = inp['spa_conv_b'][i, 0:128]
        cols[f"spa_cb{i}_1"] = inp['spa_conv_b'][i, 128:256]
        cols[f"spa_cbBC{i}"] = inp['spa_conv_b'][i, 256:384]
        dpc = np.repeat(np.asarray(inp['spa_D'][i], f32), 64)
        cols[f"spa_dpc{i}_0"] = dpc[0:128]
        cols[f"spa_dpc{i}_1"] = dpc[128:256]
        dpc2 = np.repeat(np.asarray(inp['spe_D'][i], f32), 64)
        for j in range(4):
            cols[f"spe_dpc{i}_{j}"] = dpc2[j * 128:(j + 1) * 128]
        for j in range(4):
            cols[f"spe_cb{i}_{j}"] = inp['spe_conv_b'][i, j * 128:(j + 1) * 128]
        cols[f"spe_cbBC{i}"] = inp['spe_conv_b'][i, 512:640]
    cols["lnw_norm"] = inp['norm_w']
    cols["lnb_norm"] = inp['norm_b']
    cols["cprj_b"] = inp['cprj_b']
    for nm in ('aq', 'ak', 'av', 'ao'):
        cols[nm + "_b"] = inp[nm + '_b']
    cols["sq_b0"] = inp['sq_b'][0:128]
    cols["sq_b1"] = inp['sq_b'][128:256]
    cols["sk_b0"] = inp['sk_b'][0:128]
    cols["sk_b1"] = inp['sk_b'][128:256]
    pk = np.zeros((128, len(COL_ORDER)), f32)
    for k, v in cols.items():
        v = np.asarray(v, f32)
        pk[0:v.shape[0], CIDX[k]] = v
    w['colpak'] = pk
    return w


W_SHAPES = {
    'spa_in_fold': ([2, 128, 644], BF), 'spa_conv_pk': ([128, 2, 3, 4], F32),
    'spa_out_pk': ([128, 2, 2, 128], BF),
    'spe_in_pk': ([2, 128, 2, 1160], BF), 'spe_conv_pk': ([128, 2, 5, 4], F32),
    'spe_out_pk': ([2, 128, 4, 256], BF),
    'cprj_pk': ([128, 5, 128], BF),
    'aqT': ([128, 128], BF), 'akT': ([128, 128], BF), 'avT': ([128, 128], BF),
    'aoT': ([128, 128], BF),
    'sqT': ([128, 2, 256], BF), 'skT': ([128, 2, 256], BF), 'svT': ([128, 2, 256], BF),
    'soT': ([128, 2, 256], BF), 'svbB': ([128, 256], BF), 'sobB': ([128, 256], BF),
    'sqkb': ([128, 2, 2], F32),
    'dsw_pk': ([128, 9, 128], BF), 'ds_ln_wB': ([64, 128], F32), 'ds_ln_bB': ([64, 128], F32),
    'colpak': ([128, len(COL_ORDER)], F32),
}

CST_DT = {'pe_fm': BF, 'ident': BF, 'ident32': F32, 'iotaC': F32,
          'maskT_spa': BF, 'maskT_spe': BF, 'E_spaJ': BF, 'E_speJ': BF,
          'E_attn': BF, 'Emask_q': BF}

# ---- blob packing: all params as column ranges of two [128, N] blobs ----
BLOB_BF = [
    ('pe_fm', 128, [256]), ('ident', 128, [128]),
    ('maskT_spa', 128, [2, 256]), ('maskT_spe', 128, [128]),
    ('Emask_q', 128, [8]), ('E_spaJ', 4, [2, 128]), ('E_speJ', 8, [4, 128]),
    ('E_attn', 8, [128]),
    ('spa_in_fold', 128, [2, 644]),
    ('spa_out_pk', 128, [2, 2, 128]),
    ('spe_in_pk', 128, [2, 2, 1160]),
    ('spe_out_pk', 128, [2, 4, 256]),
    ('cprj_pk', 128, [5, 128]),
    ('aqT', 128, [128]), ('akT', 128, [128]), ('avT', 128, [128]),
    ('aoT', 128, [128]),
    ('sqT', 128, [2, 256]), ('skT', 128, [2, 256]), ('svT', 128, [2, 256]),
    ('soT', 128, [2, 256]), ('svbB', 128, [256]), ('sobB', 128, [256]),
    ('dsw_pk', 128, [9, 128]),
]
BLOB_F32 = [
    ('colpak', 128, [len(COL_ORDER)]),
    ('spa_conv_pk', 128, [2, 3, 4]), ('spe_conv_pk', 128, [2, 5, 4]),
    ('iotaC', 128, [2]), ('sqkb', 128, [2, 2]), ('ident32', 8, [8]),
    ('ds_ln_wB', 64, [128]), ('ds_ln_bB', 64, [128]),
]


def _blob_offsets(spec):
    offs = {}
    c = 0
    for name, _, vshape in spec:
        n = int(np.prod(vshape))
        offs[name] = (c, n)
        c += n
    return offs, c


BF_OFFS, BF_COLS = _blob_offsets(BLOB_BF)
F32_OFFS, F32_COLS = _blob_offsets(BLOB_F32)
_LAYER_MAJOR = {'spa_in_fold': (1, 0, 2), 'spe_in_pk': (1, 0, 2, 3),
                'spe_out_pk': (1, 0, 2, 3)}


def pack_blobs(cst, w):
    pool = dict(cst)
    pool.update(w)
    bf = np.zeros((128, BF_COLS), BF16NP)
    f32 = np.zeros((128, F32_COLS), np.float32)
    for spec, blob, offs in ((BLOB_BF, bf, BF_OFFS), (BLOB_F32, f32, F32_OFFS)):
        for name, rows, vshape in spec:
            a = np.asarray(pool[name])
            if name in _LAYER_MAJOR:
                a = np.transpose(a, _LAYER_MAJOR[name])
            off, n = offs[name]
            blob[0:rows, off:off + n] = a.reshape(rows, n)
    return bf, f32


# ---------------------------------------------------------------------------
def build_program(taps=()):
    nc = bass.Bass()

    def din(name, shape, dt=F32):
        return nc.dram_tensor(name, shape, dt, kind="ExternalInput")

    x2 = din("x2", [BPC, C, L], BF)
    idx = din("idx", [BPC, L], BF)
    inv = din("inv", [BPC, L], BF)

    blob_bf_t = din("blob_bf", [128, BF_COLS], BF)
    blob_f32_t = din("blob_f32", [128, F32_COLS], F32)

    out = nc.dram_tensor("out", [BPC, 8, 8, C], F32, kind="ExternalOutput")
    tap_t = {}

    with tile.TileContext(nc) as tc:
        import contextlib
        stk = contextlib.ExitStack()
        sb = stk.enter_context(tc.tile_pool(name="sb", bufs=1))
        ps1 = stk.enter_context(tc.tile_pool(name="ps1", bufs=3, space="PSUM"))
        ps2 = stk.enter_context(tc.tile_pool(name="ps2", bufs=4, space="PSUM"))
        psD = stk.enter_context(tc.tile_pool(name="psD", bufs=1, space="PSUM"))

        class _PSShim:
            def tile(self, shape, dt, tag="small", name="small"):
                return ps2.tile(shape, dt, tag="b256", name="ps_sm")

        psS = _PSShim()

        BUFS2 = {"cv_a0", "cv_a1", "rowA", "rowB", "rowC", "tm_tmp", "ssd_Dt",
                 "ssd_Et", "ssd_MT", "spa_xtm", "spe_xtm", "sq_tmp", "x2f_tmp",
                 "ssd_m0m", "spa_acumT", "spe_acumT", "spa_ygt", "spa_y0t",
                 "spa_ynt", "spe_ygt", "spe_y0t", "spe_ynt", "ds_cmp",
                 "spe_xn", "spe_h2sb", "sp2_a2T", "sp2_o2", "perm_oh", "sp2_t3",
                 "ds_xrp", "xc_0", "xc_1", "xc_2", "xc_3", "xc_BC", "xc_C",
                 "cv_x0", "cv_x1", "cv_x2", "cv_x3", "cv_BC", "aflat", "cv_g0",
                 "cv_g1"}

        def T(shape, tag, dt=F32):
            return sb.tile(shape, dt, tag=tag, name=tag,
                           bufs=2 if tag in BUFS2 else 1)

        def TB(shape, tag):
            return T(shape, tag, BF)

        def P512(tag="b512"):
            return ps1.tile([128, 512], F32, tag="b512", name="b512")

        def P256(tag="b256"):
            return ps2.tile([128, 256], F32, tag="b256", name="b256")

        def PT(tag="bT"):
            return ps2.tile([128, 256], BF, tag="b256", name="bT")

        def tap(name, ap_fn):
            if name in taps:
                shape, writer, dt = ap_fn()
                t = nc.dram_tensor("t_" + name, shape, dt, kind="ExternalOutput")
                tap_t[name] = t
                writer(t)

        dma = nc.sync.dma_start
        V = nc.vector
        S = nc.scalar
        G = nc.gpsimd
        RO = bass_isa.ReduceOp

        # ---------- inputs first, then all params via two blobs ----------
        xb = TB([128, BPC, L], "xb")
        for s in range(BPC):
            dma(xb[:, s, :], x2[s])
        idxf = TB([1, BPC, L], "irow_f")
        dma(idxf[:], idx[None, :, :])
        blob_f32 = T([128, F32_COLS], "blob_f32")
        dma(blob_f32[:], blob_f32_t[:])
        blob_bf = TB([128, BF_COLS], "blob_bf")
        CH = 4096
        for c0 in range(0, BF_COLS, CH):
            c1 = min(c0 + CH, BF_COLS)
            dma(blob_bf[:, c0:c1], blob_bf_t[:, c0:c1])

        def _view(blob, offs, name, rows, vshape):
            off, n = offs[name]
            ap = blob[0:rows, off:off + n]
            if len(vshape) == 2:
                ap = ap.rearrange("p (a b) -> p a b", a=vshape[0])
            elif len(vshape) == 3:
                ap = ap.rearrange("p (a b c) -> p a b c", a=vshape[0], b=vshape[1])
            return ap

        ct = {}
        wt = {}
        for name, rows, vshape in BLOB_BF:
            v = _view(blob_bf, BF_OFFS, name, rows, vshape)
            (ct if name in CST_DT else wt)[name] = v
        for name, rows, vshape in BLOB_F32:
            v = _view(blob_f32, F32_OFFS, name, rows, vshape)
            (ct if name in CST_DT else wt)[name] = v
        colpak = wt['colpak']
        inw_l = [wt['spa_in_fold'][:, i, :] for i in range(2)]
        inw2_l = [wt['spe_in_pk'][:, i, :, :] for i in range(2)]
        ow2_l = [wt['spe_out_pk'][:, i, :, :] for i in range(2)]

        def col(key, p=128):
            return colpak[0:p, CIDX[key]:CIDX[key] + 1]

        ones4 = TB([128, 128], "ones4")
        V.memset(ones4[:], 1.0)
        epscol = T([128, 1], "epscol")
        V.memset(epscol[:], EPS)
        onescol = ones4[:, 0:1]       # [128,1] bf16
        onesrow1 = ones4[0:1, :]      # [1,128] bf16
        ones2 = ones4[0:2, :]         # [2,128] bf16
        ident = ct['ident']
        ident32 = ct['ident32']

        # ---------- stage 0: embed + permute ----------
        x0 = TB([128, BPC, L], "x0")
        V.tensor_tensor(
            x0[:], xb[:],
            ct['pe_fm'][:].unsqueeze(1).to_broadcast((128, BPC, L)),
            op=ALU.add)

        xs = TB([128, BPC, L], "xs")
        for s in range(BPC):
            idxB = P512()
            nc.tensor.matmul(idxB[:, 0:L], onesrow1, idxf[:, s, :], start=True, stop=True)
            PmT = TB([128, 2, L], "perm_oh")
            for st in range(2):
                V.tensor_scalar(PmT[:, st, :], idxB[:, 0:L], ct['iotaC'][:, st:st + 1],
                                None, op0=ALU.is_equal)
            x0tm = TB([128, 2, 128], "tm_tmp")
            for tt in range(2):
                ptr = PT()
                nc.tensor.transpose(ptr[:, 0:128], x0[:, s, tt * 128:(tt + 1) * 128], ident[:])
                S.copy(x0tm[:, tt, :], ptr[:, 0:128])
            pxs = P256()
            for st in range(2):
                nc.tensor.matmul(pxs[:], x0tm[:, st, :], PmT[:, st, :],
                                 start=(st == 0), stop=(st == 1))
            S.copy(xs[:, s, :], pxs[:])

        def tap_batched(t_sb, shape_per_s, dt=BF):
            def writer(dram):
                for s in range(BPC):
                    dma(dram[s], t_sb[:, s, :])
            return ([BPC] + shape_per_s, writer, dt)

        tap("xs0", lambda: tap_batched(xs, [128, L]))

        # ================= shared helpers =================
        def part_ln(xflat, final=False):
            """LayerNorm over the channel (partition) dim of [128, 512] bf16.
            Non-final: scale/bias folded downstream -> returns (x-mu)*rstd."""
            sq = TB([128, 512], "sq_tmp")
            S.activation(sq[:], xflat, AF.Square)
            msum = psS.tile([1, 512], F32)
            nc.tensor.matmul(msum[:], onescol, xflat, start=True, stop=True)
            ssum = psS.tile([1, 512], F32)
            nc.tensor.matmul(ssum[:], onescol, sq[:], start=True, stop=True)
            mu2 = T([1, 512], "rowA")
            S.activation(mu2[:], msum[:], AF.Square, scale=1.0 / 128)
            var = T([1, 512], "rowB")
            V.scalar_tensor_tensor(var[:], ssum[:], 1.0 / 128, mu2[:],
                                   op0=ALU.mult, op1=ALU.subtract)
            lnv = T([1, 512], "rowA")
            S.activation(lnv[:], var[:], AF.Ln, bias=epscol[0:1, 0:1])
            rstd = TB([1, 512], "ln_rstd")
            S.activation(rstd[:], lnv[:], AF.Exp, scale=-0.5)
            r0 = TB([1, 512], "ln_r0")
            V.scalar_tensor_tensor(r0[:], msum[:], -1.0 / 128, rstd[:],
                                   op0=ALU.mult, op1=ALU.mult)
            rstdB = P512()
            nc.tensor.matmul(rstdB[:], onesrow1, rstd[:], start=True, stop=True)
            r0B = P512()
            nc.tensor.matmul(r0B[:], onesrow1, r0[:], start=True, stop=True)
            tmp = TB([128, 512], "ln_tmp")
            V.tensor_tensor(tmp[:], xflat, rstdB[:], op=ALU.mult)
            xln = TB([128, 512], "ln_out")
            if final:
                xn = T([128, 512], "ln_xn")
                V.tensor_tensor(xn[:], tmp[:], r0B[:], op=ALU.add)
                S.activation(xln[:], xn[:], AF.Identity, bias=col("lnb_norm"),
                             scale=col("lnw_norm"))
            else:
                V.tensor_tensor(xln[:], tmp[:], r0B[:], op=ALU.add)
            return xln

        def convchain(buf, wc, cb, P, W, tag, E=None):
            """Causal depthwise conv (k=4) + silu. buf [P, 2, W+3] fp32 ->
            bf16 output. E selects the elementwise engine (vector/gpsimd)."""
            E = E or V
            a0 = T([P, 2, W], "cv_a0" if E is V else "cv_g0")
            E.tensor_scalar(a0[:], buf[:, :, 0:W], wc[:, 0:1], None, op0=ALU.mult)
            a1 = T([P, 2, W], "cv_a1" if E is V else "cv_g1")
            E.scalar_tensor_tensor(a1[:], buf[:, :, 1:W + 1], wc[:, 1:2], a0[:],
                                   op0=ALU.mult, op1=ALU.add)
            a2 = T([P, 2, W], "cv_a0" if E is V else "cv_g0")
            E.scalar_tensor_tensor(a2[:], buf[:, :, 2:W + 2], wc[:, 2:3], a1[:],
                                   op0=ALU.mult, op1=ALU.add)
            a3 = T([P, 2, W], "cv_a1" if E is V else "cv_g1")
            E.scalar_tensor_tensor(a3[:], buf[:, :, 3:W + 3], wc[:, 3:4], a2[:],
                                   op0=ALU.mult, op1=ALU.add)
            xc = TB([P, 2, W], tag)
            S.activation(xc[:], a3[:], AF.Silu, bias=cb[:, 0:1])
            return xc

        def dt_ladder(pdt, nh, NW, dtb_key, negA_key):
            """softplus(dt+bias) -> dtv_bf (matmul operand), acum f32,
            hi/lo bf16 rows for the decay broadcast."""
            e1 = T([nh, NW], "rowA")
            S.activation(e1[:], pdt[:], AF.Exp, bias=col(dtb_key, nh))
            e1p = T([nh, NW], "rowB")
            V.tensor_scalar(e1p[:], e1[:], 1.0, None, op0=ALU.add)
            dtv = T([nh, NW], "mb_dtv")
            S.activation(dtv[:], e1p[:], AF.Ln)

            dtv_bf = TB([nh, NW], "mb_dtvbf")
            S.copy(dtv_bf[:], dtv[:])
            dtA = T([nh, NW], "rowA")
            V.tensor_scalar(dtA[:], dtv[:], col(negA_key, nh), None, op0=ALU.mult)
            acum = T([nh, NW], "mb_acum")
            seg = NW // BPC
            for s in range(BPC):
                V.tensor_tensor_scan(acum[:, s * seg:(s + 1) * seg],
                                     dtA[:, s * seg:(s + 1) * seg],
                                     dtA[:, s * seg:(s + 1) * seg], 0.0,
                                     op0=ALU.add, op1=ALU.bypass)
            hi = TB([nh, NW], "acum_hi")
            S.copy(hi[:], acum[:])
            lo = TB([nh, NW], "acum_lo")
            G.tensor_tensor(lo[:], acum[:], hi[:], op=ALU.subtract)
            hilo = TB([2, BPC, 1024], "aflat")
            for s in range(BPC):
                dma(hilo[0:1, s, :].rearrange("o (p f) -> o p f", p=nh),
                    hi[:, s * seg:(s + 1) * seg])
                dma(hilo[1:2, s, :].rearrange("o (p f) -> o p f", p=nh),
                    lo[:, s * seg:(s + 1) * seg])
            return dtv_bf, acum, hilo

        # ================= spa mamba =================
        def spa_mamba(i, xs):
            xflat = xs[:].rearrange("p s t -> p (s t)")
            xln = part_ln(xflat)
            tap(f"xln{i}", lambda: ([128, 512], lambda d: dma(d[:], xln[:]), BF))
            inw = inw_l[i][:]
            # dt first: its Exp/Ln then run before the silu cluster
            pdt = psS.tile([4, 512], F32)
            nc.tensor.matmul(pdt[:], inw[:, 640:644], xln[:], start=True, stop=True)
            dtv_bf, acum, hilo = dt_ladder(pdt, 4, 512, f"spa_dtb{i}", f"spa_negA{i}")
            cvx = []
            for j in range(2):
                px = P512()
                nc.tensor.matmul(px[:], inw[:, 256 + j * 128:256 + (j + 1) * 128], xln[:],
                                 start=True, stop=True)
                buf = T([128, 2, 259], f"cv_x{j}")
                G.memset(buf[:, :, 0:3], 0.0)
                S.activation(buf[:, :, 3:259], px[:].rearrange("p (s t) -> p s t", s=2),
                             AF.Identity, bias=col(f"spa_xb{i}_{j}"))
                cvx.append(buf)
            # B and C merged: one matmul, one buf, one chain (B rows 0:63, C 64:127)
            pbc = P512()
            nc.tensor.matmul(pbc[:], inw[:, 512:640], xln[:], start=True, stop=True)
            bufBC = T([128, 2, 259], "cv_BC")
            G.memset(bufBC[:, :, 0:3], 0.0)
            S.activation(bufBC[:, :, 3:259], pbc[:].rearrange("p (s t) -> p s t", s=2),
                         AF.Identity, bias=col(f"spa_BCb{i}"))
            # conv + silu (silu table region)
            xc = []
            for j in range(2):
                xc.append(convchain(cvx[j], wt['spa_conv_pk'][:, i, j, :],
                                    col(f"spa_cb{i}_{j}"), 128, 256, f"xc_{j}"))
            xcBC = convchain(bufBC, wt['spa_conv_pk'][:, i, 2, :],
                             col(f"spa_cbBC{i}"), 128, 256, "xc_BC")
            # z -> silu emitted after convs so dt's Ln precedes the silu cluster
            zsil = TB([128, 2, 512], "mb_zsil")
            for j in range(2):
                pz = P512()
                nc.tensor.matmul(pz[:], inw[:, j * 128:(j + 1) * 128], xln[:],
                                 start=True, stop=True)
                S.activation(zsil[:, j, :], pz[:], AF.Silu, bias=col(f"spa_zb{i}_{j}"))
            # C half to a partition-0-based tile (matmul needs equal base partitions)
            xcC = TB([64, 2, 256], "xc_C")
            dma(xcC[:], xcBC[64:128, :, :])
            if i == 0:
                tap("dbg_zsil", lambda: ([128, 1024], lambda d: dma(
                    d[:], zsil[:].rearrange("p j t -> p (j t)")), BF))
                tap("dbg_xc0", lambda: ([128, 512], lambda d: dma(
                    d[:], xc[0][:].rearrange("p s t -> p (s t)")), BF))
                tap("dbg_xcBC", lambda: ([128, 512], lambda d: dma(
                    d[:], xcBC[:].rearrange("p s t -> p (s t)")), BF))
                tap("dbg_xcC", lambda: ([64, 512], lambda d: dma(
                    d[:], xcC[:].rearrange("p s t -> p (s t)")), BF))
                tap("dbg_dtv", lambda: ([4, 512], lambda d: dma(d[:], dtv_bf[:]), BF))
                tap("dbg_acum", lambda: ([4, 512], lambda d: dma(d[:], acum[:]), F32))
            # dt-scaled x (feature-major)
            xp = TB([128, 2, 512], "mb_xp")
            for j in range(2):
                pdb = P512()
                nc.tensor.matmul(pdb[:], ct['E_spaJ'][:, j, :], dtv_bf[:], start=True, stop=True)
                V.tensor_tensor(xp[:, j, :], xc[j][:].rearrange("p s t -> p (s t)"), pdb[:],
                                op=ALU.mult)
            if i == 0:
                tap("dbg_xp", lambda: ([128, 1024], lambda d: dma(
                    d[:], xp[:].rearrange("p j t -> p (j t)")), BF))
            h1 = TB([128, 2, 256], "h1")
            for s in range(BPC):
                xtm = TB([128, 2, 256], "spa_xtm")
                for st in range(2):
                    for j in range(2):
                        ptr = PT()
                        nc.tensor.transpose(
                            ptr[:, 0:128],
                            xp[:, j, s * 256 + st * 128: s * 256 + (st + 1) * 128],
                            ident[:])
                        V.tensor_copy(xtm[:, st, j * 128:(j + 1) * 128], ptr[:, 0:128])
                m0m = TB([128, 2, 256], "ssd_m0m")
                for st in range(2):
                    pm0 = P256()
                    nc.tensor.matmul(pm0[:], xcBC[0:64, s, st * 128:(st + 1) * 128],
                                     xcC[:, s, :], start=True, stop=True)
                    V.tensor_tensor(m0m[:, st, :], pm0[:], ct['maskT_spa'][:, st, :],
                                    op=ALU.mult)
                acumT = T([128, 2, 4], "spa_acumT")
                for tt in range(2):
                    ptr2 = P256()
                    nc.tensor.transpose(ptr2[:, 0:4],
                                        acum[:, s * 256 + tt * 128: s * 256 + (tt + 1) * 128],
                                        ident32[0:4, 0:4])
                    S.copy(acumT[:, tt, :], ptr2[:, 0:4])
                pb1 = P512()
                nc.tensor.matmul(pb1[:], ones2, hilo[:, s, 0:512], start=True, stop=True)
                pb2 = P512()
                nc.tensor.matmul(pb2[:], ones2, hilo[:, s, 512:1024], start=True, stop=True)
                yps = P512()
                for st in range(2):
                    Dt = T([128, 4, 256], "ssd_Dt")
                    for h in range(H1):
                        pbx = pb1 if h < 2 else pb2
                        V.tensor_scalar(Dt[:, h, :],
                                        pbx[:, (h % 2) * 256:(h % 2 + 1) * 256],
                                        acumT[:, st, h:h + 1], 0.0,
                                        op0=ALU.subtract, op1=ALU.min)
                    Et = TB([128, 4, 256], "ssd_Et")
                    S.activation(Et[:].rearrange("p h t -> p (h t)"),
                                 Dt[:].rearrange("p h t -> p (h t)"), AF.Exp)
                    MT = TB([128, 4, 256], "ssd_MT")
                    V.tensor_tensor(MT[:], Et[:],
                                    m0m[:, st, :].unsqueeze(1).to_broadcast((128, 4, 256)),
                                    op=ALU.mult)
                    for h in range(H1):
                        nc.tensor.matmul(
                            yps[(h % 2) * 64:(h % 2) * 64 + 64,
                                (h // 2) * 256:(h // 2) * 256 + 256],
                            xtm[:, st, h * 64:(h + 1) * 64],
                            MT[:, h, :],
                            start=(st == 0), stop=(st == 1),
                            tile_position=(0, (h % 2) * 64),
                            skip_group_check=True)
                if i == 0 and s == 0:
                    tap("dbg_xtm", lambda: ([128, 512], lambda d: dma(
                        d[:], xtm[:].rearrange("p s t -> p (s t)")), BF))
                    tap("dbg_m0m", lambda: ([128, 512], lambda d: dma(
                        d[:], m0m[:].rearrange("p s t -> p (s t)")), BF))
                    if "dbg_yps" in taps:
                        ypc = T([128, 512], "dbg_ypc")
                        S.copy(ypc[:], yps[:])
                        tap("dbg_yps", lambda: ([128, 512], lambda d: dma(
                            d[:], ypc[:]), F32))
                y0t = TB([128, 2, 256], "spa_y0t")
                for j in range(2):
                    V.scalar_tensor_tensor(y0t[:, j, :], xc[j][:, s, :],
                                           col(f"spa_dpc{i}_{j}"),
                                           yps[:, j * 256:(j + 1) * 256],
                                           op0=ALU.mult, op1=ALU.add)
                ygt = TB([128, 2, 256], "spa_ygt")
                V.tensor_tensor(ygt[:], y0t[:],
                                zsil[:, :, s * 256:(s + 1) * 256], op=ALU.mult)
                if i == 0 and s == 0:
                    tap("dbg_ygt", lambda: ([128, 512], lambda d: dma(
                        d[:], ygt[:].rearrange("p j t -> p (j t)")), BF))
                sqy = TB([128, 2, 256], "sq_tmp")
                S.activation(sqy[:].rearrange("p j t -> p (j t)"),
                             ygt[:].rearrange("p j t -> p (j t)"), AF.Square)
                ssy = psS.tile([1, 256], F32)
                for j in range(2):
                    nc.tensor.matmul(ssy[:], onescol, sqy[:, j, :],
                                     start=(j == 0), stop=(j == 1))
                rl = T([1, 256], "rowA")
                S.activation(rl[:], ssy[:], AF.Ln, bias=epscol[0:1, 0:1],
                             scale=1.0 / 256)
                rrow = TB([1, 256], "rowC")
                S.activation(rrow[:], rl[:], AF.Exp, scale=-0.5)
                rB = P256()
                nc.tensor.matmul(rB[:], onesrow1, rrow[:], start=True, stop=True)
                ynt = TB([128, 2, 256], "spa_ynt")
                V.tensor_tensor(ynt[:], ygt[:],
                                rB[:].unsqueeze(1).to_broadcast((128, 2, 256)),
                                op=ALU.mult)
                pop = P256()
                for j in range(2):
                    nc.tensor.matmul(pop[:], wt['spa_out_pk'][:, i, j, :], ynt[:, j, :],
                                     start=(j == 0), stop=(j == 1))
                V.tensor_tensor(h1[:, s, :], pop[:], xs[:, s, :], op=ALU.add)
            return h1

        # ================= spe mamba =================
        def spe_mamba(i, h1):
            sqd = TB([128, 2, 256], "sq_tmp")
            mus = T([128, 2], "spe_mus")
            ss2 = T([128, 2], "spe_ss2")
            mean = T([128, 2], "spe_mean")
            m2 = T([128, 2], "spe_m2")
            var2 = T([128, 2], "spe_var")
            l2t = T([128, 2], "spe_l2")
            rstd2 = T([128, 2], "spe_rstd")
            X2f = TB([128, 2, 2, 128], "x2f_tmp")
            for s in range(BPC):
                # per-sample stats: sample-0 prep overlaps spa's sample-1 tail
                V.tensor_reduce(mus[:, s:s + 1], h1[:, s, :], axis=AX.X, op=ALU.add)
                S.activation(sqd[:, s, :], h1[:, s, :], AF.Square,
                             accum_out=ss2[:, s:s + 1])
                V.tensor_scalar(mean[:, s:s + 1], mus[:, s:s + 1], 1.0 / 256, None,
                                op0=ALU.mult)
                S.activation(m2[:, s:s + 1], mean[:, s:s + 1], AF.Square)
                V.scalar_tensor_tensor(var2[:, s:s + 1], ss2[:, s:s + 1], 1.0 / 256,
                                       m2[:, s:s + 1], op0=ALU.mult, op1=ALU.subtract)
                S.activation(l2t[:, s:s + 1], var2[:, s:s + 1], AF.Ln,
                             bias=epscol[:, 0:1])
                S.activation(rstd2[:, s:s + 1], l2t[:, s:s + 1], AF.Exp, scale=-0.5)
                xn = TB([128, 256], "spe_xn")
                V.tensor_scalar(xn[:], h1[:, s, :], mean[:, s:s + 1], rstd2[:, s:s + 1],
                                op0=ALU.subtract, op1=ALU.mult)
                for ft in range(2):
                    ptr = PT()
                    nc.tensor.transpose(ptr[:, 0:128], xn[:, ft * 128:(ft + 1) * 128],
                                        ident[:])
                    V.tensor_copy(X2f[:, s, ft, :], ptr[:, 0:128])
            inw2 = inw2_l[i][:]
            ow2 = ow2_l[i][:]

            def mm2(out_ap, off, width):
                for k in range(2):
                    nc.tensor.matmul(out_ap,
                                     inw2[:, k, off:off + width],
                                     X2f[:, :, k, :],
                                     start=(k == 0), stop=(k == 1))
            # dt first (exp/ln before the silu cluster)
            pdt = psS.tile([8, 256], F32)
            mm2(pdt[:], 1152, 8)
            dtv_bf, acum, hilo = dt_ladder(pdt, 8, 256, f"spe_dtb{i}", f"spe_negA{i}")
            cvx2 = []
            for j in range(4):
                px = P256()
                mm2(px[:], 512 + j * 128, 128)
                buf = T([128, 2, 131], f"cv_x{j}")
                G.memset(buf[:, :, 0:3], 0.0)
                S.activation(buf[:, :, 3:131], px[:].rearrange("p (s t) -> p s t", s=2),
                             AF.Identity, bias=col(f"spe_xb{i}_{j}"))
                cvx2.append(buf)
            pbc = P256()
            mm2(pbc[:], 1024, 128)
            bufBC = T([128, 2, 131], "cv_BC")
            G.memset(bufBC[:, :, 0:3], 0.0)
            S.activation(bufBC[:, :, 3:131], pbc[:].rearrange("p (s t) -> p s t", s=2),
                         AF.Identity, bias=col(f"spe_BCb{i}"))
            xc2 = []
            for j in range(4):
                xc2.append(convchain(cvx2[j], wt['spe_conv_pk'][:, i, j, :],
                                     col(f"spe_cb{i}_{j}"), 128, 128, f"xc_{j}"))
            xcBC = convchain(bufBC, wt['spe_conv_pk'][:, i, 4, :],
                             col(f"spe_cbBC{i}"), 128, 128, "xc_BC")
            z2sil = TB([128, 4, 256], "mb_zsil")
            for j in range(4):
                pz = P256()
                mm2(pz[:], j * 128, 128)
                S.activation(z2sil[:, j, :], pz[:], AF.Silu, bias=col(f"spe_zb{i}_{j}"))
            xcC = TB([64, 2, 128], "xc_C")
            dma(xcC[:], xcBC[64:128, :, :])
            xp2 = TB([128, 4, 256], "mb_xp")
            for j in range(4):
                pdb = P256()
                nc.tensor.matmul(pdb[:], ct['E_speJ'][:, j, :], dtv_bf[:], start=True, stop=True)
                V.tensor_tensor(xp2[:, j, :], xc2[j][:].rearrange("p s t -> p (s t)"), pdb[:],
                                op=ALU.mult)
            xs_new = TB([128, 2, 256], "xs")
            for s in range(BPC):
                xtm2 = TB([128, 512], "spe_xtm")
                for j in range(4):
                    ptr = PT()
                    nc.tensor.transpose(ptr[:, 0:128],
                                        xp2[:, j, s * 128:(s + 1) * 128], ident[:])
                    V.tensor_copy(xtm2[:, j * 128:(j + 1) * 128], ptr[:, 0:128])
                m0m2 = TB([128, 128], "ssd_m0m")
                pm0 = P256()
                nc.tensor.matmul(pm0[:, 0:128], xcBC[0:64, s, :], xcC[:, s, :],
                                 start=True, stop=True)
                V.tensor_tensor(m0m2[:], pm0[:, 0:128], ct['maskT_spe'][:], op=ALU.mult)
                acumT = T([128, 8], "spe_acumT")
                ptr2 = P256()
                nc.tensor.transpose(ptr2[:, 0:8], acum[:, s * 128:(s + 1) * 128],
                                    ident32[0:8, 0:8])
                S.copy(acumT[:], ptr2[:, 0:8])
                pb1 = P512()
                nc.tensor.matmul(pb1[:], ones2, hilo[:, s, 0:512], start=True, stop=True)
                pb2 = P512()
                nc.tensor.matmul(pb2[:], ones2, hilo[:, s, 512:1024], start=True, stop=True)
                yps = P512()
                Dt = T([128, 8, 128], "ssd_Dt")
                for h in range(H2):
                    pbx = pb1 if h < 4 else pb2
                    V.tensor_scalar(Dt[:, h, :],
                                    pbx[:, (h % 4) * 128:(h % 4 + 1) * 128],
                                    acumT[:, h:h + 1], 0.0,
                                    op0=ALU.subtract, op1=ALU.min)
                Et = TB([128, 8, 128], "ssd_Et")
                S.activation(Et[:].rearrange("p h t -> p (h t)"),
                             Dt[:].rearrange("p h t -> p (h t)"), AF.Exp)
                MT = TB([128, 8, 128], "ssd_MT")
                V.tensor_tensor(MT[:], Et[:],
                                m0m2[:].unsqueeze(1).to_broadcast((128, 8, 128)),
                                op=ALU.mult)
                for j in range(4):
                    for hh in range(2):
                        h = 2 * j + hh
                        nc.tensor.matmul(yps[hh * 64:hh * 64 + 64, j * 128:(j + 1) * 128],
                                         xtm2[:, h * 64:(h + 1) * 64],
                                         MT[:, h, :], start=True, stop=True,
                                         tile_position=(0, hh * 64),
                                         skip_group_check=True)
                y0t2 = TB([128, 4, 128], "spe_y0t")
                for j in range(4):
                    V.scalar_tensor_tensor(y0t2[:, j, :], xc2[j][:, s, :],
                                           col(f"spe_dpc{i}_{j}"),
                                           yps[:, j * 128:(j + 1) * 128],
                                           op0=ALU.mult, op1=ALU.add)
                ygt2 = TB([128, 4, 128], "spe_ygt")
                V.tensor_tensor(ygt2[:], y0t2[:],
                                z2sil[:, :, s * 128:(s + 1) * 128], op=ALU.mult)
                sqy = TB([128, 4, 128], "sq_tmp")
                S.activation(sqy[:].rearrange("p j t -> p (j t)"),
                             ygt2[:].rearrange("p j t -> p (j t)"), AF.Square)
                ssy = psS.tile([1, 128], F32)
                for j in range(4):
                    nc.tensor.matmul(ssy[:], onescol, sqy[:, j, :],
                                     start=(j == 0), stop=(j == 3))
                rl = T([1, 128], "rowA")
                S.activation(rl[:], ssy[:], AF.Ln, bias=epscol[0:1, 0:1],
                             scale=1.0 / 512)
                rrow = TB([1, 128], "rowC")
                S.activation(rrow[:], rl[:], AF.Exp, scale=-0.5)
                rB = P256()
                nc.tensor.matmul(rB[:, 0:128], onesrow1, rrow[:], start=True, stop=True)
                ynt = TB([128, 4, 128], "spe_ynt")
                V.tensor_tensor(ynt[:], ygt2[:],
                                rB[:, 0:128].unsqueeze(1).to_broadcast((128, 4, 128)),
                                op=ALU.mult)
                for ft in range(2):
                    ph2 = P256()
                    for k in range(4):
                        nc.tensor.matmul(ph2[:, 0:128],
                                         ow2[:, k, ft * 128:(ft + 1) * 128],
                                         ynt[:, k, :], start=(k == 0), stop=(k == 3))
                    h2sb = TB([128, 128], "spe_h2sb")
                    S.copy(h2sb[:], ph2[:, 0:128])
                    ptr = PT()
                    nc.tensor.transpose(ptr[:, 0:128], h2sb[:], ident[:])
                    V.tensor_tensor(xs_new[:, s, ft * 128:(ft + 1) * 128], ptr[:, 0:128],
                                    h1[:, s, ft * 128:(ft + 1) * 128], op=ALU.add)
            return xs_new

        # ================= layers =================
        cur = xs
        for i in range(2):
            h1 = spa_mamba(i, cur)
            tap(f"h1_{i}", lambda: tap_batched(h1, [128, L]))
            cur = spe_mamba(i, h1)
            tap(f"xsl{i + 1}", lambda: tap_batched(cur, [128, L]))

        # ================= final LN =================
        xfl = part_ln(cur[:].rearrange("p s t -> p (s t)"), final=True)
        xf = xfl[:].rearrange("p (s t) -> p s t", s=BPC)
        tap("xf", lambda: ([BPC, 128, L],
                           lambda d: [dma(d[s], xf[:, s, :]) for s in range(BPC)], BF))

        # ================= spa attention (center query) =================
        pctr = psS.tile([128, 2], F32)
        for l in range(5):
            nc.tensor.matmul(pctr[:], wt['cprj_pk'][:, l, :], xf[:, :, l],
                             start=(l == 0), stop=(l == 4))
        ctr = TB([128, 2], "at_ctr")
        S.activation(ctr[:], pctr[:], AF.Identity, bias=col("cprj_b"))
        pq = psS.tile([128, 2], F32)
        nc.tensor.matmul(pq[:], wt['aqT'][:], ctr[:], start=True, stop=True)
        qsb = TB([128, 2], "at_q")
        S.activation(qsb[:], pq[:], AF.Identity, bias=col("aq_b"))
        pk = P512()
        nc.tensor.matmul(pk[:], wt['akT'][:], xfl[:], start=True, stop=True)
        Ksb = TB([128, 2, 256], "at_K")
        S.activation(Ksb[:].rearrange("p s t -> p (s t)"), pk[:], AF.Identity,
                     bias=col("ak_b"))
        pv = P512()
        nc.tensor.matmul(pv[:], wt['avT'][:], xfl[:], start=True, stop=True)
        Vsb = TB([128, 2, 256], "at_V")
        S.activation(Vsb[:].rearrange("p s t -> p (s t)"), pv[:], AF.Identity,
                     bias=col("av_b"))
        # batched softmax over both samples (per-head global max is a valid
        # stabilizer; softmax itself stays per-(head,sample))
        plg2 = psS.tile([8, 2, 256], F32)
        for s in range(BPC):
            qd = TB([128, 8], "at_qd")
            V.tensor_tensor(qd[:], qsb[:, s:s + 1].to_broadcast((128, 8)),
                            ct['Emask_q'][:], op=ALU.mult)
            nc.tensor.matmul(plg2[:, s, :], qd[:], Ksb[:, s, :], start=True, stop=True,
                             skip_group_check=True)
        nm = T([8, 1], "at_nm")
        V.tensor_reduce(nm[:], plg2[:].rearrange("p s t -> p (s t)"),
                        axis=AX.X, op=ALU.max, negate=True)
        nm4 = T([8, 1], "at_nm4")
        V.tensor_scalar(nm4[:], nm[:], 0.25, None, op0=ALU.mult)
        ex = T([8, 2, 256], "at_ex")
        S.activation(ex[:].rearrange("p s t -> p (s t)"),
                     plg2[:].rearrange("p s t -> p (s t)"),
                     AF.Exp, bias=nm4[:, 0:1], scale=0.25)
        sm = T([8, 2], "at_sm")
        V.tensor_reduce(sm[:], ex[:], axis=AX.X, op=ALU.add)
        rc = T([8, 2], "at_rc")
        V.reciprocal(rc[:], sm[:])
        aw = TB([8, 2, 256], "at_aw")
        V.tensor_tensor(aw[:], ex[:], rc[:].unsqueeze(2).to_broadcast((8, 2, 256)),
                        op=ALU.mult)
        patB = P512()
        nc.tensor.matmul(patB[:], ct['E_attn'][:], aw[:].rearrange("p s t -> p (s t)"),
                         start=True, stop=True)
        vo = TB([128, 2, 256], "at_vo")
        V.tensor_tensor(vo[:].rearrange("p s t -> p (s t)"),
                        Vsb[:].rearrange("p s t -> p (s t)"), patB[:], op=ALU.mult)
        pao = P512()
        nc.tensor.matmul(pao[:], wt['aoT'][:], vo[:].rearrange("p s t -> p (s t)"),
                         start=True, stop=True)
        xa = TB([128, 2, 256], "xa")
        V.scalar_tensor_tensor(xa[:].rearrange("p s t -> p (s t)"), pao[:],
                               col("ao_b"), xfl[:], op0=ALU.add, op1=ALU.add)
        tap("xa", lambda: tap_batched(xa, [128, L]))

        # ================= spe attention =================
        X2a = TB([128, 2, 2, 128], "x2f_tmp")
        for s in range(BPC):
            for ft in range(2):
                ptr = PT()
                nc.tensor.transpose(ptr[:, 0:128], xa[:, s, ft * 128:(ft + 1) * 128],
                                    ident[:])
                S.copy(X2a[:, s, ft, :], ptr[:, 0:128])
        q2 = TB([128, 2, 2, 128], "sp2_q2")
        k2 = TB([128, 2, 2, 128], "sp2_k2")
        pq2b = P512()
        pk2b = P512()
        for s in range(BPC):
            for ot in range(2):
                for ft in range(2):
                    nc.tensor.matmul(pq2b[:, s * 256 + ot * 128:s * 256 + (ot + 1) * 128],
                                     wt['sqT'][:, ft, ot * 128:(ot + 1) * 128],
                                     X2a[:, s, ft, :], start=(ft == 0), stop=(ft == 1),
                                     skip_group_check=True)
                    nc.tensor.matmul(pk2b[:, s * 256 + ot * 128:s * 256 + (ot + 1) * 128],
                                     wt['skT'][:, ft, ot * 128:(ot + 1) * 128],
                                     X2a[:, s, ft, :], start=(ft == 0), stop=(ft == 1),
                                     skip_group_check=True)
        V.tensor_tensor(q2[:], pq2b[:].rearrange("p (s o c) -> p s o c", s=2, o=2),
                        wt['sqkb'][:, 0].unsqueeze(1).unsqueeze(3)
                        .to_broadcast((128, 2, 2, 128)), op=ALU.add)
        V.tensor_tensor(k2[:], pk2b[:].rearrange("p (s o c) -> p s o c", s=2, o=2),
                        wt['sqkb'][:, 1].unsqueeze(1).unsqueeze(3)
                        .to_broadcast((128, 2, 2, 128)), op=ALU.add)
        # batched v2 / logits / softmax over both samples
        pv2b = P512()
        for s in range(BPC):
            for ft in range(2):
                nc.tensor.matmul(pv2b[:, s * 256:(s + 1) * 256],
                                 X2a[:, s, ft, :], wt['svT'][:, ft, :],
                                 start=(ft == 0), stop=(ft == 1),
                                 skip_group_check=True)
        v2b = TB([128, 2, 256], "sp2_v2")
        V.tensor_tensor(v2b[:], pv2b[:].rearrange("p (s t) -> p s t", s=2),
                        wt['svbB'][:].unsqueeze(1).to_broadcast((128, 2, 256)),
                        op=ALU.add)
        pa2b = P256()
        for s in range(BPC):
            for ot in range(2):
                nc.tensor.matmul(pa2b[:, s * 128:(s + 1) * 128],
                                 q2[:, s, ot, :], k2[:, s, ot, :],
                                 start=(ot == 0), stop=(ot == 1),
                                 skip_group_check=True)
        nm2 = T([128, 1], "sp2_nm")
        V.tensor_reduce(nm2[:], pa2b[:], axis=AX.X, op=ALU.max, negate=True)
        nm16 = T([128, 1], "sp2_nm16")
        V.tensor_scalar(nm16[:], nm2[:], 1.0 / 16, None, op0=ALU.mult)
        ex2 = TB([128, 2, 128], "sp2_ex")
        S.activation(ex2[:].rearrange("p s t -> p (s t)"), pa2b[:],
                     AF.Exp, bias=nm16[:, 0:1], scale=1.0 / 16)
        sm2 = T([128, 2], "sp2_sm")
        V.tensor_reduce(sm2[:], ex2[:], axis=AX.X, op=ALU.add)
        rc2 = T([128, 2], "sp2_rc")
        V.reciprocal(rc2[:], sm2[:])
        a2 = TB([128, 2, 128], "sp2_a2")
        V.tensor_tensor(a2[:], ex2[:], rc2[:].unsqueeze(2).to_broadcast((128, 2, 128)),
                        op=ALU.mult)
        po3b = P512()
        for s in range(BPC):
            pa2T = PT()
            nc.tensor.transpose(pa2T[:, 0:128], a2[:, s, :], ident[:])
            a2T = TB([128, 128], "sp2_a2T")
            S.copy(a2T[:], pa2T[:, 0:128])
            o2 = TB([128, 2, 128], "sp2_o2")
            for ot in range(2):
                po2 = P256()
                nc.tensor.matmul(po2[:, 0:128], v2b[:, s, ot * 128:(ot + 1) * 128], a2T[:],
                                 start=True, stop=True)
                S.copy(o2[:, ot, :], po2[:, 0:128])
            for ot in range(2):
                nc.tensor.matmul(po3b[:, s * 256:(s + 1) * 256],
                                 o2[:, ot, :], wt['soT'][:, ot, :],
                                 start=(ot == 0), stop=(ot == 1),
                                 skip_group_check=True)
        xs2 = TB([128, 2, 256], "xs2")
        for s in range(BPC):
            t3s = TB([128, 256], "sp2_t3")
            V.tensor_tensor(t3s[:], po3b[:, s * 256:(s + 1) * 256],
                            wt['sobB'], op=ALU.add)
            V.tensor_tensor(xs2[:, s, :], t3s[:], xa[:, s, :], op=ALU.add)
        tap("xs2", lambda: tap_batched(xs2, [128, L]))

        # ================= downsample =================
        pds = psD.tile([64, 256], F32, tag="ds", name="ds")
        invf = TB([1, BPC, L], "irow_f")
        dma(invf[:], inv[None, :, :])
        for s in range(BPC):
            invB = P512()
            nc.tensor.matmul(invB[:, 0:L], onesrow1, invf[:, s, :], start=True, stop=True)
            QT = TB([128, 2, 256], "perm_oh")
            for tt in range(2):
                V.tensor_scalar(QT[:, tt, :], invB[:, 0:L], ct['iotaC'][:, tt:tt + 1],
                                None, op0=ALU.is_equal)
            tmv = TB([128, 2, 128], "tm_tmp")
            for tt in range(2):
                ptr = PT()
                nc.tensor.transpose(ptr[:, 0:128], xs2[:, s, tt * 128:(tt + 1) * 128],
                                    ident[:])
                S.copy(tmv[:, tt, :], ptr[:, 0:128])
            pxr = P256()
            for tt in range(2):
                nc.tensor.matmul(pxr[:], tmv[:, tt, :], QT[:, tt, :],
                                 start=(tt == 0), stop=(tt == 1))
            xrp = TB([128, 324], "ds_xrp")
            G.memset(xrp[:], 0.0)
            xr3 = xrp[:].rearrange("p (h w) -> p h w", h=18)
            S.copy(xr3[:, 1:17, 1:17], pxr[:].rearrange("p (h w) -> p h w", h=16))
            for kh in range(3):
                for kw in range(3):
                    k = kh * 3 + kw
                    cmp_ = TB([128, 64], "ds_cmp")
                    (V.tensor_copy if k % 2 == 0 else S.copy)(
                        cmp_[:].rearrange("p (a b) -> p a b", a=8),
                        xr3[:, kh:kh + 16:2, kw:kw + 16:2])
                    nc.tensor.matmul(pds[:, s * 128:(s + 1) * 128],
                                     cmp_[:],
                                     wt['dsw_pk'][:, k, :],
                                     start=(k == 0), stop=(k == 8),
                                     skip_group_check=True)
        for s in range(BPC):
            view = pds[:, s * 128:(s + 1) * 128]
            mus = T([64, 1], "ds_mus")
            V.tensor_reduce(mus[:], view, axis=AX.X, op=ALU.add)
            mean = T([64, 1], "ds_mean")
            V.tensor_scalar(mean[:], mus[:], 1.0 / 128, None, op0=ALU.mult)
            sq = T([64, 128], "ds_sq")
            ss = T([64, 1], "ds_ss")
            S.activation(sq[:], view, AF.Square, accum_out=ss[:, 0:1])
            m2 = T([64, 1], "ds_m2")
            V.tensor_mul(m2[:], mean[:], mean[:])
            var = T([64, 1], "ds_var")
            V.scalar_tensor_tensor(var[:], ss[:], 1.0 / 128, m2[:],
                                   op0=ALU.mult, op1=ALU.subtract)
            lv = T([64, 1], "ds_lv")
            S.activation(lv[:], var[:], AF.Ln, bias=epscol[0:64, 0:1])
            rstd = T([64, 1], "ds_rstd")
            S.activation(rstd[:], lv[:], AF.Exp, scale=-0.5)
            xn = T([64, 128], "ds_xn")
            V.tensor_scalar(xn[:], view, mean[:, 0:1], rstd[:, 0:1],
                            op0=ALU.subtract, op1=ALU.mult)
            t1 = T([64, 128], "ds_t1")
            V.tensor_mul(t1[:], xn[:], wt['ds_ln_wB'][:])
            o1 = T([64, 128], "ds_o1")
            V.tensor_add(o1[:], t1[:], wt['ds_ln_bB'][:])
            dma(out[s].rearrange("h w c -> (h w) c"), o1[:])

        stk.close()
    from concourse.library_overlay import lower_extended_insts
    lower_extended_insts(nc)
    return nc, tap_t


# ---------------------------------------------------------------------------
_CACHE = {}


def _get_program(taps=()):
    key = tuple(sorted(taps))
    if key not in _CACHE:
        _CACHE[key] = build_program(taps)
    return _CACHE[key]


def make_inmaps(inputs, taps=()):
    cst = host_constants()
    w = prep_weights(inputs)
    blob_bf, blob_f32 = pack_blobs(cst, w)
    x = np.asarray(inputs['x'], np.float32).reshape(16, C, L)
    idx = np.asarray(inputs['sorted_index'], np.int32)
    inv = np.argsort(idx, axis=1, kind='stable').astype(np.int32)
    in_maps = []
    for c in range(NCORES):
        m = {'blob_bf': blob_bf, 'blob_f32': blob_f32}
        sl = slice(c * BPC, (c + 1) * BPC)
        m['x2'] = np.ascontiguousarray(x[sl]).astype(BF16NP)
        m['idx'] = np.ascontiguousarray(idx[sl].astype(np.float32)).astype(BF16NP)
        m['inv'] = np.ascontiguousarray(inv[sl].astype(np.float32)).astype(BF16NP)
        in_maps.append(m)
    return in_maps


def run(inputs, taps=(), trace=False):
    nc, tap_t = _get_program(taps)
    in_maps = make_inmaps(inputs, taps)
    res = run_bass_kernel_spmd(nc, in_maps, list(range(NCORES)), trace=trace)
    outs = np.concatenate([np.asarray(r['out'], np.float32) for r in res.results], axis=0)
    tapd = {}
    for name in taps:
        tapd[name] = [np.asarray(r.get('t_' + name), np.float32) for r in res.results]
    return outs, tapd, res


def kernel(**inputs):
    outs, _, _ = run(inputs)
    return outs


# revision 45
# speedup vs baseline: 1.0025x; 1.0025x over previous
"""Trainium2 Bass kernel for nn_Basic_Block_v1 (spatial/spectral Mamba2 block).

Sharding: data-parallel over batch (16 samples) across 8 NeuronCores,
2 samples per core; all parameters replicated. SSD scans are computed in
closed quadratic form on the TensorEngine. All heavy matmuls run in bf16
(1 cycle/row vs 4 for fp32); the cumulative-decay broadcast uses a bf16
hi/lo split to keep fp32-grade cancellation. LayerNorm scale/bias and the
gated-RMS weight are folded into adjacent projection weights on the host;
the Mamba D-residual is applied as a diagonal matmul accumulated into the
same PSUM as the SSD output.
"""
import sys
sys.path.insert(0, '/opt/trn_rl_repo')
import json

import numpy as np
import ml_dtypes

BF16NP = ml_dtypes.bfloat16

import concourse.bass as bass
import concourse.mybir as mybir
from concourse import tile
from concourse import bass_isa
from concourse.bass_utils import run_bass_kernel_spmd

F32 = mybir.dt.float32
BF = mybir.dt.bfloat16
I32 = mybir.dt.int32
AF = mybir.ActivationFunctionType
ALU = mybir.AluOpType
AX = mybir.AxisListType

NCORES = 8
BPC = 2          # batch per core
L = 256          # spatial tokens
C = 128          # channels
H1 = 4           # spa heads
H2 = 8           # spe heads
NST = 64         # d_state
EPS = 1e-5

# ---------------------------------------------------------------------------
# walrus in this container supports only ONE sync-wait per instruction;
# split extra waits emitted by the Tile scheduler onto preceding NoOps.
_WAIT_LIMIT = 1
_orig_to_json = bass.Bass.to_json_bytes


def _fix_block(b, ctr):
    insts = b.get('instructions')
    if insts:
        out = []
        for ins in insts:
            si = ins.get('sync_info')
            waits = (si or {}).get('on_wait') or []
            if len(waits) > _WAIT_LIMIT:
                while len(waits) > _WAIT_LIMIT:
                    chunk, waits = waits[:_WAIT_LIMIT], waits[_WAIT_LIMIT:]
                    ctr[0] += 1
                    out.append({
                        "debug": ins.get("debug"),
                        "engine": ins["engine"],
                        "ins": [],
                        "name": f"I-wsplit{ctr[0]}",
                        "opcode": "NoOp",
                        "outs": [],
                        "text_hint": "wsplit",
                        "sync_info": {"on_update": [], "on_wait": chunk},
                    })
                si['on_wait'] = waits
            out.append(ins)
        b['instructions'] = out
    for sb in b.get('blocks') or []:
        _fix_block(sb, ctr)


def _patched_to_json(self, *a, **k):
    raw = _orig_to_json(self, *a, **k)
    d = json.loads(raw)
    ctr = [0]
    for f in d.get('functions', []):
        for b in f.get('blocks', []):
            _fix_block(b, ctr)
    if ctr[0] == 0:
        return raw
    return json.dumps(d).encode()


bass.Bass.to_json_bytes = _patched_to_json


# ---------------------------------------------------------------------------
def _sincos_2d(dim, Hg):
    def e1(d, pos):
        omega = 1.0 / (10000.0 ** (np.arange(d // 2, dtype=np.float64) / (d / 2.0)))
        out = pos[:, None] * omega[None, :]
        return np.concatenate([np.sin(out), np.cos(out)], axis=-1)
    gh, gw = np.meshgrid(np.arange(Hg), np.arange(Hg), indexing='ij')
    emb = np.concatenate([e1(dim // 2, gh.reshape(-1)), e1(dim // 2, gw.reshape(-1))], axis=-1)
    return emb.astype(np.float32)


def host_constants():
    d = {}
    d['pe_fm'] = np.ascontiguousarray(_sincos_2d(C, 16).T).astype(BF16NP)   # [128, 256]
    d['ident'] = np.eye(128, dtype=np.float32).astype(BF16NP)
    d['ident32'] = np.eye(8, dtype=np.float32)
    iota = np.arange(L, dtype=np.float32)
    d['iotaC'] = np.stack([iota[:128], iota[128:]], axis=1).copy()          # [128, 2] f32
    sidx = np.arange(L)[:, None]
    tidx = np.arange(L)[None, :]
    m = (sidx <= tidx).astype(np.float32)
    d['maskT_spa'] = np.stack([m[:128], m[128:]], axis=1).copy().astype(BF16NP)
    s2 = np.arange(128)[:, None]
    t2 = np.arange(128)[None, :]
    d['maskT_spe'] = (s2 <= t2).astype(np.float32).astype(BF16NP)
    E1 = np.zeros((H1, 2, 128), np.float32)
    for j in range(2):
        for mm in range(128):
            E1[2 * j + mm // 64, j, mm] = 1.0
    d['E_spaJ'] = E1.astype(BF16NP)
    E2 = np.zeros((H2, 4, 128), np.float32)
    for j in range(4):
        for mm in range(128):
            E2[2 * j + mm // 64, j, mm] = 1.0
    d['E_speJ'] = E2.astype(BF16NP)
    EA = np.zeros((8, 128), np.float32)
    for h in range(8):
        EA[h, h * 16:(h + 1) * 16] = 1.0
    d['E_attn'] = EA.astype(BF16NP)
    d['Emask_q'] = EA.T.copy().astype(BF16NP)
    return d


def _col_order():
    cols = []
    for i in range(2):
        cols += [f"spa_dtb{i}", f"spa_negA{i}",
                 f"spa_cb{i}_0", f"spa_cb{i}_1", f"spa_cbBC{i}",
                 f"spa_zb{i}_0", f"spa_zb{i}_1", f"spa_xb{i}_0", f"spa_xb{i}_1",
                 f"spa_BCb{i}", f"spa_dpc{i}_0", f"spa_dpc{i}_1"]
    for i in range(2):
        cols += [f"spe_dtb{i}", f"spe_negA{i}"]
        cols += [f"spe_cb{i}_{j}" for j in range(4)] + [f"spe_cbBC{i}"]
        cols += [f"spe_zb{i}_{j}" for j in range(4)]
        cols += [f"spe_xb{i}_{j}" for j in range(4)]
        cols += [f"spe_BCb{i}"]
        cols += [f"spe_dpc{i}_{j}" for j in range(4)]
    cols += ["lnw_norm", "lnb_norm", "cprj_b", "aq_b", "ak_b", "av_b", "ao_b",
             "sq_b0", "sq_b1", "sk_b0", "sk_b1"]
    return cols


COL_ORDER = _col_order()
CIDX = {k: ix for ix, k in enumerate(COL_ORDER)}


def prep_weights(inp):
    """Host-side layout prep: bf16 casts, LN scale/bias folded into in_proj,
    rms weight folded into out_proj, D as diagonal matrices."""
    f32 = np.float32
    w = {}
    cols = {}
    # ---- spa in_proj with spa_ln fold ----
    w['spa_in_fold'] = np.zeros((2, 128, 644), BF16NP)
    for i in range(2):
        iw = np.asarray(inp['spa_in_w'][i], f32)                 # [644, 128]
        lw = np.asarray(inp['spa_ln_w'][i], f32)
        lb = np.asarray(inp['spa_ln_b'][i], f32)
        br = iw @ lb                                             # [644]
        w['spa_in_fold'][i] = (iw * lw[None, :]).T.astype(BF16NP)
        cols[f"spa_zb{i}_0"] = br[0:128]
        cols[f"spa_zb{i}_1"] = br[128:256]
        cols[f"spa_xb{i}_0"] = br[256:384]
        cols[f"spa_xb{i}_1"] = br[384:512]
        cols[f"spa_BCb{i}"] = br[512:640]
        cols[f"spa_dtb{i}"] = np.asarray(inp['spa_dt_bias'][i], f32) + br[640:644]
        cols[f"spa_negA{i}"] = -np.exp(np.asarray(inp['spa_A_log'][i], f32))
    cv = np.zeros((128, 2, 3, 4), f32)
    for i in range(2):
        cv[:, i, 0] = inp['spa_conv_w'][i, 0:128]
        cv[:, i, 1] = inp['spa_conv_w'][i, 128:256]
        cv[0:64, i, 2] = inp['spa_conv_w'][i, 256:320]
        cv[64:128, i, 2] = inp['spa_conv_w'][i, 320:384]
    w['spa_conv_pk'] = cv
    # out_proj with rms-weight fold: [feat, i, j, out]
    sow = np.transpose(np.asarray(inp['spa_out_w'], f32), (0, 2, 1)).reshape(2, 2, 128, 128)
    sow = sow * np.asarray(inp['spa_rms_w'], f32).reshape(2, 2, 128)[:, :, :, None]
    w['spa_out_pk'] = np.ascontiguousarray(sow.transpose(2, 0, 1, 3)).astype(BF16NP)
    # ---- spe in_proj with spe_ln fold ----
    w['spe_in_pk'] = np.zeros((2, 128, 2, 1160), BF16NP)
    for i in range(2):
        iw = np.asarray(inp['spe_in_w'][i], f32)                 # [1160, 256]
        lw = np.asarray(inp['spe_ln_w'][i], f32)
        lb = np.asarray(inp['spe_ln_b'][i], f32)
        br = iw @ lb
        iwf = (iw * lw[None, :]).T                               # [256, 1160]
        w['spe_in_pk'][i] = iwf.reshape(2, 128, 1160).transpose(1, 0, 2).astype(BF16NP)
        for j in range(4):
            cols[f"spe_zb{i}_{j}"] = br[j * 128:(j + 1) * 128]
            cols[f"spe_xb{i}_{j}"] = br[512 + j * 128:512 + (j + 1) * 128]
        cols[f"spe_BCb{i}"] = br[1024:1152]
        cols[f"spe_dtb{i}"] = np.asarray(inp['spe_dt_bias'][i], f32) + br[1152:1160]
        cols[f"spe_negA{i}"] = -np.exp(np.asarray(inp['spe_A_log'][i], f32))
    cv2 = np.zeros((128, 2, 5, 4), f32)
    for i in range(2):
        for j in range(4):
            cv2[:, i, j] = inp['spe_conv_w'][i, j * 128:(j + 1) * 128]
        cv2[0:64, i, 4] = inp['spe_conv_w'][i, 512:576]
        cv2[64:128, i, 4] = inp['spe_conv_w'][i, 576:640]
    w['spe_conv_pk'] = cv2
    sew = np.transpose(np.asarray(inp['spe_out_w'], f32), (0, 2, 1)).reshape(2, 4, 128, 256)
    sew = sew * np.asarray(inp['spe_rms_w'], f32).reshape(2, 4, 128)[:, :, :, None]
    w['spe_out_pk'] = np.ascontiguousarray(sew.transpose(0, 2, 1, 3)).astype(BF16NP)
    # ---- attention & tail ----
    w['cprj_pk'] = np.ascontiguousarray(
        np.transpose(np.asarray(inp['cprj_w'], f32), (2, 1, 0)).transpose(1, 0, 2)).astype(BF16NP)
    for nm in ('aq', 'ak', 'av', 'ao'):
        w[nm + 'T'] = np.ascontiguousarray(np.asarray(inp[nm + '_w'], f32).T).astype(BF16NP)
    for nm in ('sq', 'sk', 'sv', 'so'):
        wt_ = np.asarray(inp[nm + '_w'], f32).T.reshape(2, 128, 256)
        w[nm + 'T'] = np.ascontiguousarray(wt_.transpose(1, 0, 2)).astype(BF16NP)
    sqkb = np.zeros((128, 2, 2), f32)
    for ot in range(2):
        sqkb[:, 0, ot] = np.asarray(inp['sq_b'], f32)[ot * 128:(ot + 1) * 128]
        sqkb[:, 1, ot] = np.asarray(inp['sk_b'], f32)[ot * 128:(ot + 1) * 128]
    w['sqkb'] = sqkb
    w['svbB'] = np.ascontiguousarray(
        np.broadcast_to(np.asarray(inp['sv_b'], f32)[None, :], (128, 256))).astype(BF16NP)
    w['sobB'] = np.ascontiguousarray(
        np.broadcast_to(np.asarray(inp['so_b'], f32)[None, :], (128, 256))).astype(BF16NP)
    w['dsw_pk'] = np.ascontiguousarray(
        np.asarray(inp['ds_conv_w'], f32).reshape(9, 128, 128).transpose(1, 0, 2)).astype(BF16NP)
    w['ds_ln_wB'] = np.ascontiguousarray(
        np.broadcast_to(np.asarray(inp['ds_ln_w'], f32)[None, :], (64, 128)))
    w['ds_ln_bB'] = np.ascontiguousarray(
        np.broadcast_to(np.asarray(inp['ds_ln_b'], f32)[None, :], (64, 128)))
    # ---- small column-packed params (fp32 scalar operands) ----
    for i in range(2):
        cols[f"spa_cb{i}_0"] = inp['spa_conv_b'][i, 0:128]
        cols[f"spa_cb{i}_1"] = inp['spa_conv_b'][i, 128:256]
        cols[f"spa_cbBC{i}"] = inp['spa_conv_b'][i, 256:384]
        dpc = np.repeat(np.asarray(inp['spa_D'][i], f32), 64)
        cols[f"spa_dpc{i}_0"] = dpc[0:128]
        cols[f"spa_dpc{i}_1"] = dpc[128:256]
        dpc2 = np.repeat(np.asarray(inp['spe_D'][i], f32), 64)
        for j in range(4):
            cols[f"spe_dpc{i}_{j}"] = dpc2[j * 128:(j + 1) * 128]
        for j in range(4):
            cols[f"spe_cb{i}_{j}"] = inp['spe_conv_b'][i, j * 128:(j + 1) * 128]
        cols[f"spe_cbBC{i}"] = inp['spe_conv_b'][i, 512:640]
    cols["lnw_norm"] = inp['norm_w']
    cols["lnb_norm"] = inp['norm_b']
    cols["cprj_b"] = inp['cprj_b']
    for nm in ('aq', 'ak', 'av', 'ao'):
        cols[nm + "_b"] = inp[nm + '_b']
    cols["sq_b0"] = inp['sq_b'][0:128]
    cols["sq_b1"] = inp['sq_b'][128:256]
    cols["sk_b0"] = inp['sk_b'][0:128]
    cols["sk_b1"] = inp['sk_b'][128:256]
    pk = np.zeros((128, len(COL_ORDER)), f32)
    for k, v in cols.items():
        v = np.asarray(v, f32)
        pk[0:v.shape[0], CIDX[k]] = v
    w['colpak'] = pk
    return w


W_SHAPES = {
    'spa_in_fold': ([2, 128, 644], BF), 'spa_conv_pk': ([128, 2, 3, 4], F32),
    'spa_out_pk': ([128, 2, 2, 128], BF),
    'spe_in_pk': ([2, 128, 2, 1160], BF), 'spe_conv_pk': ([128, 2, 5, 4], F32),
    'spe_out_pk': ([2, 128, 4, 256], BF),
    'cprj_pk': ([128, 5, 128], BF),
    'aqT': ([128, 128], BF), 'akT': ([128, 128], BF), 'avT': ([128, 128], BF),
    'aoT': ([128, 128], BF),
    'sqT': ([128, 2, 256], BF), 'skT': ([128, 2, 256], BF), 'svT': ([128, 2, 256], BF),
    'soT': ([128, 2, 256], BF), 'svbB': ([128, 256], BF), 'sobB': ([128, 256], BF),
    'sqkb': ([128, 2, 2], F32),
    'dsw_pk': ([128, 9, 128], BF), 'ds_ln_wB': ([64, 128], F32), 'ds_ln_bB': ([64, 128], F32),
    'colpak': ([128, len(COL_ORDER)], F32),
}

CST_DT = {'pe_fm': BF, 'ident': BF, 'ident32': F32, 'iotaC': F32,
          'maskT_spa': BF, 'maskT_spe': BF, 'E_spaJ': BF, 'E_speJ': BF,
          'E_attn': BF, 'Emask_q': BF}

# ---- blob packing: all params as column ranges of two [128, N] blobs ----
BLOB_BF = [
    ('pe_fm', 128, [256]), ('ident', 128, [128]),
    ('maskT_spa', 128, [2, 256]), ('maskT_spe', 128, [128]),
    ('Emask_q', 128, [8]), ('E_spaJ', 4, [2, 128]), ('E_speJ', 8, [4, 128]),
    ('E_attn', 8, [128]),
    ('spa_in_fold', 128, [2, 644]),
    ('spa_out_pk', 128, [2, 2, 128]),
    ('spe_in_pk', 128, [2, 2, 1160]),
    ('spe_out_pk', 128, [2, 4, 256]),
    ('cprj_pk', 128, [5, 128]),
    ('aqT', 128, [128]), ('akT', 128, [128]), ('avT', 128, [128]),
    ('aoT', 128, [128]),
    ('sqT', 128, [2, 256]), ('skT', 128, [2, 256]), ('svT', 128, [2, 256]),
    ('soT', 128, [2, 256]), ('svbB', 128, [256]), ('sobB', 128, [256]),
    ('dsw_pk', 128, [9, 128]),
]
BLOB_F32 = [
    ('colpak', 128, [len(COL_ORDER)]),
    ('spa_conv_pk', 128, [2, 3, 4]), ('spe_conv_pk', 128, [2, 5, 4]),
    ('iotaC', 128, [2]), ('sqkb', 128, [2, 2]), ('ident32', 8, [8]),
    ('ds_ln_wB', 64, [128]), ('ds_ln_bB', 64, [128]),
]


def _blob_offsets(spec):
    offs = {}
    c = 0
    for name, _, vshape in spec:
        n = int(np.prod(vshape))
        offs[name] = (c, n)
        c += n
    return offs, c


BF_OFFS, BF_COLS = _blob_offsets(BLOB_BF)
F32_OFFS, F32_COLS = _blob_offsets(BLOB_F32)
_LAYER_MAJOR = {'spa_in_fold': (1, 0, 2), 'spe_in_pk': (1, 0, 2, 3),
                'spe_out_pk': (1, 0, 2, 3)}


def pack_blobs(cst, w):
    pool = dict(cst)
    pool.update(w)
    bf = np.zeros((128, BF_COLS), BF16NP)
    f32 = np.zeros((128, F32_COLS), np.float32)
    for spec, blob, offs in ((BLOB_BF, bf, BF_OFFS), (BLOB_F32, f32, F32_OFFS)):
        for name, rows, vshape in spec:
            a = np.asarray(pool[name])
            if name in _LAYER_MAJOR:
                a = np.transpose(a, _LAYER_MAJOR[name])
            off, n = offs[name]
            blob[0:rows, off:off + n] = a.reshape(rows, n)
    return bf, f32


# ---------------------------------------------------------------------------
def build_program(taps=()):
    nc = bass.Bass()

    def din(name, shape, dt=F32):
        return nc.dram_tensor(name, shape, dt, kind="ExternalInput")

    x2 = din("x2", [BPC, C, L], BF)
    idx = din("idx", [BPC, L], BF)
    inv = din("inv", [BPC, L], BF)

    blob_bf_t = din("blob_bf", [128, BF_COLS], BF)
    blob_f32_t = din("blob_f32", [128, F32_COLS], F32)

    out = nc.dram_tensor("out", [BPC, 8, 8, C], F32, kind="ExternalOutput")
    tap_t = {}

    with tile.TileContext(nc) as tc:
        import contextlib
        stk = contextlib.ExitStack()
        sb = stk.enter_context(tc.tile_pool(name="sb", bufs=1))
        ps1 = stk.enter_context(tc.tile_pool(name="ps1", bufs=3, space="PSUM"))
        ps2 = stk.enter_context(tc.tile_pool(name="ps2", bufs=4, space="PSUM"))
        psD = stk.enter_context(tc.tile_pool(name="psD", bufs=1, space="PSUM"))

        class _PSShim:
            def tile(self, shape, dt, tag="small", name="small"):
                return ps2.tile(shape, dt, tag="b256", name="ps_sm")

        psS = _PSShim()

        BUFS2 = {"cv_a0", "cv_a1", "rowA", "rowB", "rowC", "tm_tmp", "ssd_Dt",
                 "ssd_Et", "ssd_MT", "spa_xtm", "spe_xtm", "sq_tmp", "x2f_tmp",
                 "ssd_m0m", "spa_acumT", "spe_acumT", "spa_ygt", "spa_y0t",
                 "spa_ynt", "spe_ygt", "spe_y0t", "spe_ynt", "ds_cmp",
                 "spe_xn", "spe_h2sb", "sp2_a2T", "sp2_o2", "perm_oh", "sp2_t3",
                 "ds_xrp", "xc_0", "xc_1", "xc_2", "xc_3", "xc_BC", "xc_C",
                 "cv_x0", "cv_x1", "cv_x2", "cv_x3", "cv_BC", "aflat", "cv_g0",
                 "cv_g1"}

        def T(shape, tag, dt=F32):
            return sb.tile(shape, dt, tag=tag, name=tag,
                           bufs=2 if tag in BUFS2 else 1)

        def TB(shape, tag):
            return T(shape, tag, BF)

        def P512(tag="b512"):
            return ps1.tile([128, 512], F32, tag="b512", name="b512")

        def P256(tag="b256"):
            return ps2.tile([128, 256], F32, tag="b256", name="b256")

        def PT(tag="bT"):
            return ps2.tile([128, 256], BF, tag="b256", name="bT")

        def tap(name, ap_fn):
            if name in taps:
                shape, writer, dt = ap_fn()
                t = nc.dram_tensor("t_" + name, shape, dt, kind="ExternalOutput")
                tap_t[name] = t
                writer(t)

        dma = nc.sync.dma_start
        V = nc.vector
        S = nc.scalar
        G = nc.gpsimd
        RO = bass_isa.ReduceOp

        # ---------- inputs first, then all params via two blobs ----------
        xb = TB([128, BPC, L], "xb")
        for s in range(BPC):
            dma(xb[:, s, :], x2[s])
        idxf = TB([1, BPC, L], "irow_f")
        dma(idxf[:], idx[None, :, :])
        blob_f32 = T([128, F32_COLS], "blob_f32")
        dma(blob_f32[:], blob_f32_t[:])
        blob_bf = TB([128, BF_COLS], "blob_bf")
        CH = 4096
        for c0 in range(0, BF_COLS, CH):
            c1 = min(c0 + CH, BF_COLS)
            dma(blob_bf[:, c0:c1], blob_bf_t[:, c0:c1])

        def _view(blob, offs, name, rows, vshape):
            off, n = offs[name]
            ap = blob[0:rows, off:off + n]
            if len(vshape) == 2:
                ap = ap.rearrange("p (a b) -> p a b", a=vshape[0])
            elif len(vshape) == 3:
                ap = ap.rearrange("p (a b c) -> p a b c", a=vshape[0], b=vshape[1])
            return ap

        ct = {}
        wt = {}
        for name, rows, vshape in BLOB_BF:
            v = _view(blob_bf, BF_OFFS, name, rows, vshape)
            (ct if name in CST_DT else wt)[name] = v
        for name, rows, vshape in BLOB_F32:
            v = _view(blob_f32, F32_OFFS, name, rows, vshape)
            (ct if name in CST_DT else wt)[name] = v
        colpak = wt['colpak']
        inw_l = [wt['spa_in_fold'][:, i, :] for i in range(2)]
        inw2_l = [wt['spe_in_pk'][:, i, :, :] for i in range(2)]
        ow2_l = [wt['spe_out_pk'][:, i, :, :] for i in range(2)]

        def col(key, p=128):
            return colpak[0:p, CIDX[key]:CIDX[key] + 1]

        ones4 = TB([128, 128], "ones4")
        V.memset(ones4[:], 1.0)
        epscol = T([128, 1], "epscol")
        V.memset(epscol[:], EPS)
        onescol = ones4[:, 0:1]       # [128,1] bf16
        onesrow1 = ones4[0:1, :]      # [1,128] bf16
        ones2 = ones4[0:2, :]         # [2,128] bf16
        ident = ct['ident']
        ident32 = ct['ident32']

        # ---------- stage 0: embed + permute ----------
        x0 = TB([128, BPC, L], "x0")
        V.tensor_tensor(
            x0[:], xb[:],
            ct['pe_fm'][:].unsqueeze(1).to_broadcast((128, BPC, L)),
            op=ALU.add)

        xs = TB([128, BPC, L], "xs")
        for s in range(BPC):
            idxB = P512()
            nc.tensor.matmul(idxB[:, 0:L], onesrow1, idxf[:, s, :], start=True, stop=True)
            PmT = TB([128, 2, L], "perm_oh")
            for st in range(2):
                V.tensor_scalar(PmT[:, st, :], idxB[:, 0:L], ct['iotaC'][:, st:st + 1],
                                None, op0=ALU.is_equal)
            x0tm = TB([128, 2, 128], "tm_tmp")
            for tt in range(2):
                ptr = PT()
                nc.tensor.transpose(ptr[:, 0:128], x0[:, s, tt * 128:(tt + 1) * 128], ident[:])
                S.copy(x0tm[:, tt, :], ptr[:, 0:128])
            pxs = P256()
            for st in range(2):
                nc.tensor.matmul(pxs[:], x0tm[:, st, :], PmT[:, st, :],
                                 start=(st == 0), stop=(st == 1))
            S.copy(xs[:, s, :], pxs[:])

        def tap_batched(t_sb, shape_per_s, dt=BF):
            def writer(dram):
                for s in range(BPC):
                    dma(dram[s], t_sb[:, s, :])
            return ([BPC] + shape_per_s, writer, dt)

        tap("xs0", lambda: tap_batched(xs, [128, L]))

        # ================= shared helpers =================
        def part_ln(xflat, final=False):
            """LayerNorm over the channel (partition) dim of [128, 512] bf16.
            Non-final: scale/bias folded downstream -> returns (x-mu)*rstd."""
            sq = TB([128, 512], "sq_tmp")
            S.activation(sq[:], xflat, AF.Square)
            msum = psS.tile([1, 512], F32)
            nc.tensor.matmul(msum[:], onescol, xflat, start=True, stop=True)
            ssum = psS.tile([1, 512], F32)
            nc.tensor.matmul(ssum[:], onescol, sq[:], start=True, stop=True)
            mu2 = T([1, 512], "rowA")
            S.activation(mu2[:], msum[:], AF.Square, scale=1.0 / 128)
            var = T([1, 512], "rowB")
            V.scalar_tensor_tensor(var[:], ssum[:], 1.0 / 128, mu2[:],
                                   op0=ALU.mult, op1=ALU.subtract)
            lnv = T([1, 512], "rowA")
            S.activation(lnv[:], var[:], AF.Ln, bias=epscol[0:1, 0:1])
            rstd = TB([1, 512], "ln_rstd")
            S.activation(rstd[:], lnv[:], AF.Exp, scale=-0.5)
            r0 = TB([1, 512], "ln_r0")
            V.scalar_tensor_tensor(r0[:], msum[:], -1.0 / 128, rstd[:],
                                   op0=ALU.mult, op1=ALU.mult)
            rstdB = P512()
            nc.tensor.matmul(rstdB[:], onesrow1, rstd[:], start=True, stop=True)
            r0B = P512()
            nc.tensor.matmul(r0B[:], onesrow1, r0[:], start=True, stop=True)
            tmp = TB([128, 512], "ln_tmp")
            V.tensor_tensor(tmp[:], xflat, rstdB[:], op=ALU.mult)
            xln = TB([128, 512], "ln_out")
            if final:
                xn = T([128, 512], "ln_xn")
                V.tensor_tensor(xn[:], tmp[:], r0B[:], op=ALU.add)
                S.activation(xln[:], xn[:], AF.Identity, bias=col("lnb_norm"),
                             scale=col("lnw_norm"))
            else:
                V.tensor_tensor(xln[:], tmp[:], r0B[:], op=ALU.add)
            return xln

        def convchain(buf, wc, cb, P, W, tag, E=None):
            """Causal depthwise conv (k=4) + silu. buf [P, 2, W+3] fp32 ->
            bf16 output. E selects the elementwise engine (vector/gpsimd)."""
            E = E or V
            a0 = T([P, 2, W], "cv_a0" if E is V else "cv_g0")
            E.tensor_scalar(a0[:], buf[:, :, 0:W], wc[:, 0:1], None, op0=ALU.mult)
            a1 = T([P, 2, W], "cv_a1" if E is V else "cv_g1")
            E.scalar_tensor_tensor(a1[:], buf[:, :, 1:W + 1], wc[:, 1:2], a0[:],
                                   op0=ALU.mult, op1=ALU.add)
            a2 = T([P, 2, W], "cv_a0" if E is V else "cv_g0")
            E.scalar_tensor_tensor(a2[:], buf[:, :, 2:W + 2], wc[:, 2:3], a1[:],
                                   op0=ALU.mult, op1=ALU.add)
            a3 = T([P, 2, W], "cv_a1" if E is V else "cv_g1")
            E.scalar_tensor_tensor(a3[:], buf[:, :, 3:W + 3], wc[:, 3:4], a2[:],
                                   op0=ALU.mult, op1=ALU.add)
            xc = TB([P, 2, W], tag)
            S.activation(xc[:], a3[:], AF.Silu, bias=cb[:, 0:1])
            return xc

        def dt_ladder(pdt, nh, NW, dtb_key, negA_key):
            """softplus(dt+bias) -> dtv_bf (matmul operand), acum f32,
            hi/lo bf16 rows for the decay broadcast."""
            e1 = T([nh, NW], "rowA")
            S.activation(e1[:], pdt[:], AF.Exp, bias=col(dtb_key, nh))
            e1p = T([nh, NW], "rowB")
            V.tensor_scalar(e1p[:], e1[:], 1.0, None, op0=ALU.add)
            dtv = T([nh, NW], "mb_dtv")
            S.activation(dtv[:], e1p[:], AF.Ln)

            dtv_bf = TB([nh, NW], "mb_dtvbf")
            S.copy(dtv_bf[:], dtv[:])
            dtA = T([nh, NW], "rowA")
            V.tensor_scalar(dtA[:], dtv[:], col(negA_key, nh), None, op0=ALU.mult)
            acum = T([nh, NW], "mb_acum")
            seg = NW // BPC
            for s in range(BPC):
                V.tensor_tensor_scan(acum[:, s * seg:(s + 1) * seg],
                                     dtA[:, s * seg:(s + 1) * seg],
                                     dtA[:, s * seg:(s + 1) * seg], 0.0,
                                     op0=ALU.add, op1=ALU.bypass)
            hi = TB([nh, NW], "acum_hi")
            S.copy(hi[:], acum[:])
            lo = TB([nh, NW], "acum_lo")
            G.tensor_tensor(lo[:], acum[:], hi[:], op=ALU.subtract)
            hilo = TB([2, BPC, 1024], "aflat")
            for s in range(BPC):
                dma(hilo[0:1, s, :].rearrange("o (p f) -> o p f", p=nh),
                    hi[:, s * seg:(s + 1) * seg])
                dma(hilo[1:2, s, :].rearrange("o (p f) -> o p f", p=nh),
                    lo[:, s * seg:(s + 1) * seg])
            return dtv_bf, acum, hilo

        # ================= spa mamba =================
        def spa_mamba(i, xs):
            xflat = xs[:].rearrange("p s t -> p (s t)")
            xln = part_ln(xflat)
            tap(f"xln{i}", lambda: ([128, 512], lambda d: dma(d[:], xln[:]), BF))
            inw = inw_l[i][:]
            # dt first: its Exp/Ln then run before the silu cluster
            pdt = psS.tile([4, 512], F32)
            nc.tensor.matmul(pdt[:], inw[:, 640:644], xln[:], start=True, stop=True)
            dtv_bf, acum, hilo = dt_ladder(pdt, 4, 512, f"spa_dtb{i}", f"spa_negA{i}")
            cvx = []
            for j in range(2):
                px = P512()
                nc.tensor.matmul(px[:], inw[:, 256 + j * 128:256 + (j + 1) * 128], xln[:],
                                 start=True, stop=True)
                buf = T([128, 2, 259], f"cv_x{j}")
                G.memset(buf[:, :, 0:3], 0.0)
                S.activation(buf[:, :, 3:259], px[:].rearrange("p (s t) -> p s t", s=2),
                             AF.Identity, bias=col(f"spa_xb{i}_{j}"))
                cvx.append(buf)
            # B and C merged: one matmul, one buf, one chain (B rows 0:63, C 64:127)
            pbc = P512()
            nc.tensor.matmul(pbc[:], inw[:, 512:640], xln[:], start=True, stop=True)
            bufBC = T([128, 2, 259], "cv_BC")
            G.memset(bufBC[:, :, 0:3], 0.0)
            S.activation(bufBC[:, :, 3:259], pbc[:].rearrange("p (s t) -> p s t", s=2),
                         AF.Identity, bias=col(f"spa_BCb{i}"))
            # conv + silu (silu table region)
            xc = []
            for j in range(2):
                xc.append(convchain(cvx[j], wt['spa_conv_pk'][:, i, j, :],
                                    col(f"spa_cb{i}_{j}"), 128, 256, f"xc_{j}"))
            xcBC = convchain(bufBC, wt['spa_conv_pk'][:, i, 2, :],
                             col(f"spa_cbBC{i}"), 128, 256, "xc_BC")
            # z -> silu emitted after convs so dt's Ln precedes the silu cluster
            zsil = TB([128, 2, 512], "mb_zsil")
            for j in range(2):
                pz = P512()
                nc.tensor.matmul(pz[:], inw[:, j * 128:(j + 1) * 128], xln[:],
                                 start=True, stop=True)
                S.activation(zsil[:, j, :], pz[:], AF.Silu, bias=col(f"spa_zb{i}_{j}"))
            # C half to a partition-0-based tile (matmul needs equal base partitions)
            xcC = TB([64, 2, 256], "xc_C")
            dma(xcC[:], xcBC[64:128, :, :])
            if i == 0:
                tap("dbg_zsil", lambda: ([128, 1024], lambda d: dma(
                    d[:], zsil[:].rearrange("p j t -> p (j t)")), BF))
                tap("dbg_xc0", lambda: ([128, 512], lambda d: dma(
                    d[:], xc[0][:].rearrange("p s t -> p (s t)")), BF))
                tap("dbg_xcBC", lambda: ([128, 512], lambda d: dma(
                    d[:], xcBC[:].rearrange("p s t -> p (s t)")), BF))
                tap("dbg_xcC", lambda: ([64, 512], lambda d: dma(
                    d[:], xcC[:].rearrange("p s t -> p (s t)")), BF))
                tap("dbg_dtv", lambda: ([4, 512], lambda d: dma(d[:], dtv_bf[:]), BF))
                tap("dbg_acum", lambda: ([4, 512], lambda d: dma(d[:], acum[:]), F32))
            # dt-scaled x (feature-major)
            xp = TB([128, 2, 512], "mb_xp")
            for j in range(2):
                pdb = P512()
                nc.tensor.matmul(pdb[:], ct['E_spaJ'][:, j, :], dtv_bf[:], start=True, stop=True)
                V.tensor_tensor(xp[:, j, :], xc[j][:].rearrange("p s t -> p (s t)"), pdb[:],
                                op=ALU.mult)
            if i == 0:
                tap("dbg_xp", lambda: ([128, 1024], lambda d: dma(
                    d[:], xp[:].rearrange("p j t -> p (j t)")), BF))
            h1 = TB([128, 2, 256], "h1")
            for s in range(BPC):
                xtm = TB([128, 2, 256], "spa_xtm")
                for st in range(2):
                    for j in range(2):
                        ptr = PT()
                        nc.tensor.transpose(
                            ptr[:, 0:128],
                            xp[:, j, s * 256 + st * 128: s * 256 + (st + 1) * 128],
                            ident[:])
                        V.tensor_copy(xtm[:, st, j * 128:(j + 1) * 128], ptr[:, 0:128])
                m0m = TB([128, 2, 256], "ssd_m0m")
                for st in range(2):
                    pm0 = P256()
                    nc.tensor.matmul(pm0[:], xcBC[0:64, s, st * 128:(st + 1) * 128],
                                     xcC[:, s, :], start=True, stop=True)
                    V.tensor_tensor(m0m[:, st, :], pm0[:], ct['maskT_spa'][:, st, :],
                                    op=ALU.mult)
                acumT = T([128, 2, 4], "spa_acumT")
                for tt in range(2):
                    ptr2 = P256()
                    nc.tensor.transpose(ptr2[:, 0:4],
                                        acum[:, s * 256 + tt * 128: s * 256 + (tt + 1) * 128],
                                        ident32[0:4, 0:4])
                    S.copy(acumT[:, tt, :], ptr2[:, 0:4])
                pb1 = P512()
                nc.tensor.matmul(pb1[:], ones2, hilo[:, s, 0:512], start=True, stop=True)
                pb2 = P512()
                nc.tensor.matmul(pb2[:], ones2, hilo[:, s, 512:1024], start=True, stop=True)
                yps = P512()
                for st in range(2):
                    Dt = T([128, 4, 256], "ssd_Dt")
                    for h in range(H1):
                        pbx = pb1 if h < 2 else pb2
                        V.tensor_scalar(Dt[:, h, :],
                                        pbx[:, (h % 2) * 256:(h % 2 + 1) * 256],
                                        acumT[:, st, h:h + 1], 0.0,
                                        op0=ALU.subtract, op1=ALU.min)
                    Et = TB([128, 4, 256], "ssd_Et")
                    S.activation(Et[:].rearrange("p h t -> p (h t)"),
                                 Dt[:].rearrange("p h t -> p (h t)"), AF.Exp)
                    MT = TB([128, 4, 256], "ssd_MT")
                    V.tensor_tensor(MT[:], Et[:],
                                    m0m[:, st, :].unsqueeze(1).to_broadcast((128, 4, 256)),
                                    op=ALU.mult)
                    for h in range(H1):
                        nc.tensor.matmul(
                            yps[(h % 2) * 64:(h % 2) * 64 + 64,
                                (h // 2) * 256:(h // 2) * 256 + 256],
                            xtm[:, st, h * 64:(h + 1) * 64],
                            MT[:, h, :],
                            start=(st == 0), stop=(st == 1),
                            tile_position=(0, (h % 2) * 64),
                            skip_group_check=True)
                if i == 0 and s == 0:
                    tap("dbg_xtm", lambda: ([128, 512], lambda d: dma(
                        d[:], xtm[:].rearrange("p s t -> p (s t)")), BF))
                    tap("dbg_m0m", lambda: ([128, 512], lambda d: dma(
                        d[:], m0m[:].rearrange("p s t -> p (s t)")), BF))
                    if "dbg_yps" in taps:
                        ypc = T([128, 512], "dbg_ypc")
                        S.copy(ypc[:], yps[:])
                        tap("dbg_yps", lambda: ([128, 512], lambda d: dma(
                            d[:], ypc[:]), F32))
                y0t = TB([128, 2, 256], "spa_y0t")
                for j in range(2):
                    V.scalar_tensor_tensor(y0t[:, j, :], xc[j][:, s, :],
                                           col(f"spa_dpc{i}_{j}"),
                                           yps[:, j * 256:(j + 1) * 256],
                                           op0=ALU.mult, op1=ALU.add)
                ygt = TB([128, 2, 256], "spa_ygt")
                V.tensor_tensor(ygt[:], y0t[:],
                                zsil[:, :, s * 256:(s + 1) * 256], op=ALU.mult)
                if i == 0 and s == 0:
                    tap("dbg_ygt", lambda: ([128, 512], lambda d: dma(
                        d[:], ygt[:].rearrange("p j t -> p (j t)")), BF))
                sqy = TB([128, 2, 256], "sq_tmp")
                S.activation(sqy[:].rearrange("p j t -> p (j t)"),
                             ygt[:].rearrange("p j t -> p (j t)"), AF.Square)
                ssy = psS.tile([1, 256], F32)
                for j in range(2):
                    nc.tensor.matmul(ssy[:], onescol, sqy[:, j, :],
                                     start=(j == 0), stop=(j == 1))
                rl = T([1, 256], "rowA")
                S.activation(rl[:], ssy[:], AF.Ln, bias=epscol[0:1, 0:1],
                             scale=1.0 / 256)
                rrow = TB([1, 256], "rowC")
                S.activation(rrow[:], rl[:], AF.Exp, scale=-0.5)
                rB = P256()
                nc.tensor.matmul(rB[:], onesrow1, rrow[:], start=True, stop=True)
                ynt = TB([128, 2, 256], "spa_ynt")
                V.tensor_tensor(ynt[:], ygt[:],
                                rB[:].unsqueeze(1).to_broadcast((128, 2, 256)),
                                op=ALU.mult)
                pop = P256()
                for j in range(2):
                    nc.tensor.matmul(pop[:], wt['spa_out_pk'][:, i, j, :], ynt[:, j, :],
                                     start=(j == 0), stop=(j == 1))
                V.tensor_tensor(h1[:, s, :], pop[:], xs[:, s, :], op=ALU.add)
            return h1

        # ================= spe mamba =================
        def spe_mamba(i, h1):
            mus = T([128, 2], "spe_mus")
            V.tensor_reduce(mus[:], h1[:], axis=AX.X, op=ALU.add)
            sqd = TB([128, 2, 256], "sq_tmp")
            ss2 = T([128, 2], "spe_ss2")
            for s in range(BPC):
                S.activation(sqd[:, s, :], h1[:, s, :], AF.Square,
                             accum_out=ss2[:, s:s + 1])
            mean = T([128, 2], "spe_mean")
            V.tensor_scalar(mean[:], mus[:], 1.0 / 256, None, op0=ALU.mult)
            m2 = T([128, 2], "spe_m2")
            S.activation(m2[:], mean[:], AF.Square)
            var2 = T([128, 2], "spe_var")
            V.scalar_tensor_tensor(var2[:], ss2[:], 1.0 / 256, m2[:],
                                   op0=ALU.mult, op1=ALU.subtract)
            l2t = T([128, 2], "spe_l2")
            S.activation(l2t[:], var2[:], AF.Ln, bias=epscol[:, 0:1])
            rstd2 = T([128, 2], "spe_rstd")
            S.activation(rstd2[:], l2t[:], AF.Exp, scale=-0.5)
            X2f = TB([128, 2, 2, 128], "x2f_tmp")
            for s in range(BPC):
                xn = TB([128, 256], "spe_xn")
                V.tensor_scalar(xn[:], h1[:, s, :], mean[:, s:s + 1], rstd2[:, s:s + 1],
                                op0=ALU.subtract, op1=ALU.mult)
                for ft in range(2):
                    ptr = PT()
                    nc.tensor.transpose(ptr[:, 0:128], xn[:, ft * 128:(ft + 1) * 128],
                                        ident[:])
                    V.tensor_copy(X2f[:, s, ft, :], ptr[:, 0:128])
            inw2 = inw2_l[i][:]
            ow2 = ow2_l[i][:]

            def mm2(out_ap, off, width):
                for k in range(2):
                    nc.tensor.matmul(out_ap,
                                     inw2[:, k, off:off + width],
                                     X2f[:, :, k, :],
                                     start=(k == 0), stop=(k == 1))
            # dt first (exp/ln before the silu cluster)
            pdt = psS.tile([8, 256], F32)
            mm2(pdt[:], 1152, 8)
            dtv_bf, acum, hilo = dt_ladder(pdt, 8, 256, f"spe_dtb{i}", f"spe_negA{i}")
            cvx2 = []
            for j in range(4):
                px = P256()
                mm2(px[:], 512 + j * 128, 128)
                buf = T([128, 2, 131], f"cv_x{j}")
                G.memset(buf[:, :, 0:3], 0.0)
                S.activation(buf[:, :, 3:131], px[:].rearrange("p (s t) -> p s t", s=2),
                             AF.Identity, bias=col(f"spe_xb{i}_{j}"))
                cvx2.append(buf)
            pbc = P256()
            mm2(pbc[:], 1024, 128)
            bufBC = T([128, 2, 131], "cv_BC")
            G.memset(bufBC[:, :, 0:3], 0.0)
            S.activation(bufBC[:, :, 3:131], pbc[:].rearrange("p (s t) -> p s t", s=2),
                         AF.Identity, bias=col(f"spe_BCb{i}"))
            xc2 = []
            for j in range(4):
                xc2.append(convchain(cvx2[j], wt['spe_conv_pk'][:, i, j, :],
                                     col(f"spe_cb{i}_{j}"), 128, 128, f"xc_{j}"))
            xcBC = convchain(bufBC, wt['spe_conv_pk'][:, i, 4, :],
                             col(f"spe_cbBC{i}"), 128, 128, "xc_BC")
            z2sil = TB([128, 4, 256], "mb_zsil")
            for j in range(4):
                pz = P256()
                mm2(pz[:], j * 128, 128)
                S.activation(z2sil[:, j, :], pz[:], AF.Silu, bias=col(f"spe_zb{i}_{j}"))
            xcC = TB([64, 2, 128], "xc_C")
            dma(xcC[:], xcBC[64:128, :, :])
            xp2 = TB([128, 4, 256], "mb_xp")
            for j in range(4):
                pdb = P256()
                nc.tensor.matmul(pdb[:], ct['E_speJ'][:, j, :], dtv_bf[:], start=True, stop=True)
                V.tensor_tensor(xp2[:, j, :], xc2[j][:].rearrange("p s t -> p (s t)"), pdb[:],
                                op=ALU.mult)
            xs_new = TB([128, 2, 256], "xs")
            for s in range(BPC):
                xtm2 = TB([128, 512], "spe_xtm")
                for j in range(4):
                    ptr = PT()
                    nc.tensor.transpose(ptr[:, 0:128],
                                        xp2[:, j, s * 128:(s + 1) * 128], ident[:])
                    V.tensor_copy(xtm2[:, j * 128:(j + 1) * 128], ptr[:, 0:128])
                m0m2 = TB([128, 128], "ssd_m0m")
                pm0 = P256()
                nc.tensor.matmul(pm0[:, 0:128], xcBC[0:64, s, :], xcC[:, s, :],
                                 start=True, stop=True)
                V.tensor_tensor(m0m2[:], pm0[:, 0:128], ct['maskT_spe'][:], op=ALU.mult)
                acumT = T([128, 8], "spe_acumT")
                ptr2 = P256()
                nc.tensor.transpose(ptr2[:, 0:8], acum[:, s * 128:(s + 1) * 128],
                                    ident32[0:8, 0:8])
                S.copy(acumT[:], ptr2[:, 0:8])
                pb1 = P512()
                nc.tensor.matmul(pb1[:], ones2, hilo[:, s, 0:512], start=True, stop=True)
                pb2 = P512()
                nc.tensor.matmul(pb2[:], ones2, hilo[:, s, 512:1024], start=True, stop=True)
                yps = P512()
                Dt = T([128, 8, 128], "ssd_Dt")
                for h in range(H2):
                    pbx = pb1 if h < 4 else pb2
                    V.tensor_scalar(Dt[:, h, :],
                                    pbx[:, (h % 4) * 128:(h % 4 + 1) * 128],
                                    acumT[:, h:h + 1], 0.0,
                                    op0=ALU.subtract, op1=ALU.min)
                Et = TB([128, 8, 128], "ssd_Et")
                S.activation(Et[:].rearrange("p h t -> p (h t)"),
                             Dt[:].rearrange("p h t -> p (h t)"), AF.Exp)
                MT = TB([128, 8, 128], "ssd_MT")
                V.tensor_tensor(MT[:], Et[:],
                                m0m2[:].unsqueeze(1).to_broadcast((128, 8, 128)),
                                op=ALU.mult)
                for j in range(4):
                    for hh in range(2):
                        h = 2 * j + hh
                        nc.tensor.matmul(yps[hh * 64:hh * 64 + 64, j * 128:(j + 1) * 128],
                                         xtm2[:, h * 64:(h + 1) * 64],
                                         MT[:, h, :], start=True, stop=True,
                                         tile_position=(0, hh * 64),
                                         skip_group_check=True)
                y0t2 = TB([128, 4, 128], "spe_y0t")
                for j in range(4):
                    V.scalar_tensor_tensor(y0t2[:, j, :], xc2[j][:, s, :],
                                           col(f"spe_dpc{i}_{j}"),
                                           yps[:, j * 128:(j + 1) * 128],
                                           op0=ALU.mult, op1=ALU.add)
                ygt2 = TB([128, 4, 128], "spe_ygt")
                V.tensor_tensor(ygt2[:], y0t2[:],
                                z2sil[:, :, s * 128:(s + 1) * 128], op=ALU.mult)
                sqy = TB([128, 4, 128], "sq_tmp")
                S.activation(sqy[:].rearrange("p j t -> p (j t)"),
                             ygt2[:].rearrange("p j t -> p (j t)"), AF.Square)
                ssy = psS.tile([1, 128], F32)
                for j in range(4):
                    nc.tensor.matmul(ssy[:], onescol, sqy[:, j, :],
                                     start=(j == 0), stop=(j == 3))
                rl = T([1, 128], "rowA")
                S.activation(rl[:], ssy[:], AF.Ln, bias=epscol[0:1, 0:1],
                             scale=1.0 / 512)
                rrow = TB([1, 128], "rowC")
                S.activation(rrow[:], rl[:], AF.Exp, scale=-0.5)
                rB = P256()
                nc.tensor.matmul(rB[:, 0:128], onesrow1, rrow[:], start=True, stop=True)
                ynt = TB([128, 4, 128], "spe_ynt")
                V.tensor_tensor(ynt[:], ygt2[:],
                                rB[:, 0:128].unsqueeze(1).to_broadcast((128, 4, 128)),
                                op=ALU.mult)
                for ft in range(2):
                    ph2 = P256()
                    for k in range(4):
                        nc.tensor.matmul(ph2[:, 0:128],
                                         ow2[:, k, ft * 128:(ft + 1) * 128],
                                         ynt[:, k, :], start=(k == 0), stop=(k == 3))
                    h2sb = TB([128, 128], "spe_h2sb")
                    S.copy(h2sb[:], ph2[:, 0:128])
                    ptr = PT()
                    nc.tensor.transpose(ptr[:, 0:128], h2sb[:], ident[:])
                    V.tensor_tensor(xs_new[:, s, ft * 128:(ft + 1) * 128], ptr[:, 0:128],
                                    h1[:, s, ft * 128:(ft + 1) * 128], op=ALU.add)
            return xs_new

        # ================= layers =================
        cur = xs
        for i in range(2):
            h1 = spa_mamba(i, cur)
            tap(f"h1_{i}", lambda: tap_batched(h1, [128, L]))
            cur = spe_mamba(i, h1)
            tap(f"xsl{i + 1}", lambda: tap_batched(cur, [128, L]))

        # ================= final LN =================
        xfl = part_ln(cur[:].rearrange("p s t -> p (s t)"), final=True)
        xf = xfl[:].rearrange("p (s t) -> p s t", s=BPC)
        tap("xf", lambda: ([BPC, 128, L],
                           lambda d: [dma(d[s], xf[:, s, :]) for s in range(BPC)], BF))

        # ================= spa attention (center query) =================
        pctr = psS.tile([128, 2], F32)
        for l in range(5):
            nc.tensor.matmul(pctr[:], wt['cprj_pk'][:, l, :], xf[:, :, l],
                             start=(l == 0), stop=(l == 4))
        ctr = TB([128, 2], "at_ctr")
        S.activation(ctr[:], pctr[:], AF.Identity, bias=col("cprj_b"))
        pq = psS.tile([128, 2], F32)
        nc.tensor.matmul(pq[:], wt['aqT'][:], ctr[:], start=True, stop=True)
        qsb = TB([128, 2], "at_q")
        S.activation(qsb[:], pq[:], AF.Identity, bias=col("aq_b"))
        pk = P512()
        nc.tensor.matmul(pk[:], wt['akT'][:], xfl[:], start=True, stop=True)
        Ksb = TB([128, 2, 256], "at_K")
        S.activation(Ksb[:].rearrange("p s t -> p (s t)"), pk[:], AF.Identity,
                     bias=col("ak_b"))
        pv = P512()
        nc.tensor.matmul(pv[:], wt['avT'][:], xfl[:], start=True, stop=True)
        Vsb = TB([128, 2, 256], "at_V")
        S.activation(Vsb[:].rearrange("p s t -> p (s t)"), pv[:], AF.Identity,
                     bias=col("av_b"))
        # batched softmax over both samples (per-head global max is a valid
        # stabilizer; softmax itself stays per-(head,sample))
        plg2 = psS.tile([8, 2, 256], F32)
        for s in range(BPC):
            qd = TB([128, 8], "at_qd")
            V.tensor_tensor(qd[:], qsb[:, s:s + 1].to_broadcast((128, 8)),
                            ct['Emask_q'][:], op=ALU.mult)
            nc.tensor.matmul(plg2[:, s, :], qd[:], Ksb[:, s, :], start=True, stop=True,
                             skip_group_check=True)
        nm = T([8, 1], "at_nm")
        V.tensor_reduce(nm[:], plg2[:].rearrange("p s t -> p (s t)"),
                        axis=AX.X, op=ALU.max, negate=True)
        nm4 = T([8, 1], "at_nm4")
        V.tensor_scalar(nm4[:], nm[:], 0.25, None, op0=ALU.mult)
        ex = T([8, 2, 256], "at_ex")
        S.activation(ex[:].rearrange("p s t -> p (s t)"),
                     plg2[:].rearrange("p s t -> p (s t)"),
                     AF.Exp, bias=nm4[:, 0:1], scale=0.25)
        sm = T([8, 2], "at_sm")
        V.tensor_reduce(sm[:], ex[:], axis=AX.X, op=ALU.add)
        rc = T([8, 2], "at_rc")
        V.reciprocal(rc[:], sm[:])
        aw = TB([8, 2, 256], "at_aw")
        V.tensor_tensor(aw[:], ex[:], rc[:].unsqueeze(2).to_broadcast((8, 2, 256)),
                        op=ALU.mult)
        patB = P512()
        nc.tensor.matmul(patB[:], ct['E_attn'][:], aw[:].rearrange("p s t -> p (s t)"),
                         start=True, stop=True)
        vo = TB([128, 2, 256], "at_vo")
        V.tensor_tensor(vo[:].rearrange("p s t -> p (s t)"),
                        Vsb[:].rearrange("p s t -> p (s t)"), patB[:], op=ALU.mult)
        pao = P512()
        nc.tensor.matmul(pao[:], wt['aoT'][:], vo[:].rearrange("p s t -> p (s t)"),
                         start=True, stop=True)
        xa = TB([128, 2, 256], "xa")
        V.scalar_tensor_tensor(xa[:].rearrange("p s t -> p (s t)"), pao[:],
                               col("ao_b"), xfl[:], op0=ALU.add, op1=ALU.add)
        tap("xa", lambda: tap_batched(xa, [128, L]))

        # ================= spe attention =================
        X2a = TB([128, 2, 2, 128], "x2f_tmp")
        for s in range(BPC):
            for ft in range(2):
                ptr = PT()
                nc.tensor.transpose(ptr[:, 0:128], xa[:, s, ft * 128:(ft + 1) * 128],
                                    ident[:])
                S.copy(X2a[:, s, ft, :], ptr[:, 0:128])
        q2 = TB([128, 2, 2, 128], "sp2_q2")
        k2 = TB([128, 2, 2, 128], "sp2_k2")
        pq2b = P512()
        pk2b = P512()
        for s in range(BPC):
            for ot in range(2):
                for ft in range(2):
                    nc.tensor.matmul(pq2b[:, s * 256 + ot * 128:s * 256 + (ot + 1) * 128],
                                     wt['sqT'][:, ft, ot * 128:(ot + 1) * 128],
                                     X2a[:, s, ft, :], start=(ft == 0), stop=(ft == 1),
                                     skip_group_check=True)
                    nc.tensor.matmul(pk2b[:, s * 256 + ot * 128:s * 256 + (ot + 1) * 128],
                                     wt['skT'][:, ft, ot * 128:(ot + 1) * 128],
                                     X2a[:, s, ft, :], start=(ft == 0), stop=(ft == 1),
                                     skip_group_check=True)
        V.tensor_tensor(q2[:], pq2b[:].rearrange("p (s o c) -> p s o c", s=2, o=2),
                        wt['sqkb'][:, 0].unsqueeze(1).unsqueeze(3)
                        .to_broadcast((128, 2, 2, 128)), op=ALU.add)
        V.tensor_tensor(k2[:], pk2b[:].rearrange("p (s o c) -> p s o c", s=2, o=2),
                        wt['sqkb'][:, 1].unsqueeze(1).unsqueeze(3)
                        .to_broadcast((128, 2, 2, 128)), op=ALU.add)
        # batched v2 / logits / softmax over both samples
        pv2b = P512()
        for s in range(BPC):
            for ft in range(2):
                nc.tensor.matmul(pv2b[:, s * 256:(s + 1) * 256],
                                 X2a[:, s, ft, :], wt['svT'][:, ft, :],
                                 start=(ft == 0), stop=(ft == 1),
                                 skip_group_check=True)
        v2b = TB([128, 2, 256], "sp2_v2")
        V.tensor_tensor(v2b[:], pv2b[:].rearrange("p (s t) -> p s t", s=2),
                        wt['svbB'][:].unsqueeze(1).to_broadcast((128, 2, 256)),
                        op=ALU.add)
        pa2b = P256()
        for s in range(BPC):
            for ot in range(2):
                nc.tensor.matmul(pa2b[:, s * 128:(s + 1) * 128],
                                 q2[:, s, ot, :], k2[:, s, ot, :],
                                 start=(ot == 0), stop=(ot == 1),
                                 skip_group_check=True)
        nm2 = T([128, 1], "sp2_nm")
        V.tensor_reduce(nm2[:], pa2b[:], axis=AX.X, op=ALU.max, negate=True)
        nm16 = T([128, 1], "sp2_nm16")
        V.tensor_scalar(nm16[:], nm2[:], 1.0 / 16, None, op0=ALU.mult)
        ex2 = TB([128, 2, 128], "sp2_ex")
        S.activation(ex2[:].rearrange("p s t -> p (s t)"), pa2b[:],
                     AF.Exp, bias=nm16[:, 0:1], scale=1.0 / 16)
        sm2 = T([128, 2], "sp2_sm")
        V.tensor_reduce(sm2[:], ex2[:], axis=AX.X, op=ALU.add)
        rc2 = T([128, 2], "sp2_rc")
        V.reciprocal(rc2[:], sm2[:])
        a2 = TB([128, 2, 128], "sp2_a2")
        V.tensor_tensor(a2[:], ex2[:], rc2[:].unsqueeze(2).to_broadcast((128, 2, 128)),
                        op=ALU.mult)
        po3b = P512()
        for s in range(BPC):
            pa2T = PT()
            nc.tensor.transpose(pa2T[:, 0:128], a2[:, s, :], ident[:])
            a2T = TB([128, 128], "sp2_a2T")
            S.copy(a2T[:], pa2T[:, 0:128])
            o2 = TB([128, 2, 128], "sp2_o2")
            for ot in range(2):
                po2 = P256()
                nc.tensor.matmul(po2[:, 0:128], v2b[:, s, ot * 128:(ot + 1) * 128], a2T[:],
                                 start=True, stop=True)
                S.copy(o2[:, ot, :], po2[:, 0:128])
            for ot in range(2):
                nc.tensor.matmul(po3b[:, s * 256:(s + 1) * 256],
                                 o2[:, ot, :], wt['soT'][:, ot, :],
                                 start=(ot == 0), stop=(ot == 1),
                                 skip_group_check=True)
        xs2 = TB([128, 2, 256], "xs2")
        for s in range(BPC):
            t3s = TB([128, 256], "sp2_t3")
            V.tensor_tensor(t3s[:], po3b[:, s * 256:(s + 1) * 256],
                            wt['sobB'], op=ALU.add)
            V.tensor_tensor(xs2[:, s, :], t3s[:], xa[:, s, :], op=ALU.add)
        tap("xs2", lambda: tap_batched(xs2, [128, L]))

        # ================= downsample =================
        pds = psD.tile([64, 256], F32, tag="ds", name="ds")
        invf = TB([1, BPC, L], "irow_f")
        dma(invf[:], inv[None, :, :])
        for s in range(BPC):
            invB = P512()
            nc.tensor.matmul(invB[:, 0:L], onesrow1, invf[:, s, :], start=True, stop=True)
            QT = TB([128, 2, 256], "perm_oh")
            for tt in range(2):
                V.tensor_scalar(QT[:, tt, :], invB[:, 0:L], ct['iotaC'][:, tt:tt + 1],
                                None, op0=ALU.is_equal)
            tmv = TB([128, 2, 128], "tm_tmp")
            for tt in range(2):
                ptr = PT()
                nc.tensor.transpose(ptr[:, 0:128], xs2[:, s, tt * 128:(tt + 1) * 128],
                                    ident[:])
                S.copy(tmv[:, tt, :], ptr[:, 0:128])
            pxr = P256()
            for tt in range(2):
                nc.tensor.matmul(pxr[:], tmv[:, tt, :], QT[:, tt, :],
                                 start=(tt == 0), stop=(tt == 1))
            xrp = TB([128, 324], "ds_xrp")
            G.memset(xrp[:], 0.0)
            xr3 = xrp[:].rearrange("p (h w) -> p h w", h=18)
            S.copy(xr3[:, 1:17, 1:17], pxr[:].rearrange("p (h w) -> p h w", h=16))
            for kh in range(3):
                for kw in range(3):
                    k = kh * 3 + kw
                    cmp_ = TB([128, 64], "ds_cmp")
                    (V.tensor_copy if k % 3 != 2 else S.copy)(
                        cmp_[:].rearrange("p (a b) -> p a b", a=8),
                        xr3[:, kh:kh + 16:2, kw:kw + 16:2])
                    nc.tensor.matmul(pds[:, s * 128:(s + 1) * 128],
                                     cmp_[:],
                                     wt['dsw_pk'][:, k, :],
                                     start=(k == 0), stop=(k == 8),
                                     skip_group_check=True)
        for s in range(BPC):
            view = pds[:, s * 128:(s + 1) * 128]
            mus = T([64, 1], "ds_mus")
            V.tensor_reduce(mus[:], view, axis=AX.X, op=ALU.add)
            mean = T([64, 1], "ds_mean")
            V.tensor_scalar(mean[:], mus[:], 1.0 / 128, None, op0=ALU.mult)
            sq = T([64, 128], "ds_sq")
            ss = T([64, 1], "ds_ss")
            S.activation(sq[:], view, AF.Square, accum_out=ss[:, 0:1])
            m2 = T([64, 1], "ds_m2")
            V.tensor_mul(m2[:], mean[:], mean[:])
            var = T([64, 1], "ds_var")
            V.scalar_tensor_tensor(var[:], ss[:], 1.0 / 128, m2[:],
                                   op0=ALU.mult, op1=ALU.subtract)
            lv = T([64, 1], "ds_lv")
            S.activation(lv[:], var[:], AF.Ln, bias=epscol[0:64, 0:1])
            rstd = T([64, 1], "ds_rstd")
            S.activation(rstd[:], lv[:], AF.Exp, scale=-0.5)
            xn = T([64, 128], "ds_xn")
            V.tensor_scalar(xn[:], view, mean[:, 0:1], rstd[:, 0:1],
                            op0=ALU.subtract, op1=ALU.mult)
            t1 = T([64, 128], "ds_t1")
            V.tensor_mul(t1[:], xn[:], wt['ds_ln_wB'][:])
            o1 = T([64, 128], "ds_o1")
            V.tensor_add(o1[:], t1[:], wt['ds_ln_bB'][:])
            dma(out[s].rearrange("h w c -> (h w) c"), o1[:])

        stk.close()
    from concourse.library_overlay import lower_extended_insts
    lower_extended_insts(nc)
    return nc, tap_t


# ---------------------------------------------------------------------------
_CACHE = {}


def _get_program(taps=()):
    key = tuple(sorted(taps))
    if key not in _CACHE:
        _CACHE[key] = build_program(taps)
    return _CACHE[key]


def make_inmaps(inputs, taps=()):
    cst = host_constants()
    w = prep_weights(inputs)
    blob_bf, blob_f32 = pack_blobs(cst, w)
    x = np.asarray(inputs['x'], np.float32).reshape(16, C, L)
    idx = np.asarray(inputs['sorted_index'], np.int32)
    inv = np.argsort(idx, axis=1, kind='stable').astype(np.int32)
    in_maps = []
    for c in range(NCORES):
        m = {'blob_bf': blob_bf, 'blob_f32': blob_f32}
        sl = slice(c * BPC, (c + 1) * BPC)
        m['x2'] = np.ascontiguousarray(x[sl]).astype(BF16NP)
        m['idx'] = np.ascontiguousarray(idx[sl].astype(np.float32)).astype(BF16NP)
        m['inv'] = np.ascontiguousarray(inv[sl].astype(np.float32)).astype(BF16NP)
        in_maps.append(m)
    return in_maps


def run(inputs, taps=(), trace=False):
    nc, tap_t = _get_program(taps)
    in_maps = make_inmaps(inputs, taps)
    res = run_bass_kernel_spmd(nc, in_maps, list(range(NCORES)), trace=trace)
    outs = np.concatenate([np.asarray(r['out'], np.float32) for r in res.results], axis=0)
    tapd = {}
    for name in taps:
        tapd[name] = [np.asarray(r.get('t_' + name), np.float32) for r in res.results]
    return outs, tapd, res


def kernel(**inputs):
    outs, _, _ = run(inputs)
    return outs


# revision 48
# speedup vs baseline: 1.0312x; 1.0286x over previous
"""Trainium2 Bass kernel for nn_Basic_Block_v1 (spatial/spectral Mamba2 block).

Sharding: data-parallel over batch (16 samples) across 8 NeuronCores,
2 samples per core; all parameters replicated. SSD scans are computed in
closed quadratic form on the TensorEngine. All heavy matmuls run in bf16
(1 cycle/row vs 4 for fp32); the cumulative-decay broadcast uses a bf16
hi/lo split to keep fp32-grade cancellation. LayerNorm scale/bias and the
gated-RMS weight are folded into adjacent projection weights on the host;
the Mamba D-residual is applied as a diagonal matmul accumulated into the
same PSUM as the SSD output.
"""
import sys
sys.path.insert(0, '/opt/trn_rl_repo')
import json

import numpy as np
import ml_dtypes

BF16NP = ml_dtypes.bfloat16

import concourse.bass as bass
import concourse.mybir as mybir
from concourse import tile
from concourse import bass_isa
from concourse.bass_utils import run_bass_kernel_spmd

F32 = mybir.dt.float32
BF = mybir.dt.bfloat16
I32 = mybir.dt.int32
AF = mybir.ActivationFunctionType
ALU = mybir.AluOpType
AX = mybir.AxisListType

NCORES = 8
BPC = 2          # batch per core
L = 256          # spatial tokens
C = 128          # channels
H1 = 4           # spa heads
H2 = 8           # spe heads
NST = 64         # d_state
EPS = 1e-5

# ---------------------------------------------------------------------------
# walrus in this container supports only ONE sync-wait per instruction;
# split extra waits emitted by the Tile scheduler onto preceding NoOps.
_WAIT_LIMIT = 1
_orig_to_json = bass.Bass.to_json_bytes


def _fix_block(b, ctr):
    insts = b.get('instructions')
    if insts:
        out = []
        for ins in insts:
            si = ins.get('sync_info')
            waits = (si or {}).get('on_wait') or []
            if len(waits) > _WAIT_LIMIT:
                while len(waits) > _WAIT_LIMIT:
                    chunk, waits = waits[:_WAIT_LIMIT], waits[_WAIT_LIMIT:]
                    ctr[0] += 1
                    out.append({
                        "debug": ins.get("debug"),
                        "engine": ins["engine"],
                        "ins": [],
                        "name": f"I-wsplit{ctr[0]}",
                        "opcode": "NoOp",
                        "outs": [],
                        "text_hint": "wsplit",
                        "sync_info": {"on_update": [], "on_wait": chunk},
                    })
                si['on_wait'] = waits
            out.append(ins)
        b['instructions'] = out
    for sb in b.get('blocks') or []:
        _fix_block(sb, ctr)


def _patched_to_json(self, *a, **k):
    raw = _orig_to_json(self, *a, **k)
    d = json.loads(raw)
    ctr = [0]
    for f in d.get('functions', []):
        for b in f.get('blocks', []):
            _fix_block(b, ctr)
    if ctr[0] == 0:
        return raw
    return json.dumps(d).encode()


bass.Bass.to_json_bytes = _patched_to_json


# ---------------------------------------------------------------------------
def _sincos_2d(dim, Hg):
    def e1(d, pos):
        omega = 1.0 / (10000.0 ** (np.arange(d // 2, dtype=np.float64) / (d / 2.0)))
        out = pos[:, None] * omega[None, :]
        return np.concatenate([np.sin(out), np.cos(out)], axis=-1)
    gh, gw = np.meshgrid(np.arange(Hg), np.arange(Hg), indexing='ij')
    emb = np.concatenate([e1(dim // 2, gh.reshape(-1)), e1(dim // 2, gw.reshape(-1))], axis=-1)
    return emb.astype(np.float32)


def host_constants():
    d = {}
    d['pe_fm'] = np.ascontiguousarray(_sincos_2d(C, 16).T).astype(BF16NP)   # [128, 256]
    d['ident'] = np.eye(128, dtype=np.float32).astype(BF16NP)
    d['ident32'] = np.eye(8, dtype=np.float32)
    iota = np.arange(L, dtype=np.float32)
    d['iotaC'] = np.stack([iota[:128], iota[128:]], axis=1).copy()          # [128, 2] f32
    sidx = np.arange(L)[:, None]
    tidx = np.arange(L)[None, :]
    m = (sidx <= tidx).astype(np.float32)
    d['maskT_spa'] = np.stack([m[:128], m[128:]], axis=1).copy().astype(BF16NP)
    s2 = np.arange(128)[:, None]
    t2 = np.arange(128)[None, :]
    d['maskT_spe'] = (s2 <= t2).astype(np.float32).astype(BF16NP)
    E1 = np.zeros((H1, 2, 128), np.float32)
    for j in range(2):
        for mm in range(128):
            E1[2 * j + mm // 64, j, mm] = 1.0
    d['E_spaJ'] = E1.astype(BF16NP)
    E2 = np.zeros((H2, 4, 128), np.float32)
    for j in range(4):
        for mm in range(128):
            E2[2 * j + mm // 64, j, mm] = 1.0
    d['E_speJ'] = E2.astype(BF16NP)
    EA = np.zeros((8, 128), np.float32)
    for h in range(8):
        EA[h, h * 16:(h + 1) * 16] = 1.0
    d['E_attn'] = EA.astype(BF16NP)
    d['Emask_q'] = EA.T.copy().astype(BF16NP)
    return d


def _col_order():
    cols = []
    for i in range(2):
        cols += [f"spa_dtb{i}", f"spa_negA{i}",
                 f"spa_cb{i}_0", f"spa_cb{i}_1", f"spa_cbBC{i}",
                 f"spa_zb{i}_0", f"spa_zb{i}_1", f"spa_xb{i}_0", f"spa_xb{i}_1",
                 f"spa_BCb{i}", f"spa_dpc{i}_0", f"spa_dpc{i}_1"]
    for i in range(2):
        cols += [f"spe_dtb{i}", f"spe_negA{i}"]
        cols += [f"spe_cb{i}_{j}" for j in range(4)] + [f"spe_cbBC{i}"]
        cols += [f"spe_zb{i}_{j}" for j in range(4)]
        cols += [f"spe_xb{i}_{j}" for j in range(4)]
        cols += [f"spe_BCb{i}"]
        cols += [f"spe_dpc{i}_{j}" for j in range(4)]
    cols += ["lnw_norm", "lnb_norm", "cprj_b", "aq_b", "ak_b", "av_b", "ao_b",
             "sq_b0", "sq_b1", "sk_b0", "sk_b1"]
    return cols


COL_ORDER = _col_order()
CIDX = {k: ix for ix, k in enumerate(COL_ORDER)}


def prep_weights(inp):
    """Host-side layout prep: bf16 casts, LN scale/bias folded into in_proj,
    rms weight folded into out_proj, D as diagonal matrices."""
    f32 = np.float32
    w = {}
    cols = {}
    # ---- spa in_proj with spa_ln fold ----
    w['spa_in_fold'] = np.zeros((2, 128, 644), BF16NP)
    for i in range(2):
        iw = np.asarray(inp['spa_in_w'][i], f32)                 # [644, 128]
        lw = np.asarray(inp['spa_ln_w'][i], f32)
        lb = np.asarray(inp['spa_ln_b'][i], f32)
        br = iw @ lb                                             # [644]
        w['spa_in_fold'][i] = (iw * lw[None, :]).T.astype(BF16NP)
        cols[f"spa_zb{i}_0"] = br[0:128]
        cols[f"spa_zb{i}_1"] = br[128:256]
        cols[f"spa_xb{i}_0"] = br[256:384]
        cols[f"spa_xb{i}_1"] = br[384:512]
        cols[f"spa_BCb{i}"] = br[512:640]
        cols[f"spa_dtb{i}"] = np.asarray(inp['spa_dt_bias'][i], f32) + br[640:644]
        cols[f"spa_negA{i}"] = -np.exp(np.asarray(inp['spa_A_log'][i], f32))
    cv = np.zeros((128, 2, 3, 4), f32)
    for i in range(2):
        cv[:, i, 0] = inp['spa_conv_w'][i, 0:128]
        cv[:, i, 1] = inp['spa_conv_w'][i, 128:256]
        cv[0:64, i, 2] = inp['spa_conv_w'][i, 256:320]
        cv[64:128, i, 2] = inp['spa_conv_w'][i, 320:384]
    w['spa_conv_pk'] = cv
    # out_proj with rms-weight fold: [feat, i, j, out]
    sow = np.transpose(np.asarray(inp['spa_out_w'], f32), (0, 2, 1)).reshape(2, 2, 128, 128)
    sow = sow * np.asarray(inp['spa_rms_w'], f32).reshape(2, 2, 128)[:, :, :, None]
    w['spa_out_pk'] = np.ascontiguousarray(sow.transpose(2, 0, 1, 3)).astype(BF16NP)
    # ---- spe in_proj with spe_ln fold ----
    w['spe_in_pk'] = np.zeros((2, 128, 2, 1160), BF16NP)
    for i in range(2):
        iw = np.asarray(inp['spe_in_w'][i], f32)                 # [1160, 256]
        lw = np.asarray(inp['spe_ln_w'][i], f32)
        lb = np.asarray(inp['spe_ln_b'][i], f32)
        br = iw @ lb
        iwf = (iw * lw[None, :]).T                               # [256, 1160]
        w['spe_in_pk'][i] = iwf.reshape(2, 128, 1160).transpose(1, 0, 2).astype(BF16NP)
        for j in range(4):
            cols[f"spe_zb{i}_{j}"] = br[j * 128:(j + 1) * 128]
            cols[f"spe_xb{i}_{j}"] = br[512 + j * 128:512 + (j + 1) * 128]
        cols[f"spe_BCb{i}"] = br[1024:1152]
        cols[f"spe_dtb{i}"] = np.asarray(inp['spe_dt_bias'][i], f32) + br[1152:1160]
        cols[f"spe_negA{i}"] = -np.exp(np.asarray(inp['spe_A_log'][i], f32))
    cv2 = np.zeros((128, 2, 5, 4), f32)
    for i in range(2):
        for j in range(4):
            cv2[:, i, j] = inp['spe_conv_w'][i, j * 128:(j + 1) * 128]
        cv2[0:64, i, 4] = inp['spe_conv_w'][i, 512:576]
        cv2[64:128, i, 4] = inp['spe_conv_w'][i, 576:640]
    w['spe_conv_pk'] = cv2
    sew = np.transpose(np.asarray(inp['spe_out_w'], f32), (0, 2, 1)).reshape(2, 4, 128, 256)
    sew = sew * np.asarray(inp['spe_rms_w'], f32).reshape(2, 4, 128)[:, :, :, None]
    w['spe_out_pk'] = np.ascontiguousarray(sew.transpose(0, 2, 1, 3)).astype(BF16NP)
    # ---- attention & tail ----
    w['cprj_pk'] = np.ascontiguousarray(
        np.transpose(np.asarray(inp['cprj_w'], f32), (2, 1, 0)).transpose(1, 0, 2)).astype(BF16NP)
    for nm in ('aq', 'ak', 'av', 'ao'):
        w[nm + 'T'] = np.ascontiguousarray(np.asarray(inp[nm + '_w'], f32).T).astype(BF16NP)
    for nm in ('sq', 'sk', 'sv', 'so'):
        wt_ = np.asarray(inp[nm + '_w'], f32).T.reshape(2, 128, 256)
        w[nm + 'T'] = np.ascontiguousarray(wt_.transpose(1, 0, 2)).astype(BF16NP)
    sqkb = np.zeros((128, 2, 2), f32)
    for ot in range(2):
        sqkb[:, 0, ot] = np.asarray(inp['sq_b'], f32)[ot * 128:(ot + 1) * 128]
        sqkb[:, 1, ot] = np.asarray(inp['sk_b'], f32)[ot * 128:(ot + 1) * 128]
    w['sqkb'] = sqkb
    w['svbB'] = np.ascontiguousarray(
        np.broadcast_to(np.asarray(inp['sv_b'], f32)[None, :], (128, 256))).astype(BF16NP)
    w['sobB'] = np.ascontiguousarray(
        np.broadcast_to(np.asarray(inp['so_b'], f32)[None, :], (128, 256))).astype(BF16NP)
    w['dsw_pk'] = np.ascontiguousarray(
        np.asarray(inp['ds_conv_w'], f32).reshape(9, 128, 128).transpose(1, 0, 2)).astype(BF16NP)
    w['ds_ln_wB'] = np.ascontiguousarray(
        np.broadcast_to(np.asarray(inp['ds_ln_w'], f32)[None, :], (64, 128)))
    w['ds_ln_bB'] = np.ascontiguousarray(
        np.broadcast_to(np.asarray(inp['ds_ln_b'], f32)[None, :], (64, 128)))
    # ---- small column-packed params (fp32 scalar operands) ----
    for i in range(2):
        cols[f"spa_cb{i}_0"] = inp['spa_conv_b'][i, 0:128]
        cols[f"spa_cb{i}_1"] = inp['spa_conv_b'][i, 128:256]
        cols[f"spa_cbBC{i}"] = inp['spa_conv_b'][i, 256:384]
        dpc = np.repeat(np.asarray(inp['spa_D'][i], f32), 64)
        cols[f"spa_dpc{i}_0"] = dpc[0:128]
        cols[f"spa_dpc{i}_1"] = dpc[128:256]
        dpc2 = np.repeat(np.asarray(inp['spe_D'][i], f32), 64)
        for j in range(4):
            cols[f"spe_dpc{i}_{j}"] = dpc2[j * 128:(j + 1) * 128]
        for j in range(4):
            cols[f"spe_cb{i}_{j}"] = inp['spe_conv_b'][i, j * 128:(j + 1) * 128]
        cols[f"spe_cbBC{i}"] = inp['spe_conv_b'][i, 512:640]
    cols["lnw_norm"] = inp['norm_w']
    cols["lnb_norm"] = inp['norm_b']
    cols["cprj_b"] = inp['cprj_b']
    for nm in ('aq', 'ak', 'av', 'ao'):
        cols[nm + "_b"] = inp[nm + '_b']
    cols["sq_b0"] = inp['sq_b'][0:128]
    cols["sq_b1"] = inp['sq_b'][128:256]
    cols["sk_b0"] = inp['sk_b'][0:128]
    cols["sk_b1"] = inp['sk_b'][128:256]
    pk = np.zeros((128, len(COL_ORDER)), f32)
    for k, v in cols.items():
        v = np.asarray(v, f32)
        pk[0:v.shape[0], CIDX[k]] = v
    w['colpak'] = pk
    return w


W_SHAPES = {
    'spa_in_fold': ([2, 128, 644], BF), 'spa_conv_pk': ([128, 2, 3, 4], F32),
    'spa_out_pk': ([128, 2, 2, 128], BF),
    'spe_in_pk': ([2, 128, 2, 1160], BF), 'spe_conv_pk': ([128, 2, 5, 4], F32),
    'spe_out_pk': ([2, 128, 4, 256], BF),
    'cprj_pk': ([128, 5, 128], BF),
    'aqT': ([128, 128], BF), 'akT': ([128, 128], BF), 'avT': ([128, 128], BF),
    'aoT': ([128, 128], BF),
    'sqT': ([128, 2, 256], BF), 'skT': ([128, 2, 256], BF), 'svT': ([128, 2, 256], BF),
    'soT': ([128, 2, 256], BF), 'svbB': ([128, 256], BF), 'sobB': ([128, 256], BF),
    'sqkb': ([128, 2, 2], F32),
    'dsw_pk': ([128, 9, 128], BF), 'ds_ln_wB': ([64, 128], F32), 'ds_ln_bB': ([64, 128], F32),
    'colpak': ([128, len(COL_ORDER)], F32),
}

CST_DT = {'pe_fm': BF, 'ident': BF, 'ident32': F32, 'iotaC': F32,
          'maskT_spa': BF, 'maskT_spe': BF, 'E_spaJ': BF, 'E_speJ': BF,
          'E_attn': BF, 'Emask_q': BF}

# ---- blob packing: all params as column ranges of two [128, N] blobs ----
BLOB_BF = [
    ('pe_fm', 128, [256]), ('ident', 128, [128]),
    ('maskT_spa', 128, [2, 256]), ('maskT_spe', 128, [128]),
    ('Emask_q', 128, [8]), ('E_spaJ', 4, [2, 128]), ('E_speJ', 8, [4, 128]),
    ('E_attn', 8, [128]),
    ('spa_in_fold', 128, [2, 644]),
    ('spa_out_pk', 128, [2, 2, 128]),
    ('spe_in_pk', 128, [2, 2, 1160]),
    ('spe_out_pk', 128, [2, 4, 256]),
    ('cprj_pk', 128, [5, 128]),
    ('aqT', 128, [128]), ('akT', 128, [128]), ('avT', 128, [128]),
    ('aoT', 128, [128]),
    ('sqT', 128, [2, 256]), ('skT', 128, [2, 256]), ('svT', 128, [2, 256]),
    ('soT', 128, [2, 256]), ('svbB', 128, [256]), ('sobB', 128, [256]),
    ('dsw_pk', 128, [9, 128]),
]
BLOB_F32 = [
    ('colpak', 128, [len(COL_ORDER)]),
    ('spa_conv_pk', 128, [2, 3, 4]), ('spe_conv_pk', 128, [2, 5, 4]),
    ('iotaC', 128, [2]), ('sqkb', 128, [2, 2]), ('ident32', 8, [8]),
    ('ds_ln_wB', 64, [128]), ('ds_ln_bB', 64, [128]),
]


def _blob_offsets(spec):
    offs = {}
    c = 0
    for name, _, vshape in spec:
        n = int(np.prod(vshape))
        offs[name] = (c, n)
        c += n
    return offs, c


BF_OFFS, BF_COLS = _blob_offsets(BLOB_BF)
F32_OFFS, F32_COLS = _blob_offsets(BLOB_F32)
_LAYER_MAJOR = {'spa_in_fold': (1, 0, 2), 'spe_in_pk': (1, 0, 2, 3),
                'spe_out_pk': (1, 0, 2, 3)}


def pack_blobs(cst, w):
    pool = dict(cst)
    pool.update(w)
    bf = np.zeros((128, BF_COLS), BF16NP)
    f32 = np.zeros((128, F32_COLS), np.float32)
    for spec, blob, offs in ((BLOB_BF, bf, BF_OFFS), (BLOB_F32, f32, F32_OFFS)):
        for name, rows, vshape in spec:
            a = np.asarray(pool[name])
            if name in _LAYER_MAJOR:
                a = np.transpose(a, _LAYER_MAJOR[name])
            off, n = offs[name]
            blob[0:rows, off:off + n] = a.reshape(rows, n)
    return bf, f32


# ---------------------------------------------------------------------------
def build_program(taps=()):
    nc = bass.Bass()

    def din(name, shape, dt=F32):
        return nc.dram_tensor(name, shape, dt, kind="ExternalInput")

    x2 = din("x2", [BPC, C, L], BF)
    idx = din("idx", [BPC, L], BF)
    inv = din("inv", [BPC, L], BF)

    blob_bf_t = din("blob_bf", [128, BF_COLS], BF)
    blob_f32_t = din("blob_f32", [128, F32_COLS], F32)

    out = nc.dram_tensor("out", [BPC, 8, 8, C], F32, kind="ExternalOutput")
    tap_t = {}

    with tile.TileContext(nc) as tc:
        import contextlib
        stk = contextlib.ExitStack()
        sb = stk.enter_context(tc.tile_pool(name="sb", bufs=1))
        ps1 = stk.enter_context(tc.tile_pool(name="ps1", bufs=3, space="PSUM"))
        ps2 = stk.enter_context(tc.tile_pool(name="ps2", bufs=4, space="PSUM"))
        psD = stk.enter_context(tc.tile_pool(name="psD", bufs=1, space="PSUM"))

        class _PSShim:
            def tile(self, shape, dt, tag="small", name="small"):
                return ps2.tile(shape, dt, tag="b256", name="ps_sm")

        psS = _PSShim()

        BUFS2 = {"cv_a0", "cv_a1", "rowA", "rowB", "rowC", "tm_tmp", "ssd_Dt",
                 "ssd_Et", "ssd_MT", "spa_xtm", "spe_xtm", "sq_tmp", "x2f_tmp",
                 "ssd_m0m", "spa_acumT", "spe_acumT", "spa_ygt", "spa_y0t",
                 "spa_ynt", "spe_ygt", "spe_y0t", "spe_ynt", "ds_cmp",
                 "spe_xn", "spe_h2sb", "sp2_a2T", "sp2_o2", "perm_oh", "spa_tsc",
                 "spa_rbs", "spe_rbs", "sp2_t3",
                 "ds_xrp", "xc_0", "xc_1", "xc_2", "xc_3", "xc_BC", "xc_C",
                 "cv_x0", "cv_x1", "cv_x2", "cv_x3", "cv_BC", "aflat", "cv_g0",
                 "cv_g1"}

        def T(shape, tag, dt=F32):
            return sb.tile(shape, dt, tag=tag, name=tag,
                           bufs=2 if tag in BUFS2 else 1)

        def TB(shape, tag):
            return T(shape, tag, BF)

        def P512(tag="b512"):
            return ps1.tile([128, 512], F32, tag="b512", name="b512")

        def P256(tag="b256"):
            return ps2.tile([128, 256], F32, tag="b256", name="b256")

        def PT(tag="bT"):
            return ps2.tile([128, 256], BF, tag="b256", name="bT")

        def tap(name, ap_fn):
            if name in taps:
                shape, writer, dt = ap_fn()
                t = nc.dram_tensor("t_" + name, shape, dt, kind="ExternalOutput")
                tap_t[name] = t
                writer(t)

        dma = nc.sync.dma_start
        V = nc.vector
        S = nc.scalar
        G = nc.gpsimd
        RO = bass_isa.ReduceOp

        # ---------- inputs first, then all params via two blobs ----------
        xb = TB([128, BPC, L], "xb")
        for s in range(BPC):
            dma(xb[:, s, :], x2[s])
        idxf = TB([1, BPC, L], "irow_f")
        dma(idxf[:], idx[None, :, :])
        blob_f32 = T([128, F32_COLS], "blob_f32")
        dma(blob_f32[:], blob_f32_t[:])
        blob_bf = TB([128, BF_COLS], "blob_bf")
        CH = 4096
        for c0 in range(0, BF_COLS, CH):
            c1 = min(c0 + CH, BF_COLS)
            dma(blob_bf[:, c0:c1], blob_bf_t[:, c0:c1])

        def _view(blob, offs, name, rows, vshape):
            off, n = offs[name]
            ap = blob[0:rows, off:off + n]
            if len(vshape) == 2:
                ap = ap.rearrange("p (a b) -> p a b", a=vshape[0])
            elif len(vshape) == 3:
                ap = ap.rearrange("p (a b c) -> p a b c", a=vshape[0], b=vshape[1])
            return ap

        ct = {}
        wt = {}
        for name, rows, vshape in BLOB_BF:
            v = _view(blob_bf, BF_OFFS, name, rows, vshape)
            (ct if name in CST_DT else wt)[name] = v
        for name, rows, vshape in BLOB_F32:
            v = _view(blob_f32, F32_OFFS, name, rows, vshape)
            (ct if name in CST_DT else wt)[name] = v
        colpak = wt['colpak']
        inw_l = [wt['spa_in_fold'][:, i, :] for i in range(2)]
        inw2_l = [wt['spe_in_pk'][:, i, :, :] for i in range(2)]
        ow2_l = [wt['spe_out_pk'][:, i, :, :] for i in range(2)]

        def col(key, p=128):
            return colpak[0:p, CIDX[key]:CIDX[key] + 1]

        ones4 = TB([128, 128], "ones4")
        V.memset(ones4[:], 1.0)
        epscol = T([128, 1], "epscol")
        V.memset(epscol[:], EPS)
        onescol = ones4[:, 0:1]       # [128,1] bf16
        onesrow1 = ones4[0:1, :]      # [1,128] bf16
        ones2 = ones4[0:2, :]         # [2,128] bf16
        ident = ct['ident']
        ident32 = ct['ident32']

        # ---------- stage 0: embed + permute ----------
        x0 = TB([128, BPC, L], "x0")
        V.tensor_tensor(
            x0[:], xb[:],
            ct['pe_fm'][:].unsqueeze(1).to_broadcast((128, BPC, L)),
            op=ALU.add)

        xs = TB([128, BPC, L], "xs")
        for s in range(BPC):
            idxB = P512()
            nc.tensor.matmul(idxB[:, 0:L], onesrow1, idxf[:, s, :], start=True, stop=True)
            PmT = TB([128, 2, L], "perm_oh")
            for st in range(2):
                V.tensor_scalar(PmT[:, st, :], idxB[:, 0:L], ct['iotaC'][:, st:st + 1],
                                None, op0=ALU.is_equal)
            x0tm = TB([128, 2, 128], "tm_tmp")
            for tt in range(2):
                ptr = PT()
                nc.tensor.transpose(ptr[:, 0:128], x0[:, s, tt * 128:(tt + 1) * 128], ident[:])
                S.copy(x0tm[:, tt, :], ptr[:, 0:128])
            pxs = P256()
            for st in range(2):
                nc.tensor.matmul(pxs[:], x0tm[:, st, :], PmT[:, st, :],
                                 start=(st == 0), stop=(st == 1))
            S.copy(xs[:, s, :], pxs[:])

        def tap_batched(t_sb, shape_per_s, dt=BF):
            def writer(dram):
                for s in range(BPC):
                    dma(dram[s], t_sb[:, s, :])
            return ([BPC] + shape_per_s, writer, dt)

        tap("xs0", lambda: tap_batched(xs, [128, L]))

        # ================= shared helpers =================
        def part_ln(xflat, final=False):
            """LayerNorm over the channel (partition) dim of [128, 512] bf16.
            Non-final: scale/bias folded downstream -> returns (x-mu)*rstd."""
            sq = TB([128, 512], "sq_tmp")
            S.activation(sq[:], xflat, AF.Square)
            msum = psS.tile([1, 512], F32)
            nc.tensor.matmul(msum[:], onescol, xflat, start=True, stop=True)
            ssum = psS.tile([1, 512], F32)
            nc.tensor.matmul(ssum[:], onescol, sq[:], start=True, stop=True)
            mu2 = T([1, 512], "rowA")
            S.activation(mu2[:], msum[:], AF.Square, scale=1.0 / 128)
            var = T([1, 512], "rowB")
            V.scalar_tensor_tensor(var[:], ssum[:], 1.0 / 128, mu2[:],
                                   op0=ALU.mult, op1=ALU.subtract)
            lnv = T([1, 512], "rowA")
            S.activation(lnv[:], var[:], AF.Ln, bias=epscol[0:1, 0:1])
            rstd = TB([1, 512], "ln_rstd")
            S.activation(rstd[:], lnv[:], AF.Exp, scale=-0.5)
            r0 = TB([1, 512], "ln_r0")
            V.scalar_tensor_tensor(r0[:], msum[:], -1.0 / 128, rstd[:],
                                   op0=ALU.mult, op1=ALU.mult)
            rstdB = P512()
            nc.tensor.matmul(rstdB[:], onesrow1, rstd[:], start=True, stop=True)
            r0B = P512()
            nc.tensor.matmul(r0B[:], onesrow1, r0[:], start=True, stop=True)
            tmp = TB([128, 512], "ln_tmp")
            V.tensor_tensor(tmp[:], xflat, rstdB[:], op=ALU.mult)
            xln = TB([128, 512], "ln_out")
            if final:
                xn = T([128, 512], "ln_xn")
                V.tensor_tensor(xn[:], tmp[:], r0B[:], op=ALU.add)
                S.activation(xln[:], xn[:], AF.Identity, bias=col("lnb_norm"),
                             scale=col("lnw_norm"))
            else:
                V.tensor_tensor(xln[:], tmp[:], r0B[:], op=ALU.add)
            return xln

        def convchain(buf, wc, cb, P, W, tag, E=None):
            """Causal depthwise conv (k=4) + silu. buf [P, 2, W+3] fp32 ->
            bf16 output. E selects the elementwise engine (vector/gpsimd)."""
            E = E or V
            a0 = T([P, 2, W], "cv_a0" if E is V else "cv_g0")
            E.tensor_scalar(a0[:], buf[:, :, 0:W], wc[:, 0:1], None, op0=ALU.mult)
            a1 = T([P, 2, W], "cv_a1" if E is V else "cv_g1")
            E.scalar_tensor_tensor(a1[:], buf[:, :, 1:W + 1], wc[:, 1:2], a0[:],
                                   op0=ALU.mult, op1=ALU.add)
            a2 = T([P, 2, W], "cv_a0" if E is V else "cv_g0")
            E.scalar_tensor_tensor(a2[:], buf[:, :, 2:W + 2], wc[:, 2:3], a1[:],
                                   op0=ALU.mult, op1=ALU.add)
            a3 = T([P, 2, W], "cv_a1" if E is V else "cv_g1")
            E.scalar_tensor_tensor(a3[:], buf[:, :, 3:W + 3], wc[:, 3:4], a2[:],
                                   op0=ALU.mult, op1=ALU.add)
            xc = TB([P, 2, W], tag)
            S.activation(xc[:], a3[:], AF.Silu, bias=cb[:, 0:1])
            return xc

        def dt_ladder(pdt, nh, NW, dtb_key, negA_key):
            """softplus(dt+bias) -> dtv_bf (matmul operand), acum f32,
            hi/lo bf16 rows for the decay broadcast."""
            e1 = T([nh, NW], "rowA")
            S.activation(e1[:], pdt[:], AF.Exp, bias=col(dtb_key, nh))
            e1p = T([nh, NW], "rowB")
            V.tensor_scalar(e1p[:], e1[:], 1.0, None, op0=ALU.add)
            dtv = T([nh, NW], "mb_dtv")
            S.activation(dtv[:], e1p[:], AF.Ln)

            dtv_bf = TB([nh, NW], "mb_dtvbf")
            S.copy(dtv_bf[:], dtv[:])
            dtA = T([nh, NW], "rowA")
            V.tensor_scalar(dtA[:], dtv[:], col(negA_key, nh), None, op0=ALU.mult)
            acum = T([nh, NW], "mb_acum")
            seg = NW // BPC
            for s in range(BPC):
                V.tensor_tensor_scan(acum[:, s * seg:(s + 1) * seg],
                                     dtA[:, s * seg:(s + 1) * seg],
                                     dtA[:, s * seg:(s + 1) * seg], 0.0,
                                     op0=ALU.add, op1=ALU.bypass)
            hi = TB([nh, NW], "acum_hi")
            S.copy(hi[:], acum[:])
            lo = TB([nh, NW], "acum_lo")
            G.tensor_tensor(lo[:], acum[:], hi[:], op=ALU.subtract)
            hilo = TB([2, BPC, 1024], "aflat")
            for s in range(BPC):
                dma(hilo[0:1, s, :].rearrange("o (p f) -> o p f", p=nh),
                    hi[:, s * seg:(s + 1) * seg])
                dma(hilo[1:2, s, :].rearrange("o (p f) -> o p f", p=nh),
                    lo[:, s * seg:(s + 1) * seg])
            return dtv_bf, acum, hilo

        # ================= spa mamba =================
        def spa_mamba(i, xs):
            xflat = xs[:].rearrange("p s t -> p (s t)")
            xln = part_ln(xflat)
            tap(f"xln{i}", lambda: ([128, 512], lambda d: dma(d[:], xln[:]), BF))
            inw = inw_l[i][:]
            # dt first: its Exp/Ln then run before the silu cluster
            pdt = psS.tile([4, 512], F32)
            nc.tensor.matmul(pdt[:], inw[:, 640:644], xln[:], start=True, stop=True)
            dtv_bf, acum, hilo = dt_ladder(pdt, 4, 512, f"spa_dtb{i}", f"spa_negA{i}")
            cvx = []
            for j in range(2):
                px = P512()
                nc.tensor.matmul(px[:], inw[:, 256 + j * 128:256 + (j + 1) * 128], xln[:],
                                 start=True, stop=True)
                buf = T([128, 2, 259], f"cv_x{j}")
                G.memset(buf[:, :, 0:3], 0.0)
                S.activation(buf[:, :, 3:259], px[:].rearrange("p (s t) -> p s t", s=2),
                             AF.Identity, bias=col(f"spa_xb{i}_{j}"))
                cvx.append(buf)
            # B and C merged: one matmul, one buf, one chain (B rows 0:63, C 64:127)
            pbc = P512()
            nc.tensor.matmul(pbc[:], inw[:, 512:640], xln[:], start=True, stop=True)
            bufBC = T([128, 2, 259], "cv_BC")
            G.memset(bufBC[:, :, 0:3], 0.0)
            S.activation(bufBC[:, :, 3:259], pbc[:].rearrange("p (s t) -> p s t", s=2),
                         AF.Identity, bias=col(f"spa_BCb{i}"))
            # conv + silu (silu table region)
            xc = []
            for j in range(2):
                xc.append(convchain(cvx[j], wt['spa_conv_pk'][:, i, j, :],
                                    col(f"spa_cb{i}_{j}"), 128, 256, f"xc_{j}"))
            xcBC = convchain(bufBC, wt['spa_conv_pk'][:, i, 2, :],
                             col(f"spa_cbBC{i}"), 128, 256, "xc_BC")
            # z -> silu emitted after convs so dt's Ln precedes the silu cluster
            zsil = TB([128, 2, 512], "mb_zsil")
            for j in range(2):
                pz = P512()
                nc.tensor.matmul(pz[:], inw[:, j * 128:(j + 1) * 128], xln[:],
                                 start=True, stop=True)
                S.activation(zsil[:, j, :], pz[:], AF.Silu, bias=col(f"spa_zb{i}_{j}"))
            # C half to a partition-0-based tile (matmul needs equal base partitions)
            xcC = TB([64, 2, 256], "xc_C")
            dma(xcC[:], xcBC[64:128, :, :])
            if i == 0:
                tap("dbg_zsil", lambda: ([128, 1024], lambda d: dma(
                    d[:], zsil[:].rearrange("p j t -> p (j t)")), BF))
                tap("dbg_xc0", lambda: ([128, 512], lambda d: dma(
                    d[:], xc[0][:].rearrange("p s t -> p (s t)")), BF))
                tap("dbg_xcBC", lambda: ([128, 512], lambda d: dma(
                    d[:], xcBC[:].rearrange("p s t -> p (s t)")), BF))
                tap("dbg_xcC", lambda: ([64, 512], lambda d: dma(
                    d[:], xcC[:].rearrange("p s t -> p (s t)")), BF))
                tap("dbg_dtv", lambda: ([4, 512], lambda d: dma(d[:], dtv_bf[:]), BF))
                tap("dbg_acum", lambda: ([4, 512], lambda d: dma(d[:], acum[:]), F32))
            # dt-scaled x (feature-major)
            xp = TB([128, 2, 512], "mb_xp")
            for j in range(2):
                pdb = P512()
                nc.tensor.matmul(pdb[:], ct['E_spaJ'][:, j, :], dtv_bf[:], start=True, stop=True)
                V.tensor_tensor(xp[:, j, :], xc[j][:].rearrange("p s t -> p (s t)"), pdb[:],
                                op=ALU.mult)
            if i == 0:
                tap("dbg_xp", lambda: ([128, 1024], lambda d: dma(
                    d[:], xp[:].rearrange("p j t -> p (j t)")), BF))
            h1 = TB([128, 2, 256], "h1")
            for s in range(BPC):
                xtm = TB([128, 2, 256], "spa_xtm")
                for st in range(2):
                    for j in range(2):
                        ptr = PT()
                        nc.tensor.transpose(
                            ptr[:, 0:128],
                            xp[:, j, s * 256 + st * 128: s * 256 + (st + 1) * 128],
                            ident[:])
                        V.tensor_copy(xtm[:, st, j * 128:(j + 1) * 128], ptr[:, 0:128])
                m0m = TB([128, 2, 256], "ssd_m0m")
                for st in range(2):
                    pm0 = P256()
                    nc.tensor.matmul(pm0[:], xcBC[0:64, s, st * 128:(st + 1) * 128],
                                     xcC[:, s, :], start=True, stop=True)
                    V.tensor_tensor(m0m[:, st, :], pm0[:], ct['maskT_spa'][:, st, :],
                                    op=ALU.mult)
                acumT = T([128, 2, 4], "spa_acumT")
                for tt in range(2):
                    ptr2 = P256()
                    nc.tensor.transpose(ptr2[:, 0:4],
                                        acum[:, s * 256 + tt * 128: s * 256 + (tt + 1) * 128],
                                        ident32[0:4, 0:4])
                    S.copy(acumT[:, tt, :], ptr2[:, 0:4])
                pb1 = P512()
                nc.tensor.matmul(pb1[:], ones2, hilo[:, s, 0:512], start=True, stop=True)
                pb2 = P512()
                nc.tensor.matmul(pb2[:], ones2, hilo[:, s, 512:1024], start=True, stop=True)
                yps = P512()
                for st in range(2):
                    Dt = T([128, 4, 256], "ssd_Dt")
                    for h in range(H1):
                        pbx = pb1 if h < 2 else pb2
                        V.tensor_scalar(Dt[:, h, :],
                                        pbx[:, (h % 2) * 256:(h % 2 + 1) * 256],
                                        acumT[:, st, h:h + 1], 0.0,
                                        op0=ALU.subtract, op1=ALU.min)
                    Et = TB([128, 4, 256], "ssd_Et")
                    S.activation(Et[:].rearrange("p h t -> p (h t)"),
                                 Dt[:].rearrange("p h t -> p (h t)"), AF.Exp)
                    MT = TB([128, 4, 256], "ssd_MT")
                    V.tensor_tensor(MT[:], Et[:],
                                    m0m[:, st, :].unsqueeze(1).to_broadcast((128, 4, 256)),
                                    op=ALU.mult)
                    for h in range(H1):
                        nc.tensor.matmul(
                            yps[(h % 2) * 64:(h % 2) * 64 + 64,
                                (h // 2) * 256:(h // 2) * 256 + 256],
                            xtm[:, st, h * 64:(h + 1) * 64],
                            MT[:, h, :],
                            start=(st == 0), stop=(st == 1),
                            tile_position=(0, (h % 2) * 64),
                            skip_group_check=True)
                if i == 0 and s == 0:
                    tap("dbg_xtm", lambda: ([128, 512], lambda d: dma(
                        d[:], xtm[:].rearrange("p s t -> p (s t)")), BF))
                    tap("dbg_m0m", lambda: ([128, 512], lambda d: dma(
                        d[:], m0m[:].rearrange("p s t -> p (s t)")), BF))
                    if "dbg_yps" in taps:
                        ypc = T([128, 512], "dbg_ypc")
                        S.copy(ypc[:], yps[:])
                        tap("dbg_yps", lambda: ([128, 512], lambda d: dma(
                            d[:], ypc[:]), F32))
                y0t = TB([128, 2, 256], "spa_y0t")
                for j in range(2):
                    V.scalar_tensor_tensor(y0t[:, j, :], xc[j][:, s, :],
                                           col(f"spa_dpc{i}_{j}"),
                                           yps[:, j * 256:(j + 1) * 256],
                                           op0=ALU.mult, op1=ALU.add)
                ygt = TB([128, 2, 256], "spa_ygt")
                V.tensor_tensor(ygt[:], y0t[:],
                                zsil[:, :, s * 256:(s + 1) * 256], op=ALU.mult)
                if i == 0 and s == 0:
                    tap("dbg_ygt", lambda: ([128, 512], lambda d: dma(
                        d[:], ygt[:].rearrange("p j t -> p (j t)")), BF))
                sqy = TB([128, 2, 256], "sq_tmp")
                S.activation(sqy[:].rearrange("p j t -> p (j t)"),
                             ygt[:].rearrange("p j t -> p (j t)"), AF.Square)
                ssy = psS.tile([1, 256], F32)
                for j in range(2):
                    nc.tensor.matmul(ssy[:], onescol, sqy[:, j, :],
                                     start=(j == 0), stop=(j == 1))
                rl = T([1, 256], "rowA")
                S.activation(rl[:], ssy[:], AF.Ln, bias=epscol[0:1, 0:1],
                             scale=1.0 / 256)
                rrow = TB([1, 256], "rowC")
                S.activation(rrow[:], rl[:], AF.Exp, scale=-0.5)
                pop = P256()
                for j in range(2):
                    nc.tensor.matmul(pop[:], wt['spa_out_pk'][:, i, j, :], ygt[:, j, :],
                                     start=(j == 0), stop=(j == 1))
                rB = P256()
                nc.tensor.matmul(rB[:], onesrow1, rrow[:], start=True, stop=True)
                # rms scale is per-token -> commutes with the linear out-proj
                rBs = TB([128, 256], "spa_rbs")
                S.copy(rBs[:], rB[:])
                tsc = TB([128, 256], "spa_tsc")
                V.tensor_tensor(tsc[:], pop[:], rBs[:], op=ALU.mult)
                V.tensor_tensor(h1[:, s, :], tsc[:], xs[:, s, :], op=ALU.add)
            return h1

        # ================= spe mamba =================
        def spe_mamba(i, h1):
            mus = T([128, 2], "spe_mus")
            V.tensor_reduce(mus[:], h1[:], axis=AX.X, op=ALU.add)
            sqd = TB([128, 2, 256], "sq_tmp")
            ss2 = T([128, 2], "spe_ss2")
            for s in range(BPC):
                S.activation(sqd[:, s, :], h1[:, s, :], AF.Square,
                             accum_out=ss2[:, s:s + 1])
            mean = T([128, 2], "spe_mean")
            V.tensor_scalar(mean[:], mus[:], 1.0 / 256, None, op0=ALU.mult)
            m2 = T([128, 2], "spe_m2")
            S.activation(m2[:], mean[:], AF.Square)
            var2 = T([128, 2], "spe_var")
            V.scalar_tensor_tensor(var2[:], ss2[:], 1.0 / 256, m2[:],
                                   op0=ALU.mult, op1=ALU.subtract)
            l2t = T([128, 2], "spe_l2")
            S.activation(l2t[:], var2[:], AF.Ln, bias=epscol[:, 0:1])
            rstd2 = T([128, 2], "spe_rstd")
            S.activation(rstd2[:], l2t[:], AF.Exp, scale=-0.5)
            X2f = TB([128, 2, 2, 128], "x2f_tmp")
            for s in range(BPC):
                xn = TB([128, 256], "spe_xn")
                V.tensor_scalar(xn[:], h1[:, s, :], mean[:, s:s + 1], rstd2[:, s:s + 1],
                                op0=ALU.subtract, op1=ALU.mult)
                for ft in range(2):
                    ptr = PT()
                    nc.tensor.transpose(ptr[:, 0:128], xn[:, ft * 128:(ft + 1) * 128],
                                        ident[:])
                    V.tensor_copy(X2f[:, s, ft, :], ptr[:, 0:128])
            inw2 = inw2_l[i][:]
            ow2 = ow2_l[i][:]

            def mm2(out_ap, off, width):
                for k in range(2):
                    nc.tensor.matmul(out_ap,
                                     inw2[:, k, off:off + width],
                                     X2f[:, :, k, :],
                                     start=(k == 0), stop=(k == 1))
            # dt first (exp/ln before the silu cluster)
            pdt = psS.tile([8, 256], F32)
            mm2(pdt[:], 1152, 8)
            dtv_bf, acum, hilo = dt_ladder(pdt, 8, 256, f"spe_dtb{i}", f"spe_negA{i}")
            cvx2 = []
            for j in range(4):
                px = P256()
                mm2(px[:], 512 + j * 128, 128)
                buf = T([128, 2, 131], f"cv_x{j}")
                G.memset(buf[:, :, 0:3], 0.0)
                S.activation(buf[:, :, 3:131], px[:].rearrange("p (s t) -> p s t", s=2),
                             AF.Identity, bias=col(f"spe_xb{i}_{j}"))
                cvx2.append(buf)
            pbc = P256()
            mm2(pbc[:], 1024, 128)
            bufBC = T([128, 2, 131], "cv_BC")
            G.memset(bufBC[:, :, 0:3], 0.0)
            S.activation(bufBC[:, :, 3:131], pbc[:].rearrange("p (s t) -> p s t", s=2),
                         AF.Identity, bias=col(f"spe_BCb{i}"))
            xc2 = []
            for j in range(4):
                xc2.append(convchain(cvx2[j], wt['spe_conv_pk'][:, i, j, :],
                                     col(f"spe_cb{i}_{j}"), 128, 128, f"xc_{j}"))
            xcBC = convchain(bufBC, wt['spe_conv_pk'][:, i, 4, :],
                             col(f"spe_cbBC{i}"), 128, 128, "xc_BC")
            z2sil = TB([128, 4, 256], "mb_zsil")
            for j in range(4):
                pz = P256()
                mm2(pz[:], j * 128, 128)
                S.activation(z2sil[:, j, :], pz[:], AF.Silu, bias=col(f"spe_zb{i}_{j}"))
            xcC = TB([64, 2, 128], "xc_C")
            dma(xcC[:], xcBC[64:128, :, :])
            xp2 = TB([128, 4, 256], "mb_xp")
            for j in range(4):
                pdb = P256()
                nc.tensor.matmul(pdb[:], ct['E_speJ'][:, j, :], dtv_bf[:], start=True, stop=True)
                V.tensor_tensor(xp2[:, j, :], xc2[j][:].rearrange("p s t -> p (s t)"), pdb[:],
                                op=ALU.mult)
            xs_new = TB([128, 2, 256], "xs")
            for s in range(BPC):
                xtm2 = TB([128, 512], "spe_xtm")
                for j in range(4):
                    ptr = PT()
                    nc.tensor.transpose(ptr[:, 0:128],
                                        xp2[:, j, s * 128:(s + 1) * 128], ident[:])
                    V.tensor_copy(xtm2[:, j * 128:(j + 1) * 128], ptr[:, 0:128])
                m0m2 = TB([128, 128], "ssd_m0m")
                pm0 = P256()
                nc.tensor.matmul(pm0[:, 0:128], xcBC[0:64, s, :], xcC[:, s, :],
                                 start=True, stop=True)
                V.tensor_tensor(m0m2[:], pm0[:, 0:128], ct['maskT_spe'][:], op=ALU.mult)
                acumT = T([128, 8], "spe_acumT")
                ptr2 = P256()
                nc.tensor.transpose(ptr2[:, 0:8], acum[:, s * 128:(s + 1) * 128],
                                    ident32[0:8, 0:8])
                S.copy(acumT[:], ptr2[:, 0:8])
                pb1 = P512()
                nc.tensor.matmul(pb1[:], ones2, hilo[:, s, 0:512], start=True, stop=True)
                pb2 = P512()
                nc.tensor.matmul(pb2[:], ones2, hilo[:, s, 512:1024], start=True, stop=True)
                yps = P512()
                Dt = T([128, 8, 128], "ssd_Dt")
                for h in range(H2):
                    pbx = pb1 if h < 4 else pb2
                    V.tensor_scalar(Dt[:, h, :],
                                    pbx[:, (h % 4) * 128:(h % 4 + 1) * 128],
                                    acumT[:, h:h + 1], 0.0,
                                    op0=ALU.subtract, op1=ALU.min)
                Et = TB([128, 8, 128], "ssd_Et")
                S.activation(Et[:].rearrange("p h t -> p (h t)"),
                             Dt[:].rearrange("p h t -> p (h t)"), AF.Exp)
                MT = TB([128, 8, 128], "ssd_MT")
                V.tensor_tensor(MT[:], Et[:],
                                m0m2[:].unsqueeze(1).to_broadcast((128, 8, 128)),
                                op=ALU.mult)
                for j in range(4):
                    for hh in range(2):
                        h = 2 * j + hh
                        nc.tensor.matmul(yps[hh * 64:hh * 64 + 64, j * 128:(j + 1) * 128],
                                         xtm2[:, h * 64:(h + 1) * 64],
                                         MT[:, h, :], start=True, stop=True,
                                         tile_position=(0, hh * 64),
                                         skip_group_check=True)
                y0t2 = TB([128, 4, 128], "spe_y0t")
                for j in range(4):
                    V.scalar_tensor_tensor(y0t2[:, j, :], xc2[j][:, s, :],
                                           col(f"spe_dpc{i}_{j}"),
                                           yps[:, j * 128:(j + 1) * 128],
                                           op0=ALU.mult, op1=ALU.add)
                ygt2 = TB([128, 4, 128], "spe_ygt")
                V.tensor_tensor(ygt2[:], y0t2[:],
                                z2sil[:, :, s * 128:(s + 1) * 128], op=ALU.mult)
                sqy = TB([128, 4, 128], "sq_tmp")
                S.activation(sqy[:].rearrange("p j t -> p (j t)"),
                             ygt2[:].rearrange("p j t -> p (j t)"), AF.Square)
                ssy = psS.tile([1, 128], F32)
                for j in range(4):
                    nc.tensor.matmul(ssy[:], onescol, sqy[:, j, :],
                                     start=(j == 0), stop=(j == 3))
                rl = T([1, 128], "rowA")
                S.activation(rl[:], ssy[:], AF.Ln, bias=epscol[0:1, 0:1],
                             scale=1.0 / 512)
                rrow = TB([1, 128], "rowC")
                S.activation(rrow[:], rl[:], AF.Exp, scale=-0.5)
                rB = P256()
                nc.tensor.matmul(rB[:, 0:128], onesrow1, rrow[:], start=True, stop=True)
                rBs2 = TB([128, 128], "spe_rbs")
                S.copy(rBs2[:], rB[:, 0:128])
                for ft in range(2):
                    ph2 = P256()
                    for k in range(4):
                        nc.tensor.matmul(ph2[:, 0:128],
                                         ow2[:, k, ft * 128:(ft + 1) * 128],
                                         ygt2[:, k, :], start=(k == 0), stop=(k == 3))
                    h2sb = TB([128, 128], "spe_h2sb")
                    V.tensor_tensor(h2sb[:], ph2[:, 0:128], rBs2[:], op=ALU.mult)
                    ptr = PT()
                    nc.tensor.transpose(ptr[:, 0:128], h2sb[:], ident[:])
                    V.tensor_tensor(xs_new[:, s, ft * 128:(ft + 1) * 128], ptr[:, 0:128],
                                    h1[:, s, ft * 128:(ft + 1) * 128], op=ALU.add)
            return xs_new

        # ================= layers =================
        cur = xs
        for i in range(2):
            h1 = spa_mamba(i, cur)
            tap(f"h1_{i}", lambda: tap_batched(h1, [128, L]))
            cur = spe_mamba(i, h1)
            tap(f"xsl{i + 1}", lambda: tap_batched(cur, [128, L]))

        # ================= final LN =================
        xfl = part_ln(cur[:].rearrange("p s t -> p (s t)"), final=True)
        xf = xfl[:].rearrange("p (s t) -> p s t", s=BPC)
        tap("xf", lambda: ([BPC, 128, L],
                           lambda d: [dma(d[s], xf[:, s, :]) for s in range(BPC)], BF))

        # ================= spa attention (center query) =================
        pctr = psS.tile([128, 2], F32)
        for l in range(5):
            nc.tensor.matmul(pctr[:], wt['cprj_pk'][:, l, :], xf[:, :, l],
                             start=(l == 0), stop=(l == 4))
        ctr = TB([128, 2], "at_ctr")
        S.activation(ctr[:], pctr[:], AF.Identity, bias=col("cprj_b"))
        pq = psS.tile([128, 2], F32)
        nc.tensor.matmul(pq[:], wt['aqT'][:], ctr[:], start=True, stop=True)
        qsb = TB([128, 2], "at_q")
        S.activation(qsb[:], pq[:], AF.Identity, bias=col("aq_b"))
        pk = P512()
        nc.tensor.matmul(pk[:], wt['akT'][:], xfl[:], start=True, stop=True)
        Ksb = TB([128, 2, 256], "at_K")
        S.activation(Ksb[:].rearrange("p s t -> p (s t)"), pk[:], AF.Identity,
                     bias=col("ak_b"))
        pv = P512()
        nc.tensor.matmul(pv[:], wt['avT'][:], xfl[:], start=True, stop=True)
        Vsb = TB([128, 2, 256], "at_V")
        S.activation(Vsb[:].rearrange("p s t -> p (s t)"), pv[:], AF.Identity,
                     bias=col("av_b"))
        # batched softmax over both samples (per-head global max is a valid
        # stabilizer; softmax itself stays per-(head,sample))
        plg2 = psS.tile([8, 2, 256], F32)
        for s in range(BPC):
            qd = TB([128, 8], "at_qd")
            V.tensor_tensor(qd[:], qsb[:, s:s + 1].to_broadcast((128, 8)),
                            ct['Emask_q'][:], op=ALU.mult)
            nc.tensor.matmul(plg2[:, s, :], qd[:], Ksb[:, s, :], start=True, stop=True,
                             skip_group_check=True)
        nm = T([8, 1], "at_nm")
        V.tensor_reduce(nm[:], plg2[:].rearrange("p s t -> p (s t)"),
                        axis=AX.X, op=ALU.max, negate=True)
        nm4 = T([8, 1], "at_nm4")
        V.tensor_scalar(nm4[:], nm[:], 0.25, None, op0=ALU.mult)
        ex = T([8, 2, 256], "at_ex")
        S.activation(ex[:].rearrange("p s t -> p (s t)"),
                     plg2[:].rearrange("p s t -> p (s t)"),
                     AF.Exp, bias=nm4[:, 0:1], scale=0.25)
        sm = T([8, 2], "at_sm")
        V.tensor_reduce(sm[:], ex[:], axis=AX.X, op=ALU.add)
        rc = T([8, 2], "at_rc")
        V.reciprocal(rc[:], sm[:])
        aw = TB([8, 2, 256], "at_aw")
        V.tensor_tensor(aw[:], ex[:], rc[:].unsqueeze(2).to_broadcast((8, 2, 256)),
                        op=ALU.mult)
        patB = P512()
        nc.tensor.matmul(patB[:], ct['E_attn'][:], aw[:].rearrange("p s t -> p (s t)"),
                         start=True, stop=True)
        vo = TB([128, 2, 256], "at_vo")
        V.tensor_tensor(vo[:].rearrange("p s t -> p (s t)"),
                        Vsb[:].rearrange("p s t -> p (s t)"), patB[:], op=ALU.mult)
        pao = P512()
        nc.tensor.matmul(pao[:], wt['aoT'][:], vo[:].rearrange("p s t -> p (s t)"),
                         start=True, stop=True)
        xa = TB([128, 2, 256], "xa")
        V.scalar_tensor_tensor(xa[:].rearrange("p s t -> p (s t)"), pao[:],
                               col("ao_b"), xfl[:], op0=ALU.add, op1=ALU.add)
        tap("xa", lambda: tap_batched(xa, [128, L]))

        # ================= spe attention =================
        X2a = TB([128, 2, 2, 128], "x2f_tmp")
        for s in range(BPC):
            for ft in range(2):
                ptr = PT()
                nc.tensor.transpose(ptr[:, 0:128], xa[:, s, ft * 128:(ft + 1) * 128],
                                    ident[:])
                S.copy(X2a[:, s, ft, :], ptr[:, 0:128])
        q2 = TB([128, 2, 2, 128], "sp2_q2")
        k2 = TB([128, 2, 2, 128], "sp2_k2")
        pq2b = P512()
        pk2b = P512()
        for s in range(BPC):
            for ot in range(2):
                for ft in range(2):
                    nc.tensor.matmul(pq2b[:, s * 256 + ot * 128:s * 256 + (ot + 1) * 128],
                                     wt['sqT'][:, ft, ot * 128:(ot + 1) * 128],
                                     X2a[:, s, ft, :], start=(ft == 0), stop=(ft == 1),
                                     skip_group_check=True)
                    nc.tensor.matmul(pk2b[:, s * 256 + ot * 128:s * 256 + (ot + 1) * 128],
                                     wt['skT'][:, ft, ot * 128:(ot + 1) * 128],
                                     X2a[:, s, ft, :], start=(ft == 0), stop=(ft == 1),
                                     skip_group_check=True)
        V.tensor_tensor(q2[:], pq2b[:].rearrange("p (s o c) -> p s o c", s=2, o=2),
                        wt['sqkb'][:, 0].unsqueeze(1).unsqueeze(3)
                        .to_broadcast((128, 2, 2, 128)), op=ALU.add)
        V.tensor_tensor(k2[:], pk2b[:].rearrange("p (s o c) -> p s o c", s=2, o=2),
                        wt['sqkb'][:, 1].unsqueeze(1).unsqueeze(3)
                        .to_broadcast((128, 2, 2, 128)), op=ALU.add)
        # batched v2 / logits / softmax over both samples
        pv2b = P512()
        for s in range(BPC):
            for ft in range(2):
                nc.tensor.matmul(pv2b[:, s * 256:(s + 1) * 256],
                                 X2a[:, s, ft, :], wt['svT'][:, ft, :],
                                 start=(ft == 0), stop=(ft == 1),
                                 skip_group_check=True)
        v2b = TB([128, 2, 256], "sp2_v2")
        V.tensor_tensor(v2b[:], pv2b[:].rearrange("p (s t) -> p s t", s=2),
                        wt['svbB'][:].unsqueeze(1).to_broadcast((128, 2, 256)),
                        op=ALU.add)
        pa2b = P256()
        for s in range(BPC):
            for ot in range(2):
                nc.tensor.matmul(pa2b[:, s * 128:(s + 1) * 128],
                                 q2[:, s, ot, :], k2[:, s, ot, :],
                                 start=(ot == 0), stop=(ot == 1),
                                 skip_group_check=True)
        nm2 = T([128, 1], "sp2_nm")
        V.tensor_reduce(nm2[:], pa2b[:], axis=AX.X, op=ALU.max, negate=True)
        nm16 = T([128, 1], "sp2_nm16")
        V.tensor_scalar(nm16[:], nm2[:], 1.0 / 16, None, op0=ALU.mult)
        ex2 = TB([128, 2, 128], "sp2_ex")
        S.activation(ex2[:].rearrange("p s t -> p (s t)"), pa2b[:],
                     AF.Exp, bias=nm16[:, 0:1], scale=1.0 / 16)
        sm2 = T([128, 2], "sp2_sm")
        V.tensor_reduce(sm2[:], ex2[:], axis=AX.X, op=ALU.add)
        rc2 = T([128, 2], "sp2_rc")
        V.reciprocal(rc2[:], sm2[:])
        a2 = TB([128, 2, 128], "sp2_a2")
        V.tensor_tensor(a2[:], ex2[:], rc2[:].unsqueeze(2).to_broadcast((128, 2, 128)),
                        op=ALU.mult)
        po3b = P512()
        for s in range(BPC):
            pa2T = PT()
            nc.tensor.transpose(pa2T[:, 0:128], a2[:, s, :], ident[:])
            a2T = TB([128, 128], "sp2_a2T")
            S.copy(a2T[:], pa2T[:, 0:128])
            o2 = TB([128, 2, 128], "sp2_o2")
            for ot in range(2):
                po2 = P256()
                nc.tensor.matmul(po2[:, 0:128], v2b[:, s, ot * 128:(ot + 1) * 128], a2T[:],
                                 start=True, stop=True)
                S.copy(o2[:, ot, :], po2[:, 0:128])
            for ot in range(2):
                nc.tensor.matmul(po3b[:, s * 256:(s + 1) * 256],
                                 o2[:, ot, :], wt['soT'][:, ot, :],
                                 start=(ot == 0), stop=(ot == 1),
                                 skip_group_check=True)
        xs2 = TB([128, 2, 256], "xs2")
        for s in range(BPC):
            t3s = TB([128, 256], "sp2_t3")
            V.tensor_tensor(t3s[:], po3b[:, s * 256:(s + 1) * 256],
                            wt['sobB'], op=ALU.add)
            V.tensor_tensor(xs2[:, s, :], t3s[:], xa[:, s, :], op=ALU.add)
        tap("xs2", lambda: tap_batched(xs2, [128, L]))

        # ================= downsample =================
        pds = psD.tile([64, 256], F32, tag="ds", name="ds")
        invf = TB([1, BPC, L], "irow_f")
        dma(invf[:], inv[None, :, :])
        for s in range(BPC):
            invB = P512()
            nc.tensor.matmul(invB[:, 0:L], onesrow1, invf[:, s, :], start=True, stop=True)
            QT = TB([128, 2, 256], "perm_oh")
            for tt in range(2):
                V.tensor_scalar(QT[:, tt, :], invB[:, 0:L], ct['iotaC'][:, tt:tt + 1],
                                None, op0=ALU.is_equal)
            tmv = TB([128, 2, 128], "tm_tmp")
            for tt in range(2):
                ptr = PT()
                nc.tensor.transpose(ptr[:, 0:128], xs2[:, s, tt * 128:(tt + 1) * 128],
                                    ident[:])
                S.copy(tmv[:, tt, :], ptr[:, 0:128])
            pxr = P256()
            for tt in range(2):
                nc.tensor.matmul(pxr[:], tmv[:, tt, :], QT[:, tt, :],
                                 start=(tt == 0), stop=(tt == 1))
            xrp = TB([128, 324], "ds_xrp")
            G.memset(xrp[:], 0.0)
            xr3 = xrp[:].rearrange("p (h w) -> p h w", h=18)
            S.copy(xr3[:, 1:17, 1:17], pxr[:].rearrange("p (h w) -> p h w", h=16))
            for kh in range(3):
                for kw in range(3):
                    k = kh * 3 + kw
                    cmp_ = TB([128, 64], "ds_cmp")
                    (V.tensor_copy if k % 2 == 0 else S.copy)(
                        cmp_[:].rearrange("p (a b) -> p a b", a=8),
                        xr3[:, kh:kh + 16:2, kw:kw + 16:2])
                    nc.tensor.matmul(pds[:, s * 128:(s + 1) * 128],
                                     cmp_[:],
                                     wt['dsw_pk'][:, k, :],
                                     start=(k == 0), stop=(k == 8),
                                     skip_group_check=True)
        for s in range(BPC):
            view = pds[:, s * 128:(s + 1) * 128]
            mus = T([64, 1], "ds_mus")
            V.tensor_reduce(mus[:], view, axis=AX.X, op=ALU.add)
            mean = T([64, 1], "ds_mean")
            V.tensor_scalar(mean[:], mus[:], 1.0 / 128, None, op0=ALU.mult)
            sq = T([64, 128], "ds_sq")
            ss = T([64, 1], "ds_ss")
            S.activation(sq[:], view, AF.Square, accum_out=ss[:, 0:1])
            m2 = T([64, 1], "ds_m2")
            V.tensor_mul(m2[:], mean[:], mean[:])
            var = T([64, 1], "ds_var")
            V.scalar_tensor_tensor(var[:], ss[:], 1.0 / 128, m2[:],
                                   op0=ALU.mult, op1=ALU.subtract)
            lv = T([64, 1], "ds_lv")
            S.activation(lv[:], var[:], AF.Ln, bias=epscol[0:64, 0:1])
            rstd = T([64, 1], "ds_rstd")
            S.activation(rstd[:], lv[:], AF.Exp, scale=-0.5)
            xn = T([64, 128], "ds_xn")
            V.tensor_scalar(xn[:], view, mean[:, 0:1], rstd[:, 0:1],
                            op0=ALU.subtract, op1=ALU.mult)
            t1 = T([64, 128], "ds_t1")
            V.tensor_mul(t1[:], xn[:], wt['ds_ln_wB'][:])
            o1 = T([64, 128], "ds_o1")
            V.tensor_add(o1[:], t1[:], wt['ds_ln_bB'][:])
            dma(out[s].rearrange("h w c -> (h w) c"), o1[:])

        stk.close()
    from concourse.library_overlay import lower_extended_insts
    lower_extended_insts(nc)
    return nc, tap_t


# ---------------------------------------------------------------------------
_CACHE = {}


def _get_program(taps=()):
    key = tuple(sorted(taps))
    if key not in _CACHE:
        _CACHE[key] = build_program(taps)
    return _CACHE[key]


def make_inmaps(inputs, taps=()):
    cst = host_constants()
    w = prep_weights(inputs)
    blob_bf, blob_f32 = pack_blobs(cst, w)
    x = np.asarray(inputs['x'], np.float32).reshape(16, C, L)
    idx = np.asarray(inputs['sorted_index'], np.int32)
    inv = np.argsort(idx, axis=1, kind='stable').astype(np.int32)
    in_maps = []
    for c in range(NCORES):
        m = {'blob_bf': blob_bf, 'blob_f32': blob_f32}
        sl = slice(c * BPC, (c + 1) * BPC)
        m['x2'] = np.ascontiguousarray(x[sl]).astype(BF16NP)
        m['idx'] = np.ascontiguousarray(idx[sl].astype(np.float32)).astype(BF16NP)
        m['inv'] = np.ascontiguousarray(inv[sl].astype(np.float32)).astype(BF16NP)
        in_maps.append(m)
    return in_maps


def run(inputs, taps=(), trace=False):
    nc, tap_t = _get_program(taps)
    in_maps = make_inmaps(inputs, taps)
    res = run_bass_kernel_spmd(nc, in_maps, list(range(NCORES)), trace=trace)
    outs = np.concatenate([np.asarray(r['out'], np.float32) for r in res.results], axis=0)
    tapd = {}
    for name in taps:
        tapd[name] = [np.asarray(r.get('t_' + name), np.float32) for r in res.results]
    return outs, tapd, res


def kernel(**inputs):
    outs, _, _ = run(inputs)
    return outs


# revision 49
# speedup vs baseline: 1.0384x; 1.0070x over previous
"""Trainium2 Bass kernel for nn_Basic_Block_v1 (spatial/spectral Mamba2 block).

Sharding: data-parallel over batch (16 samples) across 8 NeuronCores,
2 samples per core; all parameters replicated. SSD scans are computed in
closed quadratic form on the TensorEngine. All heavy matmuls run in bf16
(1 cycle/row vs 4 for fp32); the cumulative-decay broadcast uses a bf16
hi/lo split to keep fp32-grade cancellation. LayerNorm scale/bias and the
gated-RMS weight are folded into adjacent projection weights on the host;
the Mamba D-residual is applied as a diagonal matmul accumulated into the
same PSUM as the SSD output.
"""
import sys
sys.path.insert(0, '/opt/trn_rl_repo')
import json

import numpy as np
import ml_dtypes

BF16NP = ml_dtypes.bfloat16

import concourse.bass as bass
import concourse.mybir as mybir
from concourse import tile
from concourse import bass_isa
from concourse.bass_utils import run_bass_kernel_spmd

F32 = mybir.dt.float32
BF = mybir.dt.bfloat16
I32 = mybir.dt.int32
AF = mybir.ActivationFunctionType
ALU = mybir.AluOpType
AX = mybir.AxisListType

NCORES = 8
BPC = 2          # batch per core
L = 256          # spatial tokens
C = 128          # channels
H1 = 4           # spa heads
H2 = 8           # spe heads
NST = 64         # d_state
EPS = 1e-5

# ---------------------------------------------------------------------------
# walrus in this container supports only ONE sync-wait per instruction;
# split extra waits emitted by the Tile scheduler onto preceding NoOps.
_WAIT_LIMIT = 1
_orig_to_json = bass.Bass.to_json_bytes


def _fix_block(b, ctr):
    insts = b.get('instructions')
    if insts:
        out = []
        for ins in insts:
            si = ins.get('sync_info')
            waits = (si or {}).get('on_wait') or []
            if len(waits) > _WAIT_LIMIT:
                while len(waits) > _WAIT_LIMIT:
                    chunk, waits = waits[:_WAIT_LIMIT], waits[_WAIT_LIMIT:]
                    ctr[0] += 1
                    out.append({
                        "debug": ins.get("debug"),
                        "engine": ins["engine"],
                        "ins": [],
                        "name": f"I-wsplit{ctr[0]}",
                        "opcode": "NoOp",
                        "outs": [],
                        "text_hint": "wsplit",
                        "sync_info": {"on_update": [], "on_wait": chunk},
                    })
                si['on_wait'] = waits
            out.append(ins)
        b['instructions'] = out
    for sb in b.get('blocks') or []:
        _fix_block(sb, ctr)


def _patched_to_json(self, *a, **k):
    raw = _orig_to_json(self, *a, **k)
    d = json.loads(raw)
    ctr = [0]
    for f in d.get('functions', []):
        for b in f.get('blocks', []):
            _fix_block(b, ctr)
    if ctr[0] == 0:
        return raw
    return json.dumps(d).encode()


bass.Bass.to_json_bytes = _patched_to_json


# ---------------------------------------------------------------------------
def _sincos_2d(dim, Hg):
    def e1(d, pos):
        omega = 1.0 / (10000.0 ** (np.arange(d // 2, dtype=np.float64) / (d / 2.0)))
        out = pos[:, None] * omega[None, :]
        return np.concatenate([np.sin(out), np.cos(out)], axis=-1)
    gh, gw = np.meshgrid(np.arange(Hg), np.arange(Hg), indexing='ij')
    emb = np.concatenate([e1(dim // 2, gh.reshape(-1)), e1(dim // 2, gw.reshape(-1))], axis=-1)
    return emb.astype(np.float32)


def host_constants():
    d = {}
    d['pe_fm'] = np.ascontiguousarray(_sincos_2d(C, 16).T).astype(BF16NP)   # [128, 256]
    d['ident'] = np.eye(128, dtype=np.float32).astype(BF16NP)
    d['ident32'] = np.eye(8, dtype=np.float32)
    iota = np.arange(L, dtype=np.float32)
    d['iotaC'] = np.stack([iota[:128], iota[128:]], axis=1).copy()          # [128, 2] f32
    sidx = np.arange(L)[:, None]
    tidx = np.arange(L)[None, :]
    m = (sidx <= tidx).astype(np.float32)
    d['maskT_spa'] = np.stack([m[:128], m[128:]], axis=1).copy().astype(BF16NP)
    s2 = np.arange(128)[:, None]
    t2 = np.arange(128)[None, :]
    d['maskT_spe'] = (s2 <= t2).astype(np.float32).astype(BF16NP)
    E1 = np.zeros((H1, 2, 128), np.float32)
    for j in range(2):
        for mm in range(128):
            E1[2 * j + mm // 64, j, mm] = 1.0
    d['E_spaJ'] = E1.astype(BF16NP)
    E2 = np.zeros((H2, 4, 128), np.float32)
    for j in range(4):
        for mm in range(128):
            E2[2 * j + mm // 64, j, mm] = 1.0
    d['E_speJ'] = E2.astype(BF16NP)
    EA = np.zeros((8, 128), np.float32)
    for h in range(8):
        EA[h, h * 16:(h + 1) * 16] = 1.0
    d['E_attn'] = EA.astype(BF16NP)
    d['Emask_q'] = EA.T.copy().astype(BF16NP)
    return d


def _col_order():
    cols = []
    for i in range(2):
        cols += [f"spa_dtb{i}", f"spa_negA{i}",
                 f"spa_cb{i}_0", f"spa_cb{i}_1", f"spa_cbBC{i}",
                 f"spa_zb{i}_0", f"spa_zb{i}_1", f"spa_xb{i}_0", f"spa_xb{i}_1",
                 f"spa_BCb{i}", f"spa_dpc{i}_0", f"spa_dpc{i}_1"]
    for i in range(2):
        cols += [f"spe_dtb{i}", f"spe_negA{i}"]
        cols += [f"spe_cb{i}_{j}" for j in range(4)] + [f"spe_cbBC{i}"]
        cols += [f"spe_zb{i}_{j}" for j in range(4)]
        cols += [f"spe_xb{i}_{j}" for j in range(4)]
        cols += [f"spe_BCb{i}"]
        cols += [f"spe_dpc{i}_{j}" for j in range(4)]
    cols += ["lnw_norm", "lnb_norm", "cprj_b", "aq_b", "ak_b", "av_b", "ao_b",
             "sq_b0", "sq_b1", "sk_b0", "sk_b1"]
    return cols


COL_ORDER = _col_order()
CIDX = {k: ix for ix, k in enumerate(COL_ORDER)}


def prep_weights(inp):
    """Host-side layout prep: bf16 casts, LN scale/bias folded into in_proj,
    rms weight folded into out_proj, D as diagonal matrices."""
    f32 = np.float32
    w = {}
    cols = {}
    # ---- spa in_proj with spa_ln fold ----
    w['spa_in_fold'] = np.zeros((2, 128, 644), BF16NP)
    for i in range(2):
        iw = np.asarray(inp['spa_in_w'][i], f32)                 # [644, 128]
        lw = np.asarray(inp['spa_ln_w'][i], f32)
        lb = np.asarray(inp['spa_ln_b'][i], f32)
        br = iw @ lb                                             # [644]
        w['spa_in_fold'][i] = (iw * lw[None, :]).T.astype(BF16NP)
        cols[f"spa_zb{i}_0"] = br[0:128]
        cols[f"spa_zb{i}_1"] = br[128:256]
        cols[f"spa_xb{i}_0"] = br[256:384]
        cols[f"spa_xb{i}_1"] = br[384:512]
        cols[f"spa_BCb{i}"] = br[512:640]
        cols[f"spa_dtb{i}"] = np.asarray(inp['spa_dt_bias'][i], f32) + br[640:644]
        cols[f"spa_negA{i}"] = -np.exp(np.asarray(inp['spa_A_log'][i], f32))
    cv = np.zeros((128, 2, 3, 4), f32)
    for i in range(2):
        cv[:, i, 0] = inp['spa_conv_w'][i, 0:128]
        cv[:, i, 1] = inp['spa_conv_w'][i, 128:256]
        cv[0:64, i, 2] = inp['spa_conv_w'][i, 256:320]
        cv[64:128, i, 2] = inp['spa_conv_w'][i, 320:384]
    w['spa_conv_pk'] = cv
    # out_proj with rms-weight fold: [feat, i, j, out]
    sow = np.transpose(np.asarray(inp['spa_out_w'], f32), (0, 2, 1)).reshape(2, 2, 128, 128)
    sow = sow * np.asarray(inp['spa_rms_w'], f32).reshape(2, 2, 128)[:, :, :, None]
    w['spa_out_pk'] = np.ascontiguousarray(sow.transpose(2, 0, 1, 3)).astype(BF16NP)
    # ---- spe in_proj with spe_ln fold ----
    w['spe_in_pk'] = np.zeros((2, 128, 2, 1160), BF16NP)
    for i in range(2):
        iw = np.asarray(inp['spe_in_w'][i], f32)                 # [1160, 256]
        lw = np.asarray(inp['spe_ln_w'][i], f32)
        lb = np.asarray(inp['spe_ln_b'][i], f32)
        br = iw @ lb
        iwf = (iw * lw[None, :]).T                               # [256, 1160]
        w['spe_in_pk'][i] = iwf.reshape(2, 128, 1160).transpose(1, 0, 2).astype(BF16NP)
        for j in range(4):
            cols[f"spe_zb{i}_{j}"] = br[j * 128:(j + 1) * 128]
            cols[f"spe_xb{i}_{j}"] = br[512 + j * 128:512 + (j + 1) * 128]
        cols[f"spe_BCb{i}"] = br[1024:1152]
        cols[f"spe_dtb{i}"] = np.asarray(inp['spe_dt_bias'][i], f32) + br[1152:1160]
        cols[f"spe_negA{i}"] = -np.exp(np.asarray(inp['spe_A_log'][i], f32))
    cv2 = np.zeros((128, 2, 5, 4), f32)
    for i in range(2):
        for j in range(4):
            cv2[:, i, j] = inp['spe_conv_w'][i, j * 128:(j + 1) * 128]
        cv2[0:64, i, 4] = inp['spe_conv_w'][i, 512:576]
        cv2[64:128, i, 4] = inp['spe_conv_w'][i, 576:640]
    w['spe_conv_pk'] = cv2
    sew = np.transpose(np.asarray(inp['spe_out_w'], f32), (0, 2, 1)).reshape(2, 4, 128, 256)
    sew = sew * np.asarray(inp['spe_rms_w'], f32).reshape(2, 4, 128)[:, :, :, None]
    w['spe_out_pk'] = np.ascontiguousarray(sew.transpose(0, 2, 1, 3)).astype(BF16NP)
    # ---- attention & tail ----
    w['cprj_pk'] = np.ascontiguousarray(
        np.transpose(np.asarray(inp['cprj_w'], f32), (2, 1, 0)).transpose(1, 0, 2)).astype(BF16NP)
    for nm in ('aq', 'ak', 'av', 'ao'):
        w[nm + 'T'] = np.ascontiguousarray(np.asarray(inp[nm + '_w'], f32).T).astype(BF16NP)
    for nm in ('sq', 'sk', 'sv', 'so'):
        wt_ = np.asarray(inp[nm + '_w'], f32).T.reshape(2, 128, 256)
        w[nm + 'T'] = np.ascontiguousarray(wt_.transpose(1, 0, 2)).astype(BF16NP)
    sqkb = np.zeros((128, 2, 2), f32)
    for ot in range(2):
        sqkb[:, 0, ot] = np.asarray(inp['sq_b'], f32)[ot * 128:(ot + 1) * 128]
        sqkb[:, 1, ot] = np.asarray(inp['sk_b'], f32)[ot * 128:(ot + 1) * 128]
    w['sqkb'] = sqkb
    w['svbB'] = np.ascontiguousarray(
        np.broadcast_to(np.asarray(inp['sv_b'], f32)[None, :], (128, 256))).astype(BF16NP)
    w['sobB'] = np.ascontiguousarray(
        np.broadcast_to(np.asarray(inp['so_b'], f32)[None, :], (128, 256))).astype(BF16NP)
    w['dsw_pk'] = np.ascontiguousarray(
        np.asarray(inp['ds_conv_w'], f32).reshape(9, 128, 128).transpose(1, 0, 2)).astype(BF16NP)
    w['ds_ln_wB'] = np.ascontiguousarray(
        np.broadcast_to(np.asarray(inp['ds_ln_w'], f32)[None, :], (64, 128)))
    w['ds_ln_bB'] = np.ascontiguousarray(
        np.broadcast_to(np.asarray(inp['ds_ln_b'], f32)[None, :], (64, 128)))
    # ---- small column-packed params (fp32 scalar operands) ----
    for i in range(2):
        cols[f"spa_cb{i}_0"] = inp['spa_conv_b'][i, 0:128]
        cols[f"spa_cb{i}_1"] = inp['spa_conv_b'][i, 128:256]
        cols[f"spa_cbBC{i}"] = inp['spa_conv_b'][i, 256:384]
        dpc = np.repeat(np.asarray(inp['spa_D'][i], f32), 64)
        cols[f"spa_dpc{i}_0"] = dpc[0:128]
        cols[f"spa_dpc{i}_1"] = dpc[128:256]
        dpc2 = np.repeat(np.asarray(inp['spe_D'][i], f32), 64)
        for j in range(4):
            cols[f"spe_dpc{i}_{j}"] = dpc2[j * 128:(j + 1) * 128]
        for j in range(4):
            cols[f"spe_cb{i}_{j}"] = inp['spe_conv_b'][i, j * 128:(j + 1) * 128]
        cols[f"spe_cbBC{i}"] = inp['spe_conv_b'][i, 512:640]
    cols["lnw_norm"] = inp['norm_w']
    cols["lnb_norm"] = inp['norm_b']
    cols["cprj_b"] = inp['cprj_b']
    for nm in ('aq', 'ak', 'av', 'ao'):
        cols[nm + "_b"] = inp[nm + '_b']
    cols["sq_b0"] = inp['sq_b'][0:128]
    cols["sq_b1"] = inp['sq_b'][128:256]
    cols["sk_b0"] = inp['sk_b'][0:128]
    cols["sk_b1"] = inp['sk_b'][128:256]
    pk = np.zeros((128, len(COL_ORDER)), f32)
    for k, v in cols.items():
        v = np.asarray(v, f32)
        pk[0:v.shape[0], CIDX[k]] = v
    w['colpak'] = pk
    return w


W_SHAPES = {
    'spa_in_fold': ([2, 128, 644], BF), 'spa_conv_pk': ([128, 2, 3, 4], F32),
    'spa_out_pk': ([128, 2, 2, 128], BF),
    'spe_in_pk': ([2, 128, 2, 1160], BF), 'spe_conv_pk': ([128, 2, 5, 4], F32),
    'spe_out_pk': ([2, 128, 4, 256], BF),
    'cprj_pk': ([128, 5, 128], BF),
    'aqT': ([128, 128], BF), 'akT': ([128, 128], BF), 'avT': ([128, 128], BF),
    'aoT': ([128, 128], BF),
    'sqT': ([128, 2, 256], BF), 'skT': ([128, 2, 256], BF), 'svT': ([128, 2, 256], BF),
    'soT': ([128, 2, 256], BF), 'svbB': ([128, 256], BF), 'sobB': ([128, 256], BF),
    'sqkb': ([128, 2, 2], F32),
    'dsw_pk': ([128, 9, 128], BF), 'ds_ln_wB': ([64, 128], F32), 'ds_ln_bB': ([64, 128], F32),
    'colpak': ([128, len(COL_ORDER)], F32),
}

CST_DT = {'pe_fm': BF, 'ident': BF, 'ident32': F32, 'iotaC': F32,
          'maskT_spa': BF, 'maskT_spe': BF, 'E_spaJ': BF, 'E_speJ': BF,
          'E_attn': BF, 'Emask_q': BF}

# ---- blob packing: all params as column ranges of two [128, N] blobs ----
BLOB_BF = [
    ('pe_fm', 128, [256]), ('ident', 128, [128]),
    ('maskT_spa', 128, [2, 256]), ('maskT_spe', 128, [128]),
    ('Emask_q', 128, [8]), ('E_spaJ', 4, [2, 128]), ('E_speJ', 8, [4, 128]),
    ('E_attn', 8, [128]),
    ('spa_in_fold', 128, [2, 644]),
    ('spa_out_pk', 128, [2, 2, 128]),
    ('spe_in_pk', 128, [2, 2, 1160]),
    ('spe_out_pk', 128, [2, 4, 256]),
    ('cprj_pk', 128, [5, 128]),
    ('aqT', 128, [128]), ('akT', 128, [128]), ('avT', 128, [128]),
    ('aoT', 128, [128]),
    ('sqT', 128, [2, 256]), ('skT', 128, [2, 256]), ('svT', 128, [2, 256]),
    ('soT', 128, [2, 256]), ('svbB', 128, [256]), ('sobB', 128, [256]),
    ('dsw_pk', 128, [9, 128]),
]
BLOB_F32 = [
    ('colpak', 128, [len(COL_ORDER)]),
    ('spa_conv_pk', 128, [2, 3, 4]), ('spe_conv_pk', 128, [2, 5, 4]),
    ('iotaC', 128, [2]), ('sqkb', 128, [2, 2]), ('ident32', 8, [8]),
    ('ds_ln_wB', 64, [128]), ('ds_ln_bB', 64, [128]),
]


def _blob_offsets(spec):
    offs = {}
    c = 0
    for name, _, vshape in spec:
        n = int(np.prod(vshape))
        offs[name] = (c, n)
        c += n
    return offs, c


BF_OFFS, BF_COLS = _blob_offsets(BLOB_BF)
F32_OFFS, F32_COLS = _blob_offsets(BLOB_F32)
_LAYER_MAJOR = {'spa_in_fold': (1, 0, 2), 'spe_in_pk': (1, 0, 2, 3),
                'spe_out_pk': (1, 0, 2, 3)}


def pack_blobs(cst, w):
    pool = dict(cst)
    pool.update(w)
    bf = np.zeros((128, BF_COLS), BF16NP)
    f32 = np.zeros((128, F32_COLS), np.float32)
    for spec, blob, offs in ((BLOB_BF, bf, BF_OFFS), (BLOB_F32, f32, F32_OFFS)):
        for name, rows, vshape in spec:
            a = np.asarray(pool[name])
            if name in _LAYER_MAJOR:
                a = np.transpose(a, _LAYER_MAJOR[name])
            off, n = offs[name]
            blob[0:rows, off:off + n] = a.reshape(rows, n)
    return bf, f32


# ---------------------------------------------------------------------------
def build_program(taps=()):
    nc = bass.Bass()

    def din(name, shape, dt=F32):
        return nc.dram_tensor(name, shape, dt, kind="ExternalInput")

    x2 = din("x2", [BPC, C, L], BF)
    idx = din("idx", [BPC, L], BF)
    inv = din("inv", [BPC, L], BF)

    blob_bf_t = din("blob_bf", [128, BF_COLS], BF)
    blob_f32_t = din("blob_f32", [128, F32_COLS], F32)

    out = nc.dram_tensor("out", [BPC, 8, 8, C], F32, kind="ExternalOutput")
    tap_t = {}

    with tile.TileContext(nc) as tc:
        import contextlib
        stk = contextlib.ExitStack()
        sb = stk.enter_context(tc.tile_pool(name="sb", bufs=1))
        ps1 = stk.enter_context(tc.tile_pool(name="ps1", bufs=3, space="PSUM"))
        ps2 = stk.enter_context(tc.tile_pool(name="ps2", bufs=4, space="PSUM"))
        psD = stk.enter_context(tc.tile_pool(name="psD", bufs=1, space="PSUM"))

        class _PSShim:
            def tile(self, shape, dt, tag="small", name="small"):
                return ps2.tile(shape, dt, tag="b256", name="ps_sm")

        psS = _PSShim()

        BUFS2 = {"cv_a0", "cv_a1", "rowA", "rowB", "rowC", "tm_tmp", "ssd_Dt",
                 "ssd_Et", "ssd_MT", "spa_xtm", "spe_xtm", "sq_tmp", "x2f_tmp",
                 "ssd_m0m", "spa_acumT", "spe_acumT", "spa_ygt", "spa_y0t",
                 "spa_ynt", "spe_ygt", "spe_y0t", "spe_ynt", "ds_cmp",
                 "spe_xn", "spe_h2sb", "sp2_a2T", "sp2_o2", "perm_oh", "spa_tsc",
                 "spa_rbs", "spe_rbs", "ds_mus", "ds_mean", "ds_sq", "ds_ss",
                 "ds_m2", "ds_var", "ds_lv", "ds_rstd", "ds_xn", "ds_t1", "ds_o1", "sp2_t3",
                 "ds_xrp", "xc_0", "xc_1", "xc_2", "xc_3", "xc_BC", "xc_C",
                 "cv_x0", "cv_x1", "cv_x2", "cv_x3", "cv_BC", "aflat", "cv_g0",
                 "cv_g1"}

        def T(shape, tag, dt=F32):
            return sb.tile(shape, dt, tag=tag, name=tag,
                           bufs=2 if tag in BUFS2 else 1)

        def TB(shape, tag):
            return T(shape, tag, BF)

        def P512(tag="b512"):
            return ps1.tile([128, 512], F32, tag="b512", name="b512")

        def P256(tag="b256"):
            return ps2.tile([128, 256], F32, tag="b256", name="b256")

        def PT(tag="bT"):
            return ps2.tile([128, 256], BF, tag="b256", name="bT")

        def tap(name, ap_fn):
            if name in taps:
                shape, writer, dt = ap_fn()
                t = nc.dram_tensor("t_" + name, shape, dt, kind="ExternalOutput")
                tap_t[name] = t
                writer(t)

        dma = nc.sync.dma_start
        V = nc.vector
        S = nc.scalar
        G = nc.gpsimd
        RO = bass_isa.ReduceOp

        # ---------- inputs first, then all params via two blobs ----------
        xb = TB([128, BPC, L], "xb")
        for s in range(BPC):
            dma(xb[:, s, :], x2[s])
        idxf = TB([1, BPC, L], "irow_f")
        dma(idxf[:], idx[None, :, :])
        blob_f32 = T([128, F32_COLS], "blob_f32")
        dma(blob_f32[:], blob_f32_t[:])
        blob_bf = TB([128, BF_COLS], "blob_bf")
        CH = 4096
        for c0 in range(0, BF_COLS, CH):
            c1 = min(c0 + CH, BF_COLS)
            dma(blob_bf[:, c0:c1], blob_bf_t[:, c0:c1])

        def _view(blob, offs, name, rows, vshape):
            off, n = offs[name]
            ap = blob[0:rows, off:off + n]
            if len(vshape) == 2:
                ap = ap.rearrange("p (a b) -> p a b", a=vshape[0])
            elif len(vshape) == 3:
                ap = ap.rearrange("p (a b c) -> p a b c", a=vshape[0], b=vshape[1])
            return ap

        ct = {}
        wt = {}
        for name, rows, vshape in BLOB_BF:
            v = _view(blob_bf, BF_OFFS, name, rows, vshape)
            (ct if name in CST_DT else wt)[name] = v
        for name, rows, vshape in BLOB_F32:
            v = _view(blob_f32, F32_OFFS, name, rows, vshape)
            (ct if name in CST_DT else wt)[name] = v
        colpak = wt['colpak']
        inw_l = [wt['spa_in_fold'][:, i, :] for i in range(2)]
        inw2_l = [wt['spe_in_pk'][:, i, :, :] for i in range(2)]
        ow2_l = [wt['spe_out_pk'][:, i, :, :] for i in range(2)]

        def col(key, p=128):
            return colpak[0:p, CIDX[key]:CIDX[key] + 1]

        ones4 = TB([128, 128], "ones4")
        V.memset(ones4[:], 1.0)
        epscol = T([128, 1], "epscol")
        V.memset(epscol[:], EPS)
        onescol = ones4[:, 0:1]       # [128,1] bf16
        onesrow1 = ones4[0:1, :]      # [1,128] bf16
        ones2 = ones4[0:2, :]         # [2,128] bf16
        ident = ct['ident']
        ident32 = ct['ident32']

        # ---------- stage 0: embed + permute ----------
        x0 = TB([128, BPC, L], "x0")
        V.tensor_tensor(
            x0[:], xb[:],
            ct['pe_fm'][:].unsqueeze(1).to_broadcast((128, BPC, L)),
            op=ALU.add)

        xs = TB([128, BPC, L], "xs")
        for s in range(BPC):
            idxB = P512()
            nc.tensor.matmul(idxB[:, 0:L], onesrow1, idxf[:, s, :], start=True, stop=True)
            PmT = TB([128, 2, L], "perm_oh")
            for st in range(2):
                V.tensor_scalar(PmT[:, st, :], idxB[:, 0:L], ct['iotaC'][:, st:st + 1],
                                None, op0=ALU.is_equal)
            x0tm = TB([128, 2, 128], "tm_tmp")
            for tt in range(2):
                ptr = PT()
                nc.tensor.transpose(ptr[:, 0:128], x0[:, s, tt * 128:(tt + 1) * 128], ident[:])
                S.copy(x0tm[:, tt, :], ptr[:, 0:128])
            pxs = P256()
            for st in range(2):
                nc.tensor.matmul(pxs[:], x0tm[:, st, :], PmT[:, st, :],
                                 start=(st == 0), stop=(st == 1))
            S.copy(xs[:, s, :], pxs[:])

        def tap_batched(t_sb, shape_per_s, dt=BF):
            def writer(dram):
                for s in range(BPC):
                    dma(dram[s], t_sb[:, s, :])
            return ([BPC] + shape_per_s, writer, dt)

        tap("xs0", lambda: tap_batched(xs, [128, L]))

        # ================= shared helpers =================
        def part_ln(xflat, final=False):
            """LayerNorm over the channel (partition) dim of [128, 512] bf16.
            Non-final: scale/bias folded downstream -> returns (x-mu)*rstd."""
            sq = TB([128, 512], "sq_tmp")
            S.activation(sq[:], xflat, AF.Square)
            msum = psS.tile([1, 512], F32)
            nc.tensor.matmul(msum[:], onescol, xflat, start=True, stop=True)
            ssum = psS.tile([1, 512], F32)
            nc.tensor.matmul(ssum[:], onescol, sq[:], start=True, stop=True)
            mu2 = T([1, 512], "rowA")
            S.activation(mu2[:], msum[:], AF.Square, scale=1.0 / 128)
            var = T([1, 512], "rowB")
            V.scalar_tensor_tensor(var[:], ssum[:], 1.0 / 128, mu2[:],
                                   op0=ALU.mult, op1=ALU.subtract)
            lnv = T([1, 512], "rowA")
            S.activation(lnv[:], var[:], AF.Ln, bias=epscol[0:1, 0:1])
            rstd = TB([1, 512], "ln_rstd")
            S.activation(rstd[:], lnv[:], AF.Exp, scale=-0.5)
            r0 = TB([1, 512], "ln_r0")
            V.scalar_tensor_tensor(r0[:], msum[:], -1.0 / 128, rstd[:],
                                   op0=ALU.mult, op1=ALU.mult)
            rstdB = P512()
            nc.tensor.matmul(rstdB[:], onesrow1, rstd[:], start=True, stop=True)
            r0B = P512()
            nc.tensor.matmul(r0B[:], onesrow1, r0[:], start=True, stop=True)
            tmp = TB([128, 512], "ln_tmp")
            V.tensor_tensor(tmp[:], xflat, rstdB[:], op=ALU.mult)
            xln = TB([128, 512], "ln_out")
            if final:
                xn = T([128, 512], "ln_xn")
                V.tensor_tensor(xn[:], tmp[:], r0B[:], op=ALU.add)
                S.activation(xln[:], xn[:], AF.Identity, bias=col("lnb_norm"),
                             scale=col("lnw_norm"))
            else:
                V.tensor_tensor(xln[:], tmp[:], r0B[:], op=ALU.add)
            return xln

        def convchain(buf, wc, cb, P, W, tag, E=None):
            """Causal depthwise conv (k=4) + silu. buf [P, 2, W+3] fp32 ->
            bf16 output. E selects the elementwise engine (vector/gpsimd)."""
            E = E or V
            a0 = T([P, 2, W], "cv_a0" if E is V else "cv_g0")
            E.tensor_scalar(a0[:], buf[:, :, 0:W], wc[:, 0:1], None, op0=ALU.mult)
            a1 = T([P, 2, W], "cv_a1" if E is V else "cv_g1")
            E.scalar_tensor_tensor(a1[:], buf[:, :, 1:W + 1], wc[:, 1:2], a0[:],
                                   op0=ALU.mult, op1=ALU.add)
            a2 = T([P, 2, W], "cv_a0" if E is V else "cv_g0")
            E.scalar_tensor_tensor(a2[:], buf[:, :, 2:W + 2], wc[:, 2:3], a1[:],
                                   op0=ALU.mult, op1=ALU.add)
            a3 = T([P, 2, W], "cv_a1" if E is V else "cv_g1")
            E.scalar_tensor_tensor(a3[:], buf[:, :, 3:W + 3], wc[:, 3:4], a2[:],
                                   op0=ALU.mult, op1=ALU.add)
            xc = TB([P, 2, W], tag)
            S.activation(xc[:], a3[:], AF.Silu, bias=cb[:, 0:1])
            return xc

        def dt_ladder(pdt, nh, NW, dtb_key, negA_key):
            """softplus(dt+bias) -> dtv_bf (matmul operand), acum f32,
            hi/lo bf16 rows for the decay broadcast."""
            e1 = T([nh, NW], "rowA")
            S.activation(e1[:], pdt[:], AF.Exp, bias=col(dtb_key, nh))
            e1p = T([nh, NW], "rowB")
            V.tensor_scalar(e1p[:], e1[:], 1.0, None, op0=ALU.add)
            dtv = T([nh, NW], "mb_dtv")
            S.activation(dtv[:], e1p[:], AF.Ln)

            dtv_bf = TB([nh, NW], "mb_dtvbf")
            S.copy(dtv_bf[:], dtv[:])
            dtA = T([nh, NW], "rowA")
            V.tensor_scalar(dtA[:], dtv[:], col(negA_key, nh), None, op0=ALU.mult)
            acum = T([nh, NW], "mb_acum")
            seg = NW // BPC
            for s in range(BPC):
                V.tensor_tensor_scan(acum[:, s * seg:(s + 1) * seg],
                                     dtA[:, s * seg:(s + 1) * seg],
                                     dtA[:, s * seg:(s + 1) * seg], 0.0,
                                     op0=ALU.add, op1=ALU.bypass)
            hi = TB([nh, NW], "acum_hi")
            S.copy(hi[:], acum[:])
            lo = TB([nh, NW], "acum_lo")
            G.tensor_tensor(lo[:], acum[:], hi[:], op=ALU.subtract)
            hilo = TB([2, BPC, 1024], "aflat")
            for s in range(BPC):
                dma(hilo[0:1, s, :].rearrange("o (p f) -> o p f", p=nh),
                    hi[:, s * seg:(s + 1) * seg])
                dma(hilo[1:2, s, :].rearrange("o (p f) -> o p f", p=nh),
                    lo[:, s * seg:(s + 1) * seg])
            return dtv_bf, acum, hilo

        # ================= spa mamba =================
        def spa_mamba(i, xs):
            xflat = xs[:].rearrange("p s t -> p (s t)")
            xln = part_ln(xflat)
            tap(f"xln{i}", lambda: ([128, 512], lambda d: dma(d[:], xln[:]), BF))
            inw = inw_l[i][:]
            # dt first: its Exp/Ln then run before the silu cluster
            pdt = psS.tile([4, 512], F32)
            nc.tensor.matmul(pdt[:], inw[:, 640:644], xln[:], start=True, stop=True)
            dtv_bf, acum, hilo = dt_ladder(pdt, 4, 512, f"spa_dtb{i}", f"spa_negA{i}")
            cvx = []
            for j in range(2):
                px = P512()
                nc.tensor.matmul(px[:], inw[:, 256 + j * 128:256 + (j + 1) * 128], xln[:],
                                 start=True, stop=True)
                buf = T([128, 2, 259], f"cv_x{j}")
                G.memset(buf[:, :, 0:3], 0.0)
                S.activation(buf[:, :, 3:259], px[:].rearrange("p (s t) -> p s t", s=2),
                             AF.Identity, bias=col(f"spa_xb{i}_{j}"))
                cvx.append(buf)
            # B and C merged: one matmul, one buf, one chain (B rows 0:63, C 64:127)
            pbc = P512()
            nc.tensor.matmul(pbc[:], inw[:, 512:640], xln[:], start=True, stop=True)
            bufBC = T([128, 2, 259], "cv_BC")
            G.memset(bufBC[:, :, 0:3], 0.0)
            S.activation(bufBC[:, :, 3:259], pbc[:].rearrange("p (s t) -> p s t", s=2),
                         AF.Identity, bias=col(f"spa_BCb{i}"))
            # conv + silu (silu table region)
            xc = []
            for j in range(2):
                xc.append(convchain(cvx[j], wt['spa_conv_pk'][:, i, j, :],
                                    col(f"spa_cb{i}_{j}"), 128, 256, f"xc_{j}"))
            xcBC = convchain(bufBC, wt['spa_conv_pk'][:, i, 2, :],
                             col(f"spa_cbBC{i}"), 128, 256, "xc_BC")
            # z -> silu emitted after convs so dt's Ln precedes the silu cluster
            zsil = TB([128, 2, 512], "mb_zsil")
            for j in range(2):
                pz = P512()
                nc.tensor.matmul(pz[:], inw[:, j * 128:(j + 1) * 128], xln[:],
                                 start=True, stop=True)
                S.activation(zsil[:, j, :], pz[:], AF.Silu, bias=col(f"spa_zb{i}_{j}"))
            # C half to a partition-0-based tile (matmul needs equal base partitions)
            xcC = TB([64, 2, 256], "xc_C")
            dma(xcC[:], xcBC[64:128, :, :])
            if i == 0:
                tap("dbg_zsil", lambda: ([128, 1024], lambda d: dma(
                    d[:], zsil[:].rearrange("p j t -> p (j t)")), BF))
                tap("dbg_xc0", lambda: ([128, 512], lambda d: dma(
                    d[:], xc[0][:].rearrange("p s t -> p (s t)")), BF))
                tap("dbg_xcBC", lambda: ([128, 512], lambda d: dma(
                    d[:], xcBC[:].rearrange("p s t -> p (s t)")), BF))
                tap("dbg_xcC", lambda: ([64, 512], lambda d: dma(
                    d[:], xcC[:].rearrange("p s t -> p (s t)")), BF))
                tap("dbg_dtv", lambda: ([4, 512], lambda d: dma(d[:], dtv_bf[:]), BF))
                tap("dbg_acum", lambda: ([4, 512], lambda d: dma(d[:], acum[:]), F32))
            # dt-scaled x (feature-major)
            xp = TB([128, 2, 512], "mb_xp")
            for j in range(2):
                pdb = P512()
                nc.tensor.matmul(pdb[:], ct['E_spaJ'][:, j, :], dtv_bf[:], start=True, stop=True)
                V.tensor_tensor(xp[:, j, :], xc[j][:].rearrange("p s t -> p (s t)"), pdb[:],
                                op=ALU.mult)
            if i == 0:
                tap("dbg_xp", lambda: ([128, 1024], lambda d: dma(
                    d[:], xp[:].rearrange("p j t -> p (j t)")), BF))
            h1 = TB([128, 2, 256], "h1")
            for s in range(BPC):
                xtm = TB([128, 2, 256], "spa_xtm")
                for st in range(2):
                    for j in range(2):
                        ptr = PT()
                        nc.tensor.transpose(
                            ptr[:, 0:128],
                            xp[:, j, s * 256 + st * 128: s * 256 + (st + 1) * 128],
                            ident[:])
                        V.tensor_copy(xtm[:, st, j * 128:(j + 1) * 128], ptr[:, 0:128])
                m0m = TB([128, 2, 256], "ssd_m0m")
                for st in range(2):
                    pm0 = P256()
                    nc.tensor.matmul(pm0[:], xcBC[0:64, s, st * 128:(st + 1) * 128],
                                     xcC[:, s, :], start=True, stop=True)
                    V.tensor_tensor(m0m[:, st, :], pm0[:], ct['maskT_spa'][:, st, :],
                                    op=ALU.mult)
                acumT = T([128, 2, 4], "spa_acumT")
                for tt in range(2):
                    ptr2 = P256()
                    nc.tensor.transpose(ptr2[:, 0:4],
                                        acum[:, s * 256 + tt * 128: s * 256 + (tt + 1) * 128],
                                        ident32[0:4, 0:4])
                    S.copy(acumT[:, tt, :], ptr2[:, 0:4])
                pb1 = P512()
                nc.tensor.matmul(pb1[:], ones2, hilo[:, s, 0:512], start=True, stop=True)
                pb2 = P512()
                nc.tensor.matmul(pb2[:], ones2, hilo[:, s, 512:1024], start=True, stop=True)
                yps = P512()
                for st in range(2):
                    Dt = T([128, 4, 256], "ssd_Dt")
                    for h in range(H1):
                        pbx = pb1 if h < 2 else pb2
                        V.tensor_scalar(Dt[:, h, :],
                                        pbx[:, (h % 2) * 256:(h % 2 + 1) * 256],
                                        acumT[:, st, h:h + 1], 0.0,
                                        op0=ALU.subtract, op1=ALU.min)
                    Et = TB([128, 4, 256], "ssd_Et")
                    S.activation(Et[:].rearrange("p h t -> p (h t)"),
                                 Dt[:].rearrange("p h t -> p (h t)"), AF.Exp)
                    MT = TB([128, 4, 256], "ssd_MT")
                    V.tensor_tensor(MT[:], Et[:],
                                    m0m[:, st, :].unsqueeze(1).to_broadcast((128, 4, 256)),
                                    op=ALU.mult)
                    for h in range(H1):
                        nc.tensor.matmul(
                            yps[(h % 2) * 64:(h % 2) * 64 + 64,
                                (h // 2) * 256:(h // 2) * 256 + 256],
                            xtm[:, st, h * 64:(h + 1) * 64],
                            MT[:, h, :],
                            start=(st == 0), stop=(st == 1),
                            tile_position=(0, (h % 2) * 64),
                            skip_group_check=True)
                if i == 0 and s == 0:
                    tap("dbg_xtm", lambda: ([128, 512], lambda d: dma(
                        d[:], xtm[:].rearrange("p s t -> p (s t)")), BF))
                    tap("dbg_m0m", lambda: ([128, 512], lambda d: dma(
                        d[:], m0m[:].rearrange("p s t -> p (s t)")), BF))
                    if "dbg_yps" in taps:
                        ypc = T([128, 512], "dbg_ypc")
                        S.copy(ypc[:], yps[:])
                        tap("dbg_yps", lambda: ([128, 512], lambda d: dma(
                            d[:], ypc[:]), F32))
                y0t = TB([128, 2, 256], "spa_y0t")
                for j in range(2):
                    V.scalar_tensor_tensor(y0t[:, j, :], xc[j][:, s, :],
                                           col(f"spa_dpc{i}_{j}"),
                                           yps[:, j * 256:(j + 1) * 256],
                                           op0=ALU.mult, op1=ALU.add)
                ygt = TB([128, 2, 256], "spa_ygt")
                V.tensor_tensor(ygt[:], y0t[:],
                                zsil[:, :, s * 256:(s + 1) * 256], op=ALU.mult)
                if i == 0 and s == 0:
                    tap("dbg_ygt", lambda: ([128, 512], lambda d: dma(
                        d[:], ygt[:].rearrange("p j t -> p (j t)")), BF))
                sqy = TB([128, 2, 256], "sq_tmp")
                S.activation(sqy[:].rearrange("p j t -> p (j t)"),
                             ygt[:].rearrange("p j t -> p (j t)"), AF.Square)
                ssy = psS.tile([1, 256], F32)
                for j in range(2):
                    nc.tensor.matmul(ssy[:], onescol, sqy[:, j, :],
                                     start=(j == 0), stop=(j == 1))
                rl = T([1, 256], "rowA")
                S.activation(rl[:], ssy[:], AF.Ln, bias=epscol[0:1, 0:1],
                             scale=1.0 / 256)
                rrow = TB([1, 256], "rowC")
                S.activation(rrow[:], rl[:], AF.Exp, scale=-0.5)
                pop = P256()
                for j in range(2):
                    nc.tensor.matmul(pop[:], wt['spa_out_pk'][:, i, j, :], ygt[:, j, :],
                                     start=(j == 0), stop=(j == 1))
                rB = P256()
                nc.tensor.matmul(rB[:], onesrow1, rrow[:], start=True, stop=True)
                # rms scale is per-token -> commutes with the linear out-proj
                rBs = TB([128, 256], "spa_rbs")
                S.copy(rBs[:], rB[:])
                tsc = TB([128, 256], "spa_tsc")
                V.tensor_tensor(tsc[:], pop[:], rBs[:], op=ALU.mult)
                V.tensor_tensor(h1[:, s, :], tsc[:], xs[:, s, :], op=ALU.add)
            return h1

        # ================= spe mamba =================
        def spe_mamba(i, h1):
            mus = T([128, 2], "spe_mus")
            V.tensor_reduce(mus[:], h1[:], axis=AX.X, op=ALU.add)
            sqd = TB([128, 2, 256], "sq_tmp")
            ss2 = T([128, 2], "spe_ss2")
            for s in range(BPC):
                S.activation(sqd[:, s, :], h1[:, s, :], AF.Square,
                             accum_out=ss2[:, s:s + 1])
            mean = T([128, 2], "spe_mean")
            V.tensor_scalar(mean[:], mus[:], 1.0 / 256, None, op0=ALU.mult)
            m2 = T([128, 2], "spe_m2")
            S.activation(m2[:], mean[:], AF.Square)
            var2 = T([128, 2], "spe_var")
            V.scalar_tensor_tensor(var2[:], ss2[:], 1.0 / 256, m2[:],
                                   op0=ALU.mult, op1=ALU.subtract)
            l2t = T([128, 2], "spe_l2")
            S.activation(l2t[:], var2[:], AF.Ln, bias=epscol[:, 0:1])
            rstd2 = T([128, 2], "spe_rstd")
            S.activation(rstd2[:], l2t[:], AF.Exp, scale=-0.5)
            X2f = TB([128, 2, 2, 128], "x2f_tmp")
            for s in range(BPC):
                xn = TB([128, 256], "spe_xn")
                V.tensor_scalar(xn[:], h1[:, s, :], mean[:, s:s + 1], rstd2[:, s:s + 1],
                                op0=ALU.subtract, op1=ALU.mult)
                for ft in range(2):
                    ptr = PT()
                    nc.tensor.transpose(ptr[:, 0:128], xn[:, ft * 128:(ft + 1) * 128],
                                        ident[:])
                    V.tensor_copy(X2f[:, s, ft, :], ptr[:, 0:128])
            inw2 = inw2_l[i][:]
            ow2 = ow2_l[i][:]

            def mm2(out_ap, off, width):
                for k in range(2):
                    nc.tensor.matmul(out_ap,
                                     inw2[:, k, off:off + width],
                                     X2f[:, :, k, :],
                                     start=(k == 0), stop=(k == 1))
            # dt first (exp/ln before the silu cluster)
            pdt = psS.tile([8, 256], F32)
            mm2(pdt[:], 1152, 8)
            dtv_bf, acum, hilo = dt_ladder(pdt, 8, 256, f"spe_dtb{i}", f"spe_negA{i}")
            cvx2 = []
            for j in range(4):
                px = P256()
                mm2(px[:], 512 + j * 128, 128)
                buf = T([128, 2, 131], f"cv_x{j}")
                G.memset(buf[:, :, 0:3], 0.0)
                S.activation(buf[:, :, 3:131], px[:].rearrange("p (s t) -> p s t", s=2),
                             AF.Identity, bias=col(f"spe_xb{i}_{j}"))
                cvx2.append(buf)
            pbc = P256()
            mm2(pbc[:], 1024, 128)
            bufBC = T([128, 2, 131], "cv_BC")
            G.memset(bufBC[:, :, 0:3], 0.0)
            S.activation(bufBC[:, :, 3:131], pbc[:].rearrange("p (s t) -> p s t", s=2),
                         AF.Identity, bias=col(f"spe_BCb{i}"))
            xc2 = []
            for j in range(4):
                xc2.append(convchain(cvx2[j], wt['spe_conv_pk'][:, i, j, :],
                                     col(f"spe_cb{i}_{j}"), 128, 128, f"xc_{j}"))
            xcBC = convchain(bufBC, wt['spe_conv_pk'][:, i, 4, :],
                             col(f"spe_cbBC{i}"), 128, 128, "xc_BC")
            z2sil = TB([128, 4, 256], "mb_zsil")
            for j in range(4):
                pz = P256()
                mm2(pz[:], j * 128, 128)
                S.activation(z2sil[:, j, :], pz[:], AF.Silu, bias=col(f"spe_zb{i}_{j}"))
            xcC = TB([64, 2, 128], "xc_C")
            dma(xcC[:], xcBC[64:128, :, :])
            xp2 = TB([128, 4, 256], "mb_xp")
            for j in range(4):
                pdb = P256()
                nc.tensor.matmul(pdb[:], ct['E_speJ'][:, j, :], dtv_bf[:], start=True, stop=True)
                V.tensor_tensor(xp2[:, j, :], xc2[j][:].rearrange("p s t -> p (s t)"), pdb[:],
                                op=ALU.mult)
            xs_new = TB([128, 2, 256], "xs")
            for s in range(BPC):
                xtm2 = TB([128, 512], "spe_xtm")
                for j in range(4):
                    ptr = PT()
                    nc.tensor.transpose(ptr[:, 0:128],
                                        xp2[:, j, s * 128:(s + 1) * 128], ident[:])
                    V.tensor_copy(xtm2[:, j * 128:(j + 1) * 128], ptr[:, 0:128])
                m0m2 = TB([128, 128], "ssd_m0m")
                pm0 = P256()
                nc.tensor.matmul(pm0[:, 0:128], xcBC[0:64, s, :], xcC[:, s, :],
                                 start=True, stop=True)
                V.tensor_tensor(m0m2[:], pm0[:, 0:128], ct['maskT_spe'][:], op=ALU.mult)
                acumT = T([128, 8], "spe_acumT")
                ptr2 = P256()
                nc.tensor.transpose(ptr2[:, 0:8], acum[:, s * 128:(s + 1) * 128],
                                    ident32[0:8, 0:8])
                S.copy(acumT[:], ptr2[:, 0:8])
                pb1 = P512()
                nc.tensor.matmul(pb1[:], ones2, hilo[:, s, 0:512], start=True, stop=True)
                pb2 = P512()
                nc.tensor.matmul(pb2[:], ones2, hilo[:, s, 512:1024], start=True, stop=True)
                yps = P512()
                Dt = T([128, 8, 128], "ssd_Dt")
                for h in range(H2):
                    pbx = pb1 if h < 4 else pb2
                    V.tensor_scalar(Dt[:, h, :],
                                    pbx[:, (h % 4) * 128:(h % 4 + 1) * 128],
                                    acumT[:, h:h + 1], 0.0,
                                    op0=ALU.subtract, op1=ALU.min)
                Et = TB([128, 8, 128], "ssd_Et")
                S.activation(Et[:].rearrange("p h t -> p (h t)"),
                             Dt[:].rearrange("p h t -> p (h t)"), AF.Exp)
                MT = TB([128, 8, 128], "ssd_MT")
                V.tensor_tensor(MT[:], Et[:],
                                m0m2[:].unsqueeze(1).to_broadcast((128, 8, 128)),
                                op=ALU.mult)
                for j in range(4):
                    for hh in range(2):
                        h = 2 * j + hh
                        nc.tensor.matmul(yps[hh * 64:hh * 64 + 64, j * 128:(j + 1) * 128],
                                         xtm2[:, h * 64:(h + 1) * 64],
                                         MT[:, h, :], start=True, stop=True,
                                         tile_position=(0, hh * 64),
                                         skip_group_check=True)
                y0t2 = TB([128, 4, 128], "spe_y0t")
                for j in range(4):
                    V.scalar_tensor_tensor(y0t2[:, j, :], xc2[j][:, s, :],
                                           col(f"spe_dpc{i}_{j}"),
                                           yps[:, j * 128:(j + 1) * 128],
                                           op0=ALU.mult, op1=ALU.add)
                ygt2 = TB([128, 4, 128], "spe_ygt")
                V.tensor_tensor(ygt2[:], y0t2[:],
                                z2sil[:, :, s * 128:(s + 1) * 128], op=ALU.mult)
                sqy = TB([128, 4, 128], "sq_tmp")
                S.activation(sqy[:].rearrange("p j t -> p (j t)"),
                             ygt2[:].rearrange("p j t -> p (j t)"), AF.Square)
                ssy = psS.tile([1, 128], F32)
                for j in range(4):
                    nc.tensor.matmul(ssy[:], onescol, sqy[:, j, :],
                                     start=(j == 0), stop=(j == 3))
                rl = T([1, 128], "rowA")
                S.activation(rl[:], ssy[:], AF.Ln, bias=epscol[0:1, 0:1],
                             scale=1.0 / 512)
                rrow = TB([1, 128], "rowC")
                S.activation(rrow[:], rl[:], AF.Exp, scale=-0.5)
                rB = P256()
                nc.tensor.matmul(rB[:, 0:128], onesrow1, rrow[:], start=True, stop=True)
                rBs2 = TB([128, 128], "spe_rbs")
                S.copy(rBs2[:], rB[:, 0:128])
                for ft in range(2):
                    ph2 = P256()
                    for k in range(4):
                        nc.tensor.matmul(ph2[:, 0:128],
                                         ow2[:, k, ft * 128:(ft + 1) * 128],
                                         ygt2[:, k, :], start=(k == 0), stop=(k == 3))
                    h2sb = TB([128, 128], "spe_h2sb")
                    V.tensor_tensor(h2sb[:], ph2[:, 0:128], rBs2[:], op=ALU.mult)
                    ptr = PT()
                    nc.tensor.transpose(ptr[:, 0:128], h2sb[:], ident[:])
                    V.tensor_tensor(xs_new[:, s, ft * 128:(ft + 1) * 128], ptr[:, 0:128],
                                    h1[:, s, ft * 128:(ft + 1) * 128], op=ALU.add)
            return xs_new

        # ================= layers =================
        cur = xs
        for i in range(2):
            h1 = spa_mamba(i, cur)
            tap(f"h1_{i}", lambda: tap_batched(h1, [128, L]))
            cur = spe_mamba(i, h1)
            tap(f"xsl{i + 1}", lambda: tap_batched(cur, [128, L]))

        # ================= final LN =================
        xfl = part_ln(cur[:].rearrange("p s t -> p (s t)"), final=True)
        xf = xfl[:].rearrange("p (s t) -> p s t", s=BPC)
        tap("xf", lambda: ([BPC, 128, L],
                           lambda d: [dma(d[s], xf[:, s, :]) for s in range(BPC)], BF))

        # ================= spa attention (center query) =================
        pctr = psS.tile([128, 2], F32)
        for l in range(5):
            nc.tensor.matmul(pctr[:], wt['cprj_pk'][:, l, :], xf[:, :, l],
                             start=(l == 0), stop=(l == 4))
        ctr = TB([128, 2], "at_ctr")
        S.activation(ctr[:], pctr[:], AF.Identity, bias=col("cprj_b"))
        pq = psS.tile([128, 2], F32)
        nc.tensor.matmul(pq[:], wt['aqT'][:], ctr[:], start=True, stop=True)
        qsb = TB([128, 2], "at_q")
        S.activation(qsb[:], pq[:], AF.Identity, bias=col("aq_b"))
        pk = P512()
        nc.tensor.matmul(pk[:], wt['akT'][:], xfl[:], start=True, stop=True)
        Ksb = TB([128, 2, 256], "at_K")
        S.activation(Ksb[:].rearrange("p s t -> p (s t)"), pk[:], AF.Identity,
                     bias=col("ak_b"))
        pv = P512()
        nc.tensor.matmul(pv[:], wt['avT'][:], xfl[:], start=True, stop=True)
        Vsb = TB([128, 2, 256], "at_V")
        S.activation(Vsb[:].rearrange("p s t -> p (s t)"), pv[:], AF.Identity,
                     bias=col("av_b"))
        # batched softmax over both samples (per-head global max is a valid
        # stabilizer; softmax itself stays per-(head,sample))
        plg2 = psS.tile([8, 2, 256], F32)
        for s in range(BPC):
            qd = TB([128, 8], "at_qd")
            V.tensor_tensor(qd[:], qsb[:, s:s + 1].to_broadcast((128, 8)),
                            ct['Emask_q'][:], op=ALU.mult)
            nc.tensor.matmul(plg2[:, s, :], qd[:], Ksb[:, s, :], start=True, stop=True,
                             skip_group_check=True)
        nm = T([8, 1], "at_nm")
        V.tensor_reduce(nm[:], plg2[:].rearrange("p s t -> p (s t)"),
                        axis=AX.X, op=ALU.max, negate=True)
        nm4 = T([8, 1], "at_nm4")
        V.tensor_scalar(nm4[:], nm[:], 0.25, None, op0=ALU.mult)
        ex = T([8, 2, 256], "at_ex")
        S.activation(ex[:].rearrange("p s t -> p (s t)"),
                     plg2[:].rearrange("p s t -> p (s t)"),
                     AF.Exp, bias=nm4[:, 0:1], scale=0.25)
        sm = T([8, 2], "at_sm")
        V.tensor_reduce(sm[:], ex[:], axis=AX.X, op=ALU.add)
        rc = T([8, 2], "at_rc")
        V.reciprocal(rc[:], sm[:])
        aw = TB([8, 2, 256], "at_aw")
        V.tensor_tensor(aw[:], ex[:], rc[:].unsqueeze(2).to_broadcast((8, 2, 256)),
                        op=ALU.mult)
        patB = P512()
        nc.tensor.matmul(patB[:], ct['E_attn'][:], aw[:].rearrange("p s t -> p (s t)"),
                         start=True, stop=True)
        vo = TB([128, 2, 256], "at_vo")
        V.tensor_tensor(vo[:].rearrange("p s t -> p (s t)"),
                        Vsb[:].rearrange("p s t -> p (s t)"), patB[:], op=ALU.mult)
        pao = P512()
        nc.tensor.matmul(pao[:], wt['aoT'][:], vo[:].rearrange("p s t -> p (s t)"),
                         start=True, stop=True)
        xa = TB([128, 2, 256], "xa")
        V.scalar_tensor_tensor(xa[:].rearrange("p s t -> p (s t)"), pao[:],
                               col("ao_b"), xfl[:], op0=ALU.add, op1=ALU.add)
        tap("xa", lambda: tap_batched(xa, [128, L]))

        # ================= spe attention =================
        X2a = TB([128, 2, 2, 128], "x2f_tmp")
        for s in range(BPC):
            for ft in range(2):
                ptr = PT()
                nc.tensor.transpose(ptr[:, 0:128], xa[:, s, ft * 128:(ft + 1) * 128],
                                    ident[:])
                S.copy(X2a[:, s, ft, :], ptr[:, 0:128])
        q2 = TB([128, 2, 2, 128], "sp2_q2")
        k2 = TB([128, 2, 2, 128], "sp2_k2")
        pq2b = P512()
        pk2b = P512()
        for s in range(BPC):
            for ot in range(2):
                for ft in range(2):
                    nc.tensor.matmul(pq2b[:, s * 256 + ot * 128:s * 256 + (ot + 1) * 128],
                                     wt['sqT'][:, ft, ot * 128:(ot + 1) * 128],
                                     X2a[:, s, ft, :], start=(ft == 0), stop=(ft == 1),
                                     skip_group_check=True)
                    nc.tensor.matmul(pk2b[:, s * 256 + ot * 128:s * 256 + (ot + 1) * 128],
                                     wt['skT'][:, ft, ot * 128:(ot + 1) * 128],
                                     X2a[:, s, ft, :], start=(ft == 0), stop=(ft == 1),
                                     skip_group_check=True)
        V.tensor_tensor(q2[:], pq2b[:].rearrange("p (s o c) -> p s o c", s=2, o=2),
                        wt['sqkb'][:, 0].unsqueeze(1).unsqueeze(3)
                        .to_broadcast((128, 2, 2, 128)), op=ALU.add)
        V.tensor_tensor(k2[:], pk2b[:].rearrange("p (s o c) -> p s o c", s=2, o=2),
                        wt['sqkb'][:, 1].unsqueeze(1).unsqueeze(3)
                        .to_broadcast((128, 2, 2, 128)), op=ALU.add)
        # batched v2 / logits / softmax over both samples
        pv2b = P512()
        for s in range(BPC):
            for ft in range(2):
                nc.tensor.matmul(pv2b[:, s * 256:(s + 1) * 256],
                                 X2a[:, s, ft, :], wt['svT'][:, ft, :],
                                 start=(ft == 0), stop=(ft == 1),
                                 skip_group_check=True)
        v2b = TB([128, 2, 256], "sp2_v2")
        V.tensor_tensor(v2b[:], pv2b[:].rearrange("p (s t) -> p s t", s=2),
                        wt['svbB'][:].unsqueeze(1).to_broadcast((128, 2, 256)),
                        op=ALU.add)
        pa2b = P256()
        for s in range(BPC):
            for ot in range(2):
                nc.tensor.matmul(pa2b[:, s * 128:(s + 1) * 128],
                                 q2[:, s, ot, :], k2[:, s, ot, :],
                                 start=(ot == 0), stop=(ot == 1),
                                 skip_group_check=True)
        nm2 = T([128, 1], "sp2_nm")
        V.tensor_reduce(nm2[:], pa2b[:], axis=AX.X, op=ALU.max, negate=True)
        nm16 = T([128, 1], "sp2_nm16")
        V.tensor_scalar(nm16[:], nm2[:], 1.0 / 16, None, op0=ALU.mult)
        ex2 = TB([128, 2, 128], "sp2_ex")
        S.activation(ex2[:].rearrange("p s t -> p (s t)"), pa2b[:],
                     AF.Exp, bias=nm16[:, 0:1], scale=1.0 / 16)
        sm2 = T([128, 2], "sp2_sm")
        V.tensor_reduce(sm2[:], ex2[:], axis=AX.X, op=ALU.add)
        rc2 = T([128, 2], "sp2_rc")
        V.reciprocal(rc2[:], sm2[:])
        po3b = P512()
        for s in range(BPC):
            pa2T = PT()
            nc.tensor.transpose(pa2T[:, 0:128], ex2[:, s, :], ident[:])
            a2T = TB([128, 128], "sp2_a2T")
            S.copy(a2T[:], pa2T[:, 0:128])
            o2 = TB([128, 2, 128], "sp2_o2")
            for ot in range(2):
                po2 = P256()
                nc.tensor.matmul(po2[:, 0:128], v2b[:, s, ot * 128:(ot + 1) * 128], a2T[:],
                                 start=True, stop=True)
                S.copy(o2[:, ot, :], po2[:, 0:128])
            for ot in range(2):
                nc.tensor.matmul(po3b[:, s * 256:(s + 1) * 256],
                                 o2[:, ot, :], wt['soT'][:, ot, :],
                                 start=(ot == 0), stop=(ot == 1),
                                 skip_group_check=True)
        xs2 = TB([128, 2, 256], "xs2")
        for s in range(BPC):
            t3s = TB([128, 256], "sp2_t3")
            V.scalar_tensor_tensor(t3s[:], po3b[:, s * 256:(s + 1) * 256],
                                   rc2[:, s:s + 1], wt['sobB'],
                                   op0=ALU.mult, op1=ALU.add)
            V.tensor_tensor(xs2[:, s, :], t3s[:], xa[:, s, :], op=ALU.add)
        tap("xs2", lambda: tap_batched(xs2, [128, L]))

        # ================= downsample =================
        pds = psD.tile([64, 256], F32, tag="ds", name="ds")
        invf = TB([1, BPC, L], "irow_f")
        dma(invf[:], inv[None, :, :])
        for s in range(BPC):
            invB = P512()
            nc.tensor.matmul(invB[:, 0:L], onesrow1, invf[:, s, :], start=True, stop=True)
            QT = TB([128, 2, 256], "perm_oh")
            for tt in range(2):
                V.tensor_scalar(QT[:, tt, :], invB[:, 0:L], ct['iotaC'][:, tt:tt + 1],
                                None, op0=ALU.is_equal)
            tmv = TB([128, 2, 128], "tm_tmp")
            for tt in range(2):
                ptr = PT()
                nc.tensor.transpose(ptr[:, 0:128], xs2[:, s, tt * 128:(tt + 1) * 128],
                                    ident[:])
                S.copy(tmv[:, tt, :], ptr[:, 0:128])
            pxr = P256()
            for tt in range(2):
                nc.tensor.matmul(pxr[:], tmv[:, tt, :], QT[:, tt, :],
                                 start=(tt == 0), stop=(tt == 1))
            xrp = TB([128, 324], "ds_xrp")
            G.memset(xrp[:], 0.0)
            xr3 = xrp[:].rearrange("p (h w) -> p h w", h=18)
            S.copy(xr3[:, 1:17, 1:17], pxr[:].rearrange("p (h w) -> p h w", h=16))
            for kh in range(3):
                for kw in range(3):
                    k = kh * 3 + kw
                    cmp_ = TB([128, 64], "ds_cmp")
                    (V.tensor_copy if k % 2 == 0 else S.copy)(
                        cmp_[:].rearrange("p (a b) -> p a b", a=8),
                        xr3[:, kh:kh + 16:2, kw:kw + 16:2])
                    nc.tensor.matmul(pds[:, s * 128:(s + 1) * 128],
                                     cmp_[:],
                                     wt['dsw_pk'][:, k, :],
                                     start=(k == 0), stop=(k == 8),
                                     skip_group_check=True)
        for s in range(BPC):
            view = pds[:, s * 128:(s + 1) * 128]
            mus = T([64, 1], "ds_mus")
            V.tensor_reduce(mus[:], view, axis=AX.X, op=ALU.add)
            mean = T([64, 1], "ds_mean")
            V.tensor_scalar(mean[:], mus[:], 1.0 / 128, None, op0=ALU.mult)
            sq = T([64, 128], "ds_sq")
            ss = T([64, 1], "ds_ss")
            S.activation(sq[:], view, AF.Square, accum_out=ss[:, 0:1])
            m2 = T([64, 1], "ds_m2")
            V.tensor_mul(m2[:], mean[:], mean[:])
            var = T([64, 1], "ds_var")
            V.scalar_tensor_tensor(var[:], ss[:], 1.0 / 128, m2[:],
                                   op0=ALU.mult, op1=ALU.subtract)
            lv = T([64, 1], "ds_lv")
            S.activation(lv[:], var[:], AF.Ln, bias=epscol[0:64, 0:1])
            rstd = T([64, 1], "ds_rstd")
            S.activation(rstd[:], lv[:], AF.Exp, scale=-0.5)
            xn = T([64, 128], "ds_xn")
            V.tensor_scalar(xn[:], view, mean[:, 0:1], rstd[:, 0:1],
                            op0=ALU.subtract, op1=ALU.mult)
            t1 = T([64, 128], "ds_t1")
            V.tensor_mul(t1[:], xn[:], wt['ds_ln_wB'][:])
            o1 = T([64, 128], "ds_o1")
            V.tensor_add(o1[:], t1[:], wt['ds_ln_bB'][:])
            dma(out[s].rearrange("h w c -> (h w) c"), o1[:])

        stk.close()
    from concourse.library_overlay import lower_extended_insts
    lower_extended_insts(nc)
    return nc, tap_t


# ---------------------------------------------------------------------------
_CACHE = {}


def _get_program(taps=()):
    key = tuple(sorted(taps))
    if key not in _CACHE:
        _CACHE[key] = build_program(taps)
    return _CACHE[key]


def make_inmaps(inputs, taps=()):
    cst = host_constants()
    w = prep_weights(inputs)
    blob_bf, blob_f32 = pack_blobs(cst, w)
    x = np.asarray(inputs['x'], np.float32).reshape(16, C, L)
    idx = np.asarray(inputs['sorted_index'], np.int32)
    inv = np.argsort(idx, axis=1, kind='stable').astype(np.int32)
    in_maps = []
    for c in range(NCORES):
        m = {'blob_bf': blob_bf, 'blob_f32': blob_f32}
        sl = slice(c * BPC, (c + 1) * BPC)
        m['x2'] = np.ascontiguousarray(x[sl]).astype(BF16NP)
        m['idx'] = np.ascontiguousarray(idx[sl].astype(np.float32)).astype(BF16NP)
        m['inv'] = np.ascontiguousarray(inv[sl].astype(np.float32)).astype(BF16NP)
        in_maps.append(m)
    return in_maps


def run(inputs, taps=(), trace=False):
    nc, tap_t = _get_program(taps)
    in_maps = make_inmaps(inputs, taps)
    res = run_bass_kernel_spmd(nc, in_maps, list(range(NCORES)), trace=trace)
    outs = np.concatenate([np.asarray(r['out'], np.float32) for r in res.results], axis=0)
    tapd = {}
    for name in taps:
        tapd[name] = [np.asarray(r.get('t_' + name), np.float32) for r in res.results]
    return outs, tapd, res


def kernel(**inputs):
    outs, _, _ = run(inputs)
    return outs
